# revision 1
# baseline (speedup 1.0000x reference)
"""HNHN hypergraph model on 8 Trainium2 NeuronCores (Bass/Tile).

Sharding: rows (nodes) of the incidence matrix and x0 over 8 cores.
Pipeline per core (v = own 1024 nodes, e = all 16384 hyperedges):
  P0   : stream B fp32 -> row sums (node_deg), cast bf16 -> DRAM scratch B16
  S0   : Y0 = x0 @ W01_0, S0 = [Y0*v_beta | v_beta | 1]  (hi/lo bf16 split)
  PA   : U0' = B^T S0 partials (PSUM accum over own v)  -> ReduceScatter(add)
         U0'[64] = beta_denom partials, U0'[65] = edge_card partials
  mid0 : per e-shard: x1 = relu(U0/beta_denom + b01_0); Z0' = [x1@W10_0*e_a | e_a]
         hi/lo bf16 -> AllGather
  PB   : V0'^T = Z0'^T B^T via transposed-DMA reads of B16 (accum over all e)
         row 64 = alpha_denom (local);  x = relu(V0/alpha_denom + b10_0)
  PC   : layer-1 edge pass (same as PA, no extra cols) -> ReduceScatter
  mid1 : x1_l2 = relu(U1/beta_denom + b01_1); Z1' hi/lo -> AllGather
  PD   : V1^T accum;  x2 = relu(V1/alpha_denom + b10_1)
  fin  : per-core max-pool -> AllReduce(max) -> pooled @ Wout + bout
"""

import numpy as np

import concourse.bass as bass
import concourse.bacc as bacc
import concourse.mybir as mybir
import concourse.tile as tile
from concourse.bass_utils import run_bass_kernel_spmd
from concourse.masks import make_identity

F32 = mybir.dt.float32
BF16 = mybir.dt.bfloat16
AF = mybir.ActivationFunctionType
OP = mybir.AluOpType

N, E, D, H = 8192, 16384, 128, 64
NCORES = 8


def _hi_lo(nc, pool, src_f32, dst_hi, dst_lo, tmp_shape):
    """Split fp32 tile into hi+lo bf16 (dst_hi + dst_lo ~= src exactly)."""
    nc.vector.tensor_copy(out=dst_hi, in_=src_f32)
    hf = pool.tile(tmp_shape, F32, tag="hilo_hf")
    nc.vector.tensor_copy(out=hf, in_=dst_hi)
    lof = pool.tile(tmp_shape, F32, tag="hilo_lof")
    nc.vector.tensor_sub(out=lof, in0=src_f32, in1=hf)
    nc.vector.tensor_copy(out=dst_lo, in_=lof)


def build_kernel(ncores=NCORES, n_edges=E, nloc=N // NCORES):
    EE = n_edges
    NVT = nloc // 128            # v-tiles per core
    ESH = EE // ncores           # e-shard per core
    NET = EE // 128              # 128-wide e-tiles
    PASUP = min(2048, EE)        # PA/PC streaming super width
    NSUP = EE // PASUP
    TSUP = min(1024, EE)         # PB/PD transposed-read super width
    NTSUP = EE // TSUP
    ETL = TSUP // 128            # e-tiles per transposed read
    CW = min(512, nloc)          # column chunk for nloc-wide ops
    NCH = nloc // CW
    FW = ESH // 128              # fold width for shard-scalar math
    GROUPS = [list(range(ncores))]

    nc = bacc.Bacc("TRN2", target_bir_lowering=False, debug=False,
                   num_devices=ncores)

    x0 = nc.declare_dram_parameter("x0", [nloc, D], F32, isOutput=False)
    inc = nc.declare_dram_parameter("incidence", [nloc, EE], F32, isOutput=False)
    W01_0 = nc.declare_dram_parameter("W01_0", [D, H], F32, isOutput=False)
    b01_0 = nc.declare_dram_parameter("b01_0", [H, 1], F32, isOutput=False)
    W10_0 = nc.declare_dram_parameter("W10_0", [H, H], F32, isOutput=False)
    b10_0 = nc.declare_dram_parameter("b10_0", [H, 1], F32, isOutput=False)
    W01_1 = nc.declare_dram_parameter("W01_1", [D if D == H else H, H], F32,
                                      isOutput=False)
    b01_1 = nc.declare_dram_parameter("b01_1", [H, 1], F32, isOutput=False)
    W10_1 = nc.declare_dram_parameter("W10_1", [H, H], F32, isOutput=False)
    b10_1 = nc.declare_dram_parameter("b10_1", [H, 1], F32, isOutput=False)
    Wout = nc.declare_dram_parameter("Wout", [H, 1], F32, isOutput=False)
    bout = nc.declare_dram_parameter("bout", [1, 1], F32, isOutput=False)
    out = nc.declare_dram_parameter("out", [1, 1], F32, isOutput=True)

    B16 = nc.dram_tensor("b16", [nloc, EE], BF16)

    with tile.TileContext(nc, num_cores=ncores) as tc:
        with tc.tile_pool(name="persist", bufs=1) as pp, \
             tc.tile_pool(name="dram", bufs=1, space="DRAM") as dp:
            # ---- constants / weights ----
            id_f32 = pp.tile([128, 128], F32, tag="id_f32")
            make_identity(nc, id_f32[:])
            id_bf16 = pp.tile([128, 128], BF16, tag="id_bf16")
            make_identity(nc, id_bf16[:])
            w01_0 = pp.tile([D, H], F32, tag="w01_0")
            nc.gpsimd.dma_start(out=w01_0[:], in_=W01_0[:])
            w10_0 = pp.tile([H, H], F32, tag="w10_0")
            nc.gpsimd.dma_start(out=w10_0[:], in_=W10_0[:])
            w01_1 = pp.tile([H, H], F32, tag="w01_1")
            nc.gpsimd.dma_start(out=w01_1[:], in_=W01_1[:])
            w10_1 = pp.tile([H, H], F32, tag="w10_1")
            nc.gpsimd.dma_start(out=w10_1[:], in_=W10_1[:])
            bb01_0 = pp.tile([H, 1], F32, tag="bb01_0")
            nc.gpsimd.dma_start(out=bb01_0[:], in_=b01_0[:])
            bb10_0 = pp.tile([H, 1], F32, tag="bb10_0")
            nc.gpsimd.dma_start(out=bb10_0[:], in_=b10_0[:])
            bb01_1 = pp.tile([H, 1], F32, tag="bb01_1")
            nc.gpsimd.dma_start(out=bb01_1[:], in_=b01_1[:])
            bb10_1 = pp.tile([H, 1], F32, tag="bb10_1")
            nc.gpsimd.dma_start(out=bb10_1[:], in_=b10_1[:])
            wout = pp.tile([H, 1], F32, tag="wout")
            nc.gpsimd.dma_start(out=wout[:], in_=Wout[:])
            bbout = pp.tile([1, 1], F32, tag="bbout")
            nc.gpsimd.dma_start(out=bbout[:], in_=bout[:])

            # ---- persistent small state ----
            SLAB = 4096 if EE % 4096 == 0 else EE
            NSL = EE // SLAB
            rs_all = pp.tile([128, NVT * NSL], F32, tag="rs_all")
            deg_all = pp.tile([128, NVT], F32, tag="deg_all")
            vb_all = pp.tile([128, NVT], F32, tag="vb_all")
            s0h = pp.tile([128, NVT, H + 2], BF16, tag="s0h")
            s0l = pp.tile([128, NVT, H + 2], BF16, tag="s0l")
            s1h = pp.tile([128, NVT, H], BF16, tag="s1h")
            s1l = pp.tile([128, NVT, H], BF16, tag="s1l")
            rbB = pp.tile([H, ESH], F32, tag="rbB")      # 1/beta_denom bcast
            eaB = pp.tile([H, ESH], F32, tag="eaB")      # e_alpha bcast
            raB = pp.tile([H, nloc], F32, tag="raB")     # 1/alpha_denom bcast
            vbB = pp.tile([H, nloc], F32, tag="vbB")     # v_beta bcast (free)

            # ================= P0: stream fp32 B, rowsum + cast =========
            with tc.tile_pool(name="p0", bufs=3) as p0:
                for vt in range(NVT):
                    for s in range(NSL):
                        tin = p0.tile([128, SLAB], F32, tag="p0in")
                        nc.sync.dma_start(
                            out=tin[:],
                            in_=inc[vt * 128:(vt + 1) * 128,
                                    s * SLAB:(s + 1) * SLAB])
                        nc.vector.tensor_reduce(
                            out=rs_all[:, vt * NSL + s:vt * NSL + s + 1],
                            in_=tin[:], axis=mybir.AxisListType.X, op=OP.add)
                        tb = p0.tile([128, SLAB], BF16, tag="p0out")
                        nc.scalar.activation(out=tb[:], in_=tin[:],
                                             func=AF.Copy)
                        nc.sync.dma_start(
                            out=B16[vt * 128:(vt + 1) * 128,
                                    s * SLAB:(s + 1) * SLAB],
                            in_=tb[:])

            # node_deg, v_beta
            for vt in range(NVT):
                nc.vector.tensor_reduce(
                    out=deg_all[:, vt:vt + 1],
                    in_=rs_all[:, vt * NSL:(vt + 1) * NSL],
                    axis=mybir.AxisListType.X, op=OP.add)
            with tc.tile_pool(name="vbp", bufs=1) as vbp:
                degc = vbp.tile([128, NVT], F32, tag="degc")
                nc.vector.tensor_scalar_max(out=degc[:], in0=deg_all[:],
                                            scalar1=1.0)
                sqd = vbp.tile([128, NVT], F32, tag="sqd")
                nc.scalar.sqrt(out=sqd[:], in_=degc[:])
                nc.vector.reciprocal(out=vb_all[:], in_=sqd[:])
                # v_beta to free-layout DRAM row then broadcast into vbB
                with tc.tile_pool(name="vbps", bufs=1, space="PSUM") as vps:
                    pt = vps.tile([NVT, 128], F32, tag="vb_t")
                    nc.tensor.transpose(pt[:], vb_all[:], id_f32[:])
                    vb8 = vbp.tile([NVT, 128], F32, tag="vb8")
                    nc.vector.tensor_copy(out=vb8[:], in_=pt[:])
                vrow = dp.tile([1, nloc], F32, tag="vrow")
                nc.gpsimd.dma_start(
                    out=vrow[:].rearrange("a (b c) -> (a b) c", b=NVT),
                    in_=vb8[:])
                nc.gpsimd.dma_start(out=vbB[:],
                                    in_=vrow[:].to_broadcast([H, nloc]))

            # ================= S0 prep ==================================
            with tc.tile_pool(name="s0p", bufs=2) as sp, \
                 tc.tile_pool(name="s0ps", bufs=2, space="PSUM") as sps:
                for vt in range(NVT):
                    xt = sp.tile([128, D], F32, tag="xt")
                    nc.sync.dma_start(out=xt[:],
                                      in_=x0[vt * 128:(vt + 1) * 128, :])
                    pxt = sps.tile([D, 128], F32, tag="pxt")
                    nc.tensor.transpose(pxt[:], xt[:], id_f32[:])
                    x0T = sp.tile([D, 128], F32, tag="x0T")
                    nc.vector.tensor_copy(out=x0T[:], in_=pxt[:])
                    py = sps.tile([128, H], F32, tag="py")
                    nc.tensor.matmul(py[:], lhsT=x0T[:], rhs=w01_0[:],
                                     start=True, stop=True)
                    s0f = sp.tile([128, H + 2], F32, tag="s0f")
                    nc.vector.tensor_scalar_mul(out=s0f[:, 0:H], in0=py[:],
                                                scalar1=vb_all[:, vt:vt + 1])
                    nc.vector.tensor_copy(out=s0f[:, H:H + 1],
                                          in_=vb_all[:, vt:vt + 1])
                    nc.vector.memset(s0f[:, H + 1:H + 2], 1.0)
                    _hi_lo(nc, sp, s0f[:], s0h[:, vt, :], s0l[:, vt, :],
                           [128, H + 2])

            # ================= PA: U0' = B^T S0 (+ RS) ==================
            u0s_d = dp.tile([H + 2, ESH], F32, tag="u0s_d")
            with tc.tile_pool(name="pa", bufs=2) as pa, \
                 tc.tile_pool(name="pa_acc", bufs=1) as paa, \
                 tc.tile_pool(name="paps", bufs=2, space="PSUM") as paps:
                u0acc = paa.tile([H + 2, EE], F32, tag="u0acc")
                for sup in range(NSUP):
                    bt = pa.tile([128, NVT, PASUP], BF16, tag="pa_bt")
                    nc.sync.dma_start(
                        out=bt[:],
                        in_=B16[:, sup * PASUP:(sup + 1) * PASUP].rearrange(
                            "(vt p) e -> p vt e", p=128))
                    pu = paps.tile([H + 2, PASUP], F32, tag="pa_pu")
                    for c in range(PASUP // 512):
                        for vt in range(NVT):
                            for hl, st in ((0, s0h), (1, s0l)):
                                nc.tensor.matmul(
                                    pu[:, c * 512:(c + 1) * 512],
                                    lhsT=st[:, vt, :],
                                    rhs=bt[:, vt, c * 512:(c + 1) * 512],
                                    start=(vt == 0 and hl == 0),
                                    stop=(vt == NVT - 1 and hl == 1))
                    nc.vector.tensor_copy(
                        out=u0acc[:, sup * PASUP:(sup + 1) * PASUP],
                        in_=pu[:])
                bu0 = dp.tile([ncores, H + 2, ESH], F32, tag="bu0")
                nc.sync.dma_start(
                    out=bu0[:].rearrange("s h e -> h s e"),
                    in_=u0acc[:].rearrange("h (s e) -> h s e", s=ncores))
                nc.gpsimd.collective_compute(
                    "ReduceScatter", OP.add, replica_groups=GROUPS,
                    ins=[bu0.opt()], outs=[u0s_d.opt()])

            # ================= mid0: shard scalars + Z0' ================
            zg = dp.tile([ncores, 2, H + 1, ESH], BF16, tag="zg")
            with tc.tile_pool(name="m0", bufs=1) as m0, \
                 tc.tile_pool(name="m0ps", bufs=2, space="PSUM") as m0ps:
                u0s = m0.tile([H + 2, ESH], F32, tag="u0s")
                nc.scalar.dma_start(out=u0s[:], in_=u0s_d[:])
                # 1/beta_denom (guard 0 -> 1), via folded layout
                bd128 = m0.tile([128, FW], F32, tag="bd128")
                nc.gpsimd.dma_start(
                    out=bd128[:],
                    in_=u0s_d[H:H + 1, :].rearrange("a (p c) -> (a p) c",
                                                    p=128))
                msk = m0.tile([128, FW], F32, tag="msk")
                nc.vector.tensor_scalar(out=msk[:], in0=bd128[:], scalar1=0.0,
                                        scalar2=None, op0=OP.is_equal)
                nc.vector.tensor_add(out=bd128[:], in0=bd128[:], in1=msk[:])
                rb128 = m0.tile([128, FW], F32, tag="rb128")
                nc.vector.reciprocal(out=rb128[:], in_=bd128[:])
                rbrow = dp.tile([1, ESH], F32, tag="rbrow")
                nc.gpsimd.dma_start(
                    out=rbrow[:].rearrange("a (p c) -> (a p) c", p=128),
                    in_=rb128[:])
                nc.gpsimd.dma_start(out=rbB[:],
                                    in_=rbrow[:].to_broadcast([H, ESH]))
                # e_alpha = ecard'^-1.5 (guard 0 -> 1)
                ec128 = m0.tile([128, FW], F32, tag="ec128")
                nc.gpsimd.dma_start(
                    out=ec128[:],
                    in_=u0s_d[H + 1:H + 2, :].rearrange("a (p c) -> (a p) c",
                                                        p=128))
                nc.vector.tensor_scalar_max(out=ec128[:], in0=ec128[:],
                                            scalar1=1.0)
                sq = m0.tile([128, FW], F32, tag="sq")
                nc.scalar.sqrt(out=sq[:], in_=ec128[:])
                nc.vector.tensor_mul(out=sq[:], in0=sq[:], in1=ec128[:])
                ea128 = m0.tile([128, FW], F32, tag="ea128")
                nc.vector.reciprocal(out=ea128[:], in_=sq[:])
                earow = dp.tile([1, ESH], F32, tag="earow")
                nc.gpsimd.dma_start(
                    out=earow[:].rearrange("a (p c) -> (a p) c", p=128),
                    in_=ea128[:])
                nc.gpsimd.dma_start(out=eaB[:],
                                    in_=earow[:].to_broadcast([H, ESH]))
                # x1 shard (transposed layout [H, ESH])
                xs = m0.tile([H, ESH], F32, tag="xs")
                nc.vector.tensor_mul(out=xs[:], in0=u0s[0:H, :], in1=rbB[:])
                nc.scalar.activation(out=xs[:], in_=xs[:], func=AF.Relu,
                                     bias=bb01_0[:])
                # Z0'^T = (W10_0^T x1^T) * e_alpha ; extra row = e_alpha
                m0f = m0.tile([H + 1, ESH], F32, tag="m0f")
                for c in range(max(1, ESH // 512)):
                    zp = m0ps.tile([H, min(512, ESH)], F32, tag="zp")
                    nc.tensor.matmul(zp[:], lhsT=w10_0[:],
                                     rhs=xs[:, c * 512:(c + 1) * 512],
                                     start=True, stop=True)
                    nc.vector.tensor_mul(out=m0f[0:H, c * 512:(c + 1) * 512],
                                         in0=zp[:],
                                         in1=eaB[:, c * 512:(c + 1) * 512])
                nc.gpsimd.dma_start(out=m0f[H:H + 1, :], in_=earow[:])
                m0h = m0.tile([H + 1, ESH], BF16, tag="m0h")
                m0l = m0.tile([H + 1, ESH], BF16, tag="m0l")
                _hi_lo(nc, m0, m0f[:], m0h[:], m0l[:], [H + 1, ESH])
                bz = dp.tile([2, H + 1, ESH], BF16, tag="bz")
                nc.gpsimd.dma_start(out=bz[0], in_=m0h[:])
                nc.gpsimd.dma_start(out=bz[1], in_=m0l[:])
                nc.gpsimd.collective_compute(
                    "AllGather", OP.bypass, replica_groups=GROUPS,
                    ins=[bz.opt()], outs=[zg.opt()])

            # ================= PB: V0'^T = Z0'^T B^T ====================
            with tc.tile_pool(name="pbz", bufs=1) as pbz, \
                 tc.tile_pool(name="pb", bufs=2) as pb, \
                 tc.tile_pool(name="pbps", bufs=1, space="PSUM") as pbps, \
                 tc.tile_pool(name="pbps2", bufs=2, space="PSUM") as pbps2:
                zall = pbz.tile([H + 1, ncores, 2, ESH], BF16, tag="zall")
                nc.scalar.dma_start(
                    out=zall[:], in_=zg[:].rearrange("g h p e -> p g h e"))
                zsth = pbz.tile([128, NET, H + 1], BF16, tag="zsth")
                zstl = pbz.tile([128, NET, H + 1], BF16, tag="zstl")
                for et in range(NET):
                    g, w = divmod(et, ESH // 128)
                    for h, dst in ((0, zsth), (1, zstl)):
                        ptz = pbps2.tile([128, H + 1], BF16, tag="ptz")
                        nc.tensor.transpose(
                            ptz[:], zall[:, g, h, w * 128:(w + 1) * 128],
                            id_bf16[:H + 1, :H + 1])
                        nc.vector.tensor_copy(out=dst[:, et, :], in_=ptz[:])
                vp = pbps.tile([H + 1, nloc], F32, tag="vp")
                for sup in range(NTSUP):
                    btile = pb.tile([128, ETL, nloc], BF16, tag="pb_bt")
                    nc.sync.dma_start_transpose(
                        btile[:], B16[:, sup * TSUP:(sup + 1) * TSUP])
                    for etl in range(ETL):
                        et = sup * ETL + etl
                        for h, st in ((0, zsth), (1, zstl)):
                            for c in range(NCH):
                                nc.tensor.matmul(
                                    vp[:, c * CW:(c + 1) * CW],
                                    lhsT=st[:, et, :],
                                    rhs=btile[:, etl, c * CW:(c + 1) * CW],
                                    start=(et == 0 and h == 0),
                                    stop=(et == NET - 1 and h == 1))
                # alpha_denom -> 1/ad broadcast ; x = relu(V0/ad + b10_0)
                with tc.tile_pool(name="pbs", bufs=1) as pbs:
                    adm = pbs.tile([1, nloc], F32, tag="adm")
                    nc.vector.tensor_scalar(out=adm[:], in0=vp[H:H + 1, :],
                                            scalar1=0.0, scalar2=None,
                                            op0=OP.is_equal)
                    nc.vector.tensor_add(out=adm[:], in0=adm[:],
                                         in1=vp[H:H + 1, :])
                    ra = pbs.tile([1, nloc], F32, tag="ra")
                    nc.vector.reciprocal(out=ra[:], in_=adm[:])
                    rarow = dp.tile([1, nloc], F32, tag="rarow")
                    nc.gpsimd.dma_start(out=rarow[:], in_=ra[:])
                    nc.gpsimd.dma_start(out=raB[:],
                                        in_=rarow[:].to_broadcast([H, nloc]))
                    xl1 = pbs.tile([H, nloc], F32, tag="xl1")
                    nc.vector.tensor_mul(out=xl1[:], in0=vp[0:H, :],
                                         in1=raB[:])
                    nc.scalar.activation(out=xl1[:], in_=xl1[:], func=AF.Relu,
                                         bias=bb10_0[:])
                    # S1^T = (W01_1^T x^T) * v_beta
                    s1tf = pbs.tile([H, nloc], F32, tag="s1tf")
                    for c in range(NCH):
                        yp = pbps2.tile([H, CW], F32, tag="yp")
                        nc.tensor.matmul(yp[:], lhsT=w01_1[:],
                                         rhs=xl1[:, c * CW:(c + 1) * CW],
                                         start=True, stop=True)
                        nc.vector.tensor_mul(
                            out=s1tf[:, c * CW:(c + 1) * CW], in0=yp[:],
                            in1=vbB[:, c * CW:(c + 1) * CW])
                    s1th = pbs.tile([H, nloc], BF16, tag="s1th")
                    s1tl = pbs.tile([H, nloc], BF16, tag="s1tl")
                    _hi_lo(nc, pbs, s1tf[:], s1th[:], s1tl[:], [H, nloc])
                    for vt in range(NVT):
                        for src, dst in ((s1th, s1h), (s1tl, s1l)):
                            pts = pbps2.tile([128, H], BF16, tag="pts")
                            nc.tensor.transpose(
                                pts[:], src[:, vt * 128:(vt + 1) * 128],
                                id_bf16[:H, :H])
                            nc.vector.tensor_copy(out=dst[:, vt, :],
                                                  in_=pts[:])

            # ================= PC: U1' = B^T S1 (+ RS) ==================
            u1s_d = dp.tile([H, ESH], F32, tag="u1s_d")
            with tc.tile_pool(name="pc", bufs=2) as pc, \
                 tc.tile_pool(name="pc_acc", bufs=1) as pca, \
                 tc.tile_pool(name="pcps", bufs=2, space="PSUM") as pcps:
                u1acc = pca.tile([H, EE], F32, tag="u1acc")
                for sup in range(NSUP):
                    bt = pc.tile([128, NVT, PASUP], BF16, tag="pc_bt")
                    nc.sync.dma_start(
                        out=bt[:],
                        in_=B16[:, sup * PASUP:(sup + 1) * PASUP].rearrange(
                            "(vt p) e -> p vt e", p=128))
                    pu = pcps.tile([H, PASUP], F32, tag="pc_pu")
                    for c in range(PASUP // 512):
                        for vt in range(NVT):
                            for hl, st in ((0, s1h), (1, s1l)):
                                nc.tensor.matmul(
                                    pu[:, c * 512:(c + 1) * 512],
                                    lhsT=st[:, vt, :],
                                    rhs=bt[:, vt, c * 512:(c + 1) * 512],
                                    start=(vt == 0 and hl == 0),
                                    stop=(vt == NVT - 1 and hl == 1))
                    nc.vector.tensor_copy(
                        out=u1acc[:, sup * PASUP:(sup + 1) * PASUP],
                        in_=pu[:])
                bu1 = dp.tile([ncores, H, ESH], F32, tag="bu1")
                nc.sync.dma_start(
                    out=bu1[:].rearrange("s h e -> h s e"),
                    in_=u1acc[:].rearrange("h (s e) -> h s e", s=ncores))
                nc.gpsimd.collective_compute(
                    "ReduceScatter", OP.add, replica_groups=GROUPS,
                    ins=[bu1.opt()], outs=[u1s_d.opt()])

            # ================= mid1 =====================================
            z1g = dp.tile([ncores, 2, H, ESH], BF16, tag="z1g")
            with tc.tile_pool(name="m1", bufs=1) as m1, \
                 tc.tile_pool(name="m1ps", bufs=2, space="PSUM") as m1ps:
                u1s = m1.tile([H, ESH], F32, tag="u1s")
                nc.scalar.dma_start(out=u1s[:], in_=u1s_d[:])
                xs2 = m1.tile([H, ESH], F32, tag="xs2")
                nc.vector.tensor_mul(out=xs2[:], in0=u1s[:], in1=rbB[:])
                nc.scalar.activation(out=xs2[:], in_=xs2[:], func=AF.Relu,
                                     bias=bb01_1[:])
                m1f = m1.tile([H, ESH], F32, tag="m1f")
                for c in range(max(1, ESH // 512)):
                    zp1 = m1ps.tile([H, min(512, ESH)], F32, tag="zp1")
                    nc.tensor.matmul(zp1[:], lhsT=w10_1[:],
                                     rhs=xs2[:, c * 512:(c + 1) * 512],
                                     start=True, stop=True)
                    nc.vector.tensor_mul(out=m1f[:, c * 512:(c + 1) * 512],
                                         in0=zp1[:],
                                         in1=eaB[:, c * 512:(c + 1) * 512])
                m1h = m1.tile([H, ESH], BF16, tag="m1h")
                m1l = m1.tile([H, ESH], BF16, tag="m1l")
                _hi_lo(nc, m1, m1f[:], m1h[:], m1l[:], [H, ESH])
                bz1 = dp.tile([2, H, ESH], BF16, tag="bz1")
                nc.gpsimd.dma_start(out=bz1[0], in_=m1h[:])
                nc.gpsimd.dma_start(out=bz1[1], in_=m1l[:])
                nc.gpsimd.collective_compute(
                    "AllGather", OP.bypass, replica_groups=GROUPS,
                    ins=[bz1.opt()], outs=[z1g.opt()])

            # ================= PD: V1^T + finale ========================
            with tc.tile_pool(name="pdz", bufs=1) as pdz, \
                 tc.tile_pool(name="pd", bufs=2) as pd, \
                 tc.tile_pool(name="pdps", bufs=1, space="PSUM") as pdps, \
                 tc.tile_pool(name="pdps2", bufs=2, space="PSUM") as pdps2:
                z1all = pdz.tile([H, ncores, 2, ESH], BF16, tag="z1all")
                nc.scalar.dma_start(
                    out=z1all[:], in_=z1g[:].rearrange("g h p e -> p g h e"))
                z1sth = pdz.tile([128, NET, H], BF16, tag="z1sth")
                z1stl = pdz.tile([128, NET, H], BF16, tag="z1stl")
                for et in range(NET):
                    g, w = divmod(et, ESH // 128)
                    for h, dst in ((0, z1sth), (1, z1stl)):
                        ptz = pdps2.tile([128, H], BF16, tag="ptz1")
                        nc.tensor.transpose(
                            ptz[:], z1all[:, g, h, w * 128:(w + 1) * 128],
                            id_bf16[:H, :H])
                        nc.vector.tensor_copy(out=dst[:, et, :], in_=ptz[:])
                vp1 = pdps.tile([H, nloc], F32, tag="vp1")
                for sup in range(NTSUP):
                    btile = pd.tile([128, ETL, nloc], BF16, tag="pd_bt")
                    nc.sync.dma_start_transpose(
                        btile[:], B16[:, sup * TSUP:(sup + 1) * TSUP])
                    for etl in range(ETL):
                        et = sup * ETL + etl
                        for h, st in ((0, z1sth), (1, z1stl)):
                            for c in range(NCH):
                                nc.tensor.matmul(
                                    vp1[:, c * CW:(c + 1) * CW],
                                    lhsT=st[:, et, :],
                                    rhs=btile[:, etl, c * CW:(c + 1) * CW],
                                    start=(et == 0 and h == 0),
                                    stop=(et == NET - 1 and h == 1))
                with tc.tile_pool(name="fin", bufs=1) as fin:
                    x2 = fin.tile([H, nloc], F32, tag="x2")
                    nc.vector.tensor_mul(out=x2[:], in0=vp1[:], in1=raB[:])
                    nc.scalar.activation(out=x2[:], in_=x2[:], func=AF.Relu,
                                         bias=bb10_1[:])
                    pool_p = fin.tile([H, 1], F32, tag="pool_p")
                    nc.vector.tensor_reduce(out=pool_p[:], in_=x2[:],
                                            axis=mybir.AxisListType.X,
                                            op=OP.max)
                    bp = dp.tile([H, 1], F32, tag="bp")
                    nc.gpsimd.dma_start(out=bp[:], in_=pool_p[:])
                    bpo = dp.tile([H, 1], F32, tag="bpo")
                    nc.gpsimd.collective_compute(
                        "AllReduce", OP.max, replica_groups=GROUPS,
                        ins=[bp.opt()], outs=[bpo.opt()])
                    pooled = fin.tile([H, 1], F32, tag="pooled")
                    nc.gpsimd.dma_start(out=pooled[:], in_=bpo[:])
                    po = pdps2.tile([1, 1], F32, tag="po")
                    nc.tensor.matmul(po[:], lhsT=pooled[:], rhs=wout[:],
                                     start=True, stop=True)
                    ob = fin.tile([1, 1], F32, tag="ob")
                    nc.vector.tensor_add(out=ob[:], in0=po[:], in1=bbout[:])
                    nc.sync.dma_start(out=out[:], in_=ob[:])

    nc.compile()
    return nc


_NC_CACHE = {}


def _get_nc():
    if "nc" not in _NC_CACHE:
        _NC_CACHE["nc"] = build_kernel()
    return _NC_CACHE["nc"]


def _make_in_maps(inputs, ncores=NCORES, nloc=N // NCORES):
    x0 = np.asarray(inputs["x0"], np.float32)
    inc = np.asarray(inputs["incidence"], np.float32)
    w = {k: np.asarray(inputs[k], np.float32) for k in
         ("W01_0", "W10_0", "W01_1", "W10_1", "Wout")}
    b = {k: np.asarray(inputs[k], np.float32).reshape(-1, 1) for k in
         ("b01_0", "b10_0", "b01_1", "b10_1", "bout")}
    in_maps = []
    for c in range(ncores):
        m = {"x0": np.ascontiguousarray(x0[c * nloc:(c + 1) * nloc]),
             "incidence": np.ascontiguousarray(inc[c * nloc:(c + 1) * nloc])}
        m.update(w)
        m.update(b)
        in_maps.append(m)
    return in_maps


def kernel(**inputs) -> np.ndarray:
    nc = _get_nc()
    in_maps = _make_in_maps(inputs)
    res = run_bass_kernel_spmd(nc, in_maps, list(range(NCORES)))
    return res.results[0]["out"].reshape(1).astype(np.float32)


if __name__ == "__main__":
    pass



# revision 7
# speedup vs baseline: 9.4272x; 9.4272x over previous
"""HNHN hypergraph model on 8 Trainium2 NeuronCores (Bass/Tile).

Wall-time-optimized: the dominant cost of a warm run is the host->device
upload through the axon tunnel (~105 MB/s). The binary incidence matrix is
bit-packed on host (512MB fp32 -> 16MB u8) and unpacked to bf16 on device;
weights/biases are consolidated into one small array.

Sharding: rows (nodes) of the incidence matrix and x0 over 8 cores.
Pipeline per core (v = own 1024 nodes, e = all 16384 hyperedges):
  P0   : stream packed bits -> unpack to bf16 B16 tile (strided bit-planes)
         -> row sums (node_deg) -> DRAM scratch B16
  S0   : Y0 = x0 @ W01_0, S0 = [Y0*v_beta | v_beta | 1]  (hi/lo bf16 split)
  PA   : U0' = B^T S0 partials (PSUM accum over own v)  -> ReduceScatter(add)
         U0'[64] = beta_denom partials, U0'[65] = edge_card partials
  mid0 : per e-shard: x1 = relu(U0/beta_denom + b01_0); Z0' = [x1@W10_0*e_a | e_a]
         hi/lo bf16 -> AllGather
  PB   : V0'^T = Z0'^T B^T via transposed-DMA reads of B16 (accum over all e)
         row 64 = alpha_denom (local);  x = relu(V0/alpha_denom + b10_0)
  PC   : layer-1 edge pass (same as PA, no extra cols) -> ReduceScatter
  mid1 : x1_l2 = relu(U1/beta_denom + b01_1); Z1' hi/lo -> AllGather
  PD   : V1^T accum;  x2 = relu(V1/alpha_denom + b10_1)
  fin  : per-core max-pool -> AllReduce(max) -> pooled @ Wout + bout
"""

import numpy as np

import concourse.bass as bass
import concourse.bacc as bacc
import concourse.mybir as mybir
import concourse.tile as tile
from concourse.bass_utils import run_bass_kernel_spmd
from concourse.masks import make_identity

F32 = mybir.dt.float32
BF16 = mybir.dt.bfloat16
U8 = mybir.dt.uint8
AF = mybir.ActivationFunctionType
OP = mybir.AluOpType

N, E, D, H = 8192, 16384, 128, 64
NCORES = 8
# packed weight array column map (see _make_in_maps); all blocks start at
# partition row 0 so on-device slices never cross partition offsets
WCOL_W01_0 = 0            # [128, 64]   rows 0:128
WCOL_W10_0 = 64           # [64, 64]    rows 0:64
WCOL_W01_1 = 128          # [64, 64]    rows 0:64
WCOL_W10_1 = 192          # [64, 64]    rows 0:64
WCOL_B01_0 = 256          # [64, 1]
WCOL_B10_0 = 257          # [64, 1]
WCOL_B01_1 = 258          # [64, 1]
WCOL_B10_1 = 259          # [64, 1]
WCOL_WOUT = 260           # [64, 1]
WCOL_BOUT = 261           # [1, 1]
WPACK_COLS = 262


def _hi_lo(nc, pool, src_f32, dst_hi, dst_lo, tmp_shape):
    """Split fp32 tile into hi+lo bf16 (dst_hi + dst_lo ~= src exactly)."""
    nc.vector.tensor_copy(out=dst_hi, in_=src_f32)
    hf = pool.tile(tmp_shape, F32, tag="hilo_hf")
    nc.vector.tensor_copy(out=hf, in_=dst_hi)
    lof = pool.tile(tmp_shape, F32, tag="hilo_lof")
    nc.vector.tensor_sub(out=lof, in0=src_f32, in1=hf)
    nc.vector.tensor_copy(out=dst_lo, in_=lof)


def build_kernel(ncores=NCORES, n_edges=E, nloc=N // NCORES):
    EE = n_edges
    EB = EE // 8                 # packed bytes per row
    NVT = nloc // 128            # v-tiles per core
    ESH = EE // ncores           # e-shard per core
    NET = EE // 128              # 128-wide e-tiles
    PASUP = min(2048, EE)        # PA/PC streaming super width
    NSUP = EE // PASUP
    TSUP = min(1024, EE)         # PB/PD transposed-read super width
    NTSUP = EE // TSUP
    ETL = TSUP // 128            # e-tiles per transposed read
    CW = min(512, nloc)          # column chunk for nloc-wide ops
    NCH = nloc // CW
    FW = ESH // 128              # fold width for shard-scalar math
    GROUPS = [list(range(ncores))]

    nc = bacc.Bacc("TRN2", target_bir_lowering=False, debug=False,
                   num_devices=ncores)

    x0 = nc.declare_dram_parameter("x0", [nloc, D], F32, isOutput=False)
    bits = nc.declare_dram_parameter("bits", [nloc, EB], U8, isOutput=False)
    wpk = nc.declare_dram_parameter("wpack", [128, WPACK_COLS], F32,
                                    isOutput=False)
    out = nc.declare_dram_parameter("out", [1, 1], F32, isOutput=True)

    B16 = nc.dram_tensor("b16", [nloc, EE], BF16)

    with tile.TileContext(nc, num_cores=ncores) as tc:
        with tc.tile_pool(name="persist", bufs=1) as pp, \
             tc.tile_pool(name="dram", bufs=1, space="DRAM") as dp:
            # ---- constants / weights ----
            id_f32 = pp.tile([128, 128], F32, tag="id_f32")
            make_identity(nc, id_f32[:])
            id_bf16 = pp.tile([128, 128], BF16, tag="id_bf16")
            make_identity(nc, id_bf16[:])
            wall = pp.tile([128, WPACK_COLS], F32, tag="wall")
            nc.sync.dma_start(out=wall[:], in_=wpk[:])
            w01_0 = wall[:, WCOL_W01_0:WCOL_W01_0 + H]            # [128,64]
            w10_0 = wall[0:H, WCOL_W10_0:WCOL_W10_0 + H]          # [64,64]
            w01_1 = wall[0:H, WCOL_W01_1:WCOL_W01_1 + H]
            w10_1 = wall[0:H, WCOL_W10_1:WCOL_W10_1 + H]
            bb01_0 = pp.tile([H, 1], F32, tag="bb01_0")
            nc.vector.tensor_copy(out=bb01_0[:],
                                  in_=wall[0:H, WCOL_B01_0:WCOL_B01_0 + 1])
            bb10_0 = pp.tile([H, 1], F32, tag="bb10_0")
            nc.vector.tensor_copy(out=bb10_0[:],
                                  in_=wall[0:H, WCOL_B10_0:WCOL_B10_0 + 1])
            bb01_1 = pp.tile([H, 1], F32, tag="bb01_1")
            nc.vector.tensor_copy(out=bb01_1[:],
                                  in_=wall[0:H, WCOL_B01_1:WCOL_B01_1 + 1])
            bb10_1 = pp.tile([H, 1], F32, tag="bb10_1")
            nc.vector.tensor_copy(out=bb10_1[:],
                                  in_=wall[0:H, WCOL_B10_1:WCOL_B10_1 + 1])
            wout = pp.tile([H, 1], F32, tag="wout")
            nc.vector.tensor_copy(out=wout[:],
                                  in_=wall[0:H, WCOL_WOUT:WCOL_WOUT + 1])
            bbout = pp.tile([1, 1], F32, tag="bbout")
            nc.vector.tensor_copy(out=bbout[:],
                                  in_=wall[0:1, WCOL_BOUT:WCOL_BOUT + 1])

            # ---- persistent small state ----
            deg_all = pp.tile([128, NVT], F32, tag="deg_all")
            vb_all = pp.tile([128, NVT], F32, tag="vb_all")
            s0h = pp.tile([128, NVT, H + 2], BF16, tag="s0h")
            s0l = pp.tile([128, NVT, H + 2], BF16, tag="s0l")
            s1h = pp.tile([128, NVT, H], BF16, tag="s1h")
            s1l = pp.tile([128, NVT, H], BF16, tag="s1l")
            rbB = pp.tile([H, ESH], F32, tag="rbB")      # 1/beta_denom bcast
            eaB = pp.tile([H, ESH], F32, tag="eaB")      # e_alpha bcast
            raB = pp.tile([H, nloc], F32, tag="raB")     # 1/alpha_denom bcast
            vbB = pp.tile([H, nloc], F32, tag="vbB")     # v_beta bcast (free)

            # ====== P0: unpack bits -> bf16 B16 + row sums (node_deg) ======
            with tc.tile_pool(name="p0", bufs=2) as p0:
                for vt in range(NVT):
                    bt = p0.tile([128, EB], U8, tag="p0bits")
                    nc.sync.dma_start(
                        out=bt[:], in_=bits[vt * 128:(vt + 1) * 128, :])
                    ub = p0.tile([128, EE], BF16, tag="p0ub")
                    ubv = ub[:].rearrange("p (j t) -> p t j", t=8)
                    for t in range(8):
                        m = p0.tile([128, EB], U8, tag="p0m")
                        nc.vector.tensor_scalar(
                            out=m[:], in0=bt[:], scalar1=1 << t,
                            scalar2=None, op0=OP.bitwise_and)
                        nc.vector.tensor_scalar(
                            out=ubv[:, t, :], in0=m[:], scalar1=0,
                            scalar2=None, op0=OP.is_gt)
                    nc.vector.tensor_reduce(
                        out=deg_all[:, vt:vt + 1], in_=ub[:],
                        axis=mybir.AxisListType.X, op=OP.add)
                    nc.sync.dma_start(
                        out=B16[vt * 128:(vt + 1) * 128, :], in_=ub[:])

            # node_deg -> v_beta
            with tc.tile_pool(name="vbp", bufs=1) as vbp:
                degc = vbp.tile([128, NVT], F32, tag="degc")
                nc.vector.tensor_scalar_max(out=degc[:], in0=deg_all[:],
                                            scalar1=1.0)
                sqd = vbp.tile([128, NVT], F32, tag="sqd")
                nc.scalar.sqrt(out=sqd[:], in_=degc[:])
                nc.vector.reciprocal(out=vb_all[:], in_=sqd[:])
                # v_beta to free-layout DRAM row then broadcast into vbB
                with tc.tile_pool(name="vbps", bufs=1, space="PSUM") as vps:
                    pt = vps.tile([NVT, 128], F32, tag="vb_t")
                    nc.tensor.transpose(pt[:], vb_all[:], id_f32[:])
                    vb8 = vbp.tile([NVT, 128], F32, tag="vb8")
                    nc.vector.tensor_copy(out=vb8[:], in_=pt[:])
                vrow = dp.tile([1, nloc], F32, tag="vrow")
                nc.gpsimd.dma_start(
                    out=vrow[:].rearrange("a (b c) -> (a b) c", b=NVT),
                    in_=vb8[:])
                nc.gpsimd.dma_start(out=vbB[:],
                                    in_=vrow[:].to_broadcast([H, nloc]))

            # ================= S0 prep ==================================
            with tc.tile_pool(name="s0p", bufs=2) as sp, \
                 tc.tile_pool(name="s0ps", bufs=2, space="PSUM") as sps:
                for vt in range(NVT):
                    xt = sp.tile([128, D], F32, tag="xt")
                    nc.sync.dma_start(out=xt[:],
                                      in_=x0[vt * 128:(vt + 1) * 128, :])
                    pxt = sps.tile([D, 128], F32, tag="pxt")
                    nc.tensor.transpose(pxt[:], xt[:], id_f32[:])
                    x0T = sp.tile([D, 128], F32, tag="x0T")
                    nc.vector.tensor_copy(out=x0T[:], in_=pxt[:])
                    py = sps.tile([128, H], F32, tag="py")
                    nc.tensor.matmul(py[:], lhsT=x0T[:], rhs=w01_0,
                                     start=True, stop=True)
                    s0f = sp.tile([128, H + 2], F32, tag="s0f")
                    nc.vector.tensor_scalar_mul(out=s0f[:, 0:H], in0=py[:],
                                                scalar1=vb_all[:, vt:vt + 1])
                    nc.vector.tensor_copy(out=s0f[:, H:H + 1],
                                          in_=vb_all[:, vt:vt + 1])
                    nc.vector.memset(s0f[:, H + 1:H + 2], 1.0)
                    _hi_lo(nc, sp, s0f[:], s0h[:, vt, :], s0l[:, vt, :],
                           [128, H + 2])

            # ================= PA: U0' = B^T S0 (+ RS) ==================
            u0s_d = dp.tile([H + 2, ESH], F32, tag="u0s_d")
            with tc.tile_pool(name="pa", bufs=2) as pa, \
                 tc.tile_pool(name="pa_acc", bufs=1) as paa, \
                 tc.tile_pool(name="paps", bufs=2, space="PSUM") as paps:
                u0acc = paa.tile([H + 2, EE], F32, tag="u0acc")
                for sup in range(NSUP):
                    bt = pa.tile([128, NVT, PASUP], BF16, tag="pa_bt")
                    nc.sync.dma_start(
                        out=bt[:],
                        in_=B16[:, sup * PASUP:(sup + 1) * PASUP].rearrange(
                            "(vt p) e -> p vt e", p=128))
                    pu = paps.tile([H + 2, PASUP], F32, tag="pa_pu")
                    for c in range(PASUP // 512):
                        for vt in range(NVT):
                            for hl, st in ((0, s0h), (1, s0l)):
                                nc.tensor.matmul(
                                    pu[:, c * 512:(c + 1) * 512],
                                    lhsT=st[:, vt, :],
                                    rhs=bt[:, vt, c * 512:(c + 1) * 512],
                                    start=(vt == 0 and hl == 0),
                                    stop=(vt == NVT - 1 and hl == 1))
                    nc.vector.tensor_copy(
                        out=u0acc[:, sup * PASUP:(sup + 1) * PASUP],
                        in_=pu[:])
                bu0 = dp.tile([ncores, H + 2, ESH], F32, tag="bu0")
                nc.sync.dma_start(
                    out=bu0[:].rearrange("s h e -> h s e"),
                    in_=u0acc[:].rearrange("h (s e) -> h s e", s=ncores))
                nc.gpsimd.collective_compute(
                    "ReduceScatter", OP.add, replica_groups=GROUPS,
                    ins=[bu0.opt()], outs=[u0s_d.opt()])

            # ================= mid0: shard scalars + Z0' ================
            zg = dp.tile([ncores, 2, H + 1, ESH], BF16, tag="zg")
            with tc.tile_pool(name="m0", bufs=1) as m0, \
                 tc.tile_pool(name="m0ps", bufs=2, space="PSUM") as m0ps:
                u0s = m0.tile([H + 2, ESH], F32, tag="u0s")
                nc.scalar.dma_start(out=u0s[:], in_=u0s_d[:])
                # 1/beta_denom (guard 0 -> 1), via folded layout
                bd128 = m0.tile([128, FW], F32, tag="bd128")
                nc.gpsimd.dma_start(
                    out=bd128[:],
                    in_=u0s_d[H:H + 1, :].rearrange("a (p c) -> (a p) c",
                                                    p=128))
                msk = m0.tile([128, FW], F32, tag="msk")
                nc.vector.tensor_scalar(out=msk[:], in0=bd128[:], scalar1=0.0,
                                        scalar2=None, op0=OP.is_equal)
                nc.vector.tensor_add(out=bd128[:], in0=bd128[:], in1=msk[:])
                rb128 = m0.tile([128, FW], F32, tag="rb128")
                nc.vector.reciprocal(out=rb128[:], in_=bd128[:])
                rbrow = dp.tile([1, ESH], F32, tag="rbrow")
                nc.gpsimd.dma_start(
                    out=rbrow[:].rearrange("a (p c) -> (a p) c", p=128),
                    in_=rb128[:])
                nc.gpsimd.dma_start(out=rbB[:],
                                    in_=rbrow[:].to_broadcast([H, ESH]))
                # e_alpha = ecard'^-1.5 (guard 0 -> 1)
                ec128 = m0.tile([128, FW], F32, tag="ec128")
                nc.gpsimd.dma_start(
                    out=ec128[:],
                    in_=u0s_d[H + 1:H + 2, :].rearrange("a (p c) -> (a p) c",
                                                        p=128))
                nc.vector.tensor_scalar_max(out=ec128[:], in0=ec128[:],
                                            scalar1=1.0)
                sq = m0.tile([128, FW], F32, tag="sq")
                nc.scalar.sqrt(out=sq[:], in_=ec128[:])
                nc.vector.tensor_mul(out=sq[:], in0=sq[:], in1=ec128[:])
                ea128 = m0.tile([128, FW], F32, tag="ea128")
                nc.vector.reciprocal(out=ea128[:], in_=sq[:])
                earow = dp.tile([1, ESH], F32, tag="earow")
                nc.gpsimd.dma_start(
                    out=earow[:].rearrange("a (p c) -> (a p) c", p=128),
                    in_=ea128[:])
                nc.gpsimd.dma_start(out=eaB[:],
                                    in_=earow[:].to_broadcast([H, ESH]))
                # x1 shard (transposed layout [H, ESH])
                xs = m0.tile([H, ESH], F32, tag="xs")
                nc.vector.tensor_mul(out=xs[:], in0=u0s[0:H, :], in1=rbB[:])
                nc.scalar.activation(out=xs[:], in_=xs[:], func=AF.Relu,
                                     bias=bb01_0[:])
                # Z0'^T = (W10_0^T x1^T) * e_alpha ; extra row = e_alpha
                m0f = m0.tile([H + 1, ESH], F32, tag="m0f")
                for c in range(max(1, ESH // 512)):
                    zp = m0ps.tile([H, min(512, ESH)], F32, tag="zp")
                    nc.tensor.matmul(zp[:], lhsT=w10_0,
                                     rhs=xs[:, c * 512:(c + 1) * 512],
                                     start=True, stop=True)
                    nc.vector.tensor_mul(out=m0f[0:H, c * 512:(c + 1) * 512],
                                         in0=zp[:],
                                         in1=eaB[:, c * 512:(c + 1) * 512])
                nc.gpsimd.dma_start(out=m0f[H:H + 1, :], in_=earow[:])
                m0h = m0.tile([H + 1, ESH], BF16, tag="m0h")
                m0l = m0.tile([H + 1, ESH], BF16, tag="m0l")
                _hi_lo(nc, m0, m0f[:], m0h[:], m0l[:], [H + 1, ESH])
                bz = dp.tile([2, H + 1, ESH], BF16, tag="bz")
                nc.gpsimd.dma_start(out=bz[0], in_=m0h[:])
                nc.gpsimd.dma_start(out=bz[1], in_=m0l[:])
                nc.gpsimd.collective_compute(
                    "AllGather", OP.bypass, replica_groups=GROUPS,
                    ins=[bz.opt()], outs=[zg.opt()])

            # ================= PB: V0'^T = Z0'^T B^T ====================
            with tc.tile_pool(name="pbz", bufs=1) as pbz, \
                 tc.tile_pool(name="pb", bufs=2) as pb, \
                 tc.tile_pool(name="pbps", bufs=1, space="PSUM") as pbps, \
                 tc.tile_pool(name="pbps2", bufs=2, space="PSUM") as pbps2:
                zall = pbz.tile([H + 1, ncores, 2, ESH], BF16, tag="zall")
                nc.scalar.dma_start(
                    out=zall[:], in_=zg[:].rearrange("g h p e -> p g h e"))
                zsth = pbz.tile([128, NET, H + 1], BF16, tag="zsth")
                zstl = pbz.tile([128, NET, H + 1], BF16, tag="zstl")
                for et in range(NET):
                    g, w = divmod(et, ESH // 128)
                    for h, dst in ((0, zsth), (1, zstl)):
                        ptz = pbps2.tile([128, H + 1], BF16, tag="ptz")
                        nc.tensor.transpose(
                            ptz[:], zall[:, g, h, w * 128:(w + 1) * 128],
                            id_bf16[:H + 1, :H + 1])
                        nc.vector.tensor_copy(out=dst[:, et, :], in_=ptz[:])
                vp = pbps.tile([H + 1, nloc], F32, tag="vp")
                for sup in range(NTSUP):
                    btile = pb.tile([128, ETL, nloc], BF16, tag="pb_bt")
                    nc.sync.dma_start_transpose(
                        btile[:], B16[:, sup * TSUP:(sup + 1) * TSUP])
                    for etl in range(ETL):
                        et = sup * ETL + etl
                        for h, st in ((0, zsth), (1, zstl)):
                            for c in range(NCH):
                                nc.tensor.matmul(
                                    vp[:, c * CW:(c + 1) * CW],
                                    lhsT=st[:, et, :],
                                    rhs=btile[:, etl, c * CW:(c + 1) * CW],
                                    start=(et == 0 and h == 0),
                                    stop=(et == NET - 1 and h == 1))
                # alpha_denom -> 1/ad broadcast ; x = relu(V0/ad + b10_0)
                with tc.tile_pool(name="pbs", bufs=1) as pbs:
                    adm = pbs.tile([1, nloc], F32, tag="adm")
                    nc.vector.tensor_scalar(out=adm[:], in0=vp[H:H + 1, :],
                                            scalar1=0.0, scalar2=None,
                                            op0=OP.is_equal)
                    nc.vector.tensor_add(out=adm[:], in0=adm[:],
                                         in1=vp[H:H + 1, :])
                    ra = pbs.tile([1, nloc], F32, tag="ra")
                    nc.vector.reciprocal(out=ra[:], in_=adm[:])
                    rarow = dp.tile([1, nloc], F32, tag="rarow")
                    nc.gpsimd.dma_start(out=rarow[:], in_=ra[:])
                    nc.gpsimd.dma_start(out=raB[:],
                                        in_=rarow[:].to_broadcast([H, nloc]))
                    xl1 = pbs.tile([H, nloc], F32, tag="xl1")
                    nc.vector.tensor_mul(out=xl1[:], in0=vp[0:H, :],
                                         in1=raB[:])
                    nc.scalar.activation(out=xl1[:], in_=xl1[:], func=AF.Relu,
                                         bias=bb10_0[:])
                    # S1^T = (W01_1^T x^T) * v_beta
                    s1tf = pbs.tile([H, nloc], F32, tag="s1tf")
                    for c in range(NCH):
                        yp = pbps2.tile([H, CW], F32, tag="yp")
                        nc.tensor.matmul(yp[:], lhsT=w01_1,
                                         rhs=xl1[:, c * CW:(c + 1) * CW],
                                         start=True, stop=True)
                        nc.vector.tensor_mul(
                            out=s1tf[:, c * CW:(c + 1) * CW], in0=yp[:],
                            in1=vbB[:, c * CW:(c + 1) * CW])
                    s1th = pbs.tile([H, nloc], BF16, tag="s1th")
                    s1tl = pbs.tile([H, nloc], BF16, tag="s1tl")
                    _hi_lo(nc, pbs, s1tf[:], s1th[:], s1tl[:], [H, nloc])
                    for vt in range(NVT):
                        for src, dst in ((s1th, s1h), (s1tl, s1l)):
                            pts = pbps2.tile([128, H], BF16, tag="pts")
                            nc.tensor.transpose(
                                pts[:], src[:, vt * 128:(vt + 1) * 128],
                                id_bf16[:H, :H])
                            nc.vector.tensor_copy(out=dst[:, vt, :],
                                                  in_=pts[:])

            # ================= PC: U1' = B^T S1 (+ RS) ==================
            u1s_d = dp.tile([H, ESH], F32, tag="u1s_d")
            with tc.tile_pool(name="pc", bufs=2) as pc, \
                 tc.tile_pool(name="pc_acc", bufs=1) as pca, \
                 tc.tile_pool(name="pcps", bufs=2, space="PSUM") as pcps:
                u1acc = pca.tile([H, EE], F32, tag="u1acc")
                for sup in range(NSUP):
                    bt = pc.tile([128, NVT, PASUP], BF16, tag="pc_bt")
                    nc.sync.dma_start(
                        out=bt[:],
                        in_=B16[:, sup * PASUP:(sup + 1) * PASUP].rearrange(
                            "(vt p) e -> p vt e", p=128))
                    pu = pcps.tile([H, PASUP], F32, tag="pc_pu")
                    for c in range(PASUP // 512):
                        for vt in range(NVT):
                            for hl, st in ((0, s1h), (1, s1l)):
                                nc.tensor.matmul(
                                    pu[:, c * 512:(c + 1) * 512],
                                    lhsT=st[:, vt, :],
                                    rhs=bt[:, vt, c * 512:(c + 1) * 512],
                                    start=(vt == 0 and hl == 0),
                                    stop=(vt == NVT - 1 and hl == 1))
                    nc.vector.tensor_copy(
                        out=u1acc[:, sup * PASUP:(sup + 1) * PASUP],
                        in_=pu[:])
                bu1 = dp.tile([ncores, H, ESH], F32, tag="bu1")
                nc.sync.dma_start(
                    out=bu1[:].rearrange("s h e -> h s e"),
                    in_=u1acc[:].rearrange("h (s e) -> h s e", s=ncores))
                nc.gpsimd.collective_compute(
                    "ReduceScatter", OP.add, replica_groups=GROUPS,
                    ins=[bu1.opt()], outs=[u1s_d.opt()])

            # ================= mid1 =====================================
            z1g = dp.tile([ncores, 2, H, ESH], BF16, tag="z1g")
            with tc.tile_pool(name="m1", bufs=1) as m1, \
                 tc.tile_pool(name="m1ps", bufs=2, space="PSUM") as m1ps:
                u1s = m1.tile([H, ESH], F32, tag="u1s")
                nc.scalar.dma_start(out=u1s[:], in_=u1s_d[:])
                xs2 = m1.tile([H, ESH], F32, tag="xs2")
                nc.vector.tensor_mul(out=xs2[:], in0=u1s[:], in1=rbB[:])
                nc.scalar.activation(out=xs2[:], in_=xs2[:], func=AF.Relu,
                                     bias=bb01_1[:])
                m1f = m1.tile([H, ESH], F32, tag="m1f")
                for c in range(max(1, ESH // 512)):
                    zp1 = m1ps.tile([H, min(512, ESH)], F32, tag="zp1")
                    nc.tensor.matmul(zp1[:], lhsT=w10_1,
                                     rhs=xs2[:, c * 512:(c + 1) * 512],
                                     start=True, stop=True)
                    nc.vector.tensor_mul(out=m1f[:, c * 512:(c + 1) * 512],
                                         in0=zp1[:],
                                         in1=eaB[:, c * 512:(c + 1) * 512])
                m1h = m1.tile([H, ESH], BF16, tag="m1h")
                m1l = m1.tile([H, ESH], BF16, tag="m1l")
                _hi_lo(nc, m1, m1f[:], m1h[:], m1l[:], [H, ESH])
                bz1 = dp.tile([2, H, ESH], BF16, tag="bz1")
                nc.gpsimd.dma_start(out=bz1[0], in_=m1h[:])
                nc.gpsimd.dma_start(out=bz1[1], in_=m1l[:])
                nc.gpsimd.collective_compute(
                    "AllGather", OP.bypass, replica_groups=GROUPS,
                    ins=[bz1.opt()], outs=[z1g.opt()])

            # ================= PD: V1^T + finale ========================
            with tc.tile_pool(name="pdz", bufs=1) as pdz, \
                 tc.tile_pool(name="pd", bufs=2) as pd, \
                 tc.tile_pool(name="pdps", bufs=1, space="PSUM") as pdps, \
                 tc.tile_pool(name="pdps2", bufs=2, space="PSUM") as pdps2:
                z1all = pdz.tile([H, ncores, 2, ESH], BF16, tag="z1all")
                nc.scalar.dma_start(
                    out=z1all[:], in_=z1g[:].rearrange("g h p e -> p g h e"))
                z1sth = pdz.tile([128, NET, H], BF16, tag="z1sth")
                z1stl = pdz.tile([128, NET, H], BF16, tag="z1stl")
                for et in range(NET):
                    g, w = divmod(et, ESH // 128)
                    for h, dst in ((0, z1sth), (1, z1stl)):
                        ptz = pdps2.tile([128, H], BF16, tag="ptz1")
                        nc.tensor.transpose(
                            ptz[:], z1all[:, g, h, w * 128:(w + 1) * 128],
                            id_bf16[:H, :H])
                        nc.vector.tensor_copy(out=dst[:, et, :], in_=ptz[:])
                vp1 = pdps.tile([H, nloc], F32, tag="vp1")
                for sup in range(NTSUP):
                    btile = pd.tile([128, ETL, nloc], BF16, tag="pd_bt")
                    nc.sync.dma_start_transpose(
                        btile[:], B16[:, sup * TSUP:(sup + 1) * TSUP])
                    for etl in range(ETL):
                        et = sup * ETL + etl
                        for h, st in ((0, z1sth), (1, z1stl)):
                            for c in range(NCH):
                                nc.tensor.matmul(
                                    vp1[:, c * CW:(c + 1) * CW],
                                    lhsT=st[:, et, :],
                                    rhs=btile[:, etl, c * CW:(c + 1) * CW],
                                    start=(et == 0 and h == 0),
                                    stop=(et == NET - 1 and h == 1))
                with tc.tile_pool(name="fin", bufs=1) as fin:
                    x2 = fin.tile([H, nloc], F32, tag="x2")
                    nc.vector.tensor_mul(out=x2[:], in0=vp1[:], in1=raB[:])
                    nc.scalar.activation(out=x2[:], in_=x2[:], func=AF.Relu,
                                         bias=bb10_1[:])
                    pool_p = fin.tile([H, 1], F32, tag="pool_p")
                    nc.vector.tensor_reduce(out=pool_p[:], in_=x2[:],
                                            axis=mybir.AxisListType.X,
                                            op=OP.max)
                    bp = dp.tile([H, 1], F32, tag="bp")
                    nc.gpsimd.dma_start(out=bp[:], in_=pool_p[:])
                    bpo = dp.tile([H, 1], F32, tag="bpo")
                    nc.gpsimd.collective_compute(
                        "AllReduce", OP.max, replica_groups=GROUPS,
                        ins=[bp.opt()], outs=[bpo.opt()])
                    pooled = fin.tile([H, 1], F32, tag="pooled")
                    nc.gpsimd.dma_start(out=pooled[:], in_=bpo[:])
                    po = pdps2.tile([1, 1], F32, tag="po")
                    nc.tensor.matmul(po[:], lhsT=pooled[:], rhs=wout[:],
                                     start=True, stop=True)
                    ob = fin.tile([1, 1], F32, tag="ob")
                    nc.vector.tensor_add(out=ob[:], in0=po[:], in1=bbout[:])
                    nc.sync.dma_start(out=out[:], in_=ob[:])

    nc.compile()
    return nc


_NC_CACHE = {}


def _get_nc():
    if "nc" not in _NC_CACHE:
        _NC_CACHE["nc"] = build_kernel()
    return _NC_CACHE["nc"]


def _make_in_maps(inputs, ncores=NCORES, nloc=N // NCORES):
    x0 = np.asarray(inputs["x0"], np.float32)
    inc = np.asarray(inputs["incidence"])
    bits = np.packbits(inc != 0, axis=1, bitorder="little")  # [N, E//8] u8
    wpack = np.zeros((128, WPACK_COLS), np.float32)
    wpack[:, WCOL_W01_0:WCOL_W01_0 + H] = np.asarray(inputs["W01_0"],
                                                     np.float32)
    wpack[0:H, WCOL_W10_0:WCOL_W10_0 + H] = np.asarray(inputs["W10_0"],
                                                       np.float32)
    wpack[0:H, WCOL_W01_1:WCOL_W01_1 + H] = np.asarray(inputs["W01_1"],
                                                       np.float32)
    wpack[0:H, WCOL_W10_1:WCOL_W10_1 + H] = np.asarray(inputs["W10_1"],
                                                       np.float32)
    wpack[0:H, WCOL_B01_0] = np.asarray(inputs["b01_0"],
                                        np.float32).reshape(-1)
    wpack[0:H, WCOL_B10_0] = np.asarray(inputs["b10_0"],
                                        np.float32).reshape(-1)
    wpack[0:H, WCOL_B01_1] = np.asarray(inputs["b01_1"],
                                        np.float32).reshape(-1)
    wpack[0:H, WCOL_B10_1] = np.asarray(inputs["b10_1"],
                                        np.float32).reshape(-1)
    wpack[0:H, WCOL_WOUT] = np.asarray(inputs["Wout"], np.float32).reshape(-1)
    wpack[0:1, WCOL_BOUT] = np.asarray(inputs["bout"], np.float32).reshape(-1)
    in_maps = []
    for c in range(ncores):
        m = {"x0": np.ascontiguousarray(x0[c * nloc:(c + 1) * nloc]),
             "bits": np.ascontiguousarray(bits[c * nloc:(c + 1) * nloc]),
             "wpack": wpack}
        in_maps.append(m)
    return in_maps


def kernel(**inputs) -> np.ndarray:
    nc = _get_nc()
    in_maps = _make_in_maps(inputs)
    res = run_bass_kernel_spmd(nc, in_maps, list(range(NCORES)))
    return res.results[0]["out"].reshape(1).astype(np.float32)


if __name__ == "__main__":
    pass


# revision 8
# speedup vs baseline: 11.6355x; 1.2342x over previous
"""HNHN hypergraph model on 8 Trainium2 NeuronCores (Bass/Tile).

Wall-time-optimized: the dominant cost of a warm run is the host->device
upload through the axon tunnel (~105 MB/s). The binary incidence matrix is
bit-packed on host (512MB fp32 -> 16MB u8) and unpacked to bf16 on device;
weights/biases are consolidated into one small array.

Sharding: rows (nodes) of the incidence matrix and x0 over 8 cores.
Pipeline per core (v = own 1024 nodes, e = all 16384 hyperedges):
  P0   : stream packed bits -> unpack to bf16 B16 tile (strided bit-planes)
         -> row sums (node_deg) -> DRAM scratch B16
  S0   : Y0 = x0 @ W01_0, S0 = [Y0*v_beta | v_beta | 1]  (hi/lo bf16 split)
  PA   : U0' = B^T S0 partials (PSUM accum over own v)  -> ReduceScatter(add)
         U0'[64] = beta_denom partials, U0'[65] = edge_card partials
  mid0 : per e-shard: x1 = relu(U0/beta_denom + b01_0); Z0' = [x1@W10_0*e_a | e_a]
         hi/lo bf16 -> AllGather
  PB   : V0'^T = Z0'^T B^T via transposed-DMA reads of B16 (accum over all e)
         row 64 = alpha_denom (local);  x = relu(V0/alpha_denom + b10_0)
  PC   : layer-1 edge pass (same as PA, no extra cols) -> ReduceScatter
  mid1 : x1_l2 = relu(U1/beta_denom + b01_1); Z1' hi/lo -> AllGather
  PD   : V1^T accum;  x2 = relu(V1/alpha_denom + b10_1)
  fin  : per-core max-pool -> AllReduce(max) -> pooled @ Wout + bout
"""

import numpy as np

import concourse.bass as bass
import concourse.bacc as bacc
import concourse.mybir as mybir
import concourse.tile as tile
from concourse.bass_utils import run_bass_kernel_spmd
from concourse.masks import make_identity

F32 = mybir.dt.float32
BF16 = mybir.dt.bfloat16
U8 = mybir.dt.uint8
AF = mybir.ActivationFunctionType
OP = mybir.AluOpType

N, E, D, H = 8192, 16384, 128, 64
NCORES = 8
# packed weight array column map (see _make_in_maps); all blocks start at
# partition row 0 so on-device slices never cross partition offsets
WCOL_W01_0 = 0            # [128, 64]   rows 0:128
WCOL_W10_0 = 64           # [64, 64]    rows 0:64
WCOL_W01_1 = 128          # [64, 64]    rows 0:64
WCOL_W10_1 = 192          # [64, 64]    rows 0:64
WCOL_B01_0 = 256          # [64, 1]
WCOL_B10_0 = 257          # [64, 1]
WCOL_B01_1 = 258          # [64, 1]
WCOL_B10_1 = 259          # [64, 1]
WCOL_WOUT = 260           # [64, 1]
WCOL_BOUT = 261           # [1, 1]
WPACK_COLS = 262


def _hi_lo(nc, pool, src_f32, dst_hi, dst_lo, tmp_shape):
    """Split fp32 tile into hi+lo bf16 (dst_hi + dst_lo ~= src exactly)."""
    nc.vector.tensor_copy(out=dst_hi, in_=src_f32)
    hf = pool.tile(tmp_shape, F32, tag="hilo_hf")
    nc.vector.tensor_copy(out=hf, in_=dst_hi)
    lof = pool.tile(tmp_shape, F32, tag="hilo_lof")
    nc.vector.tensor_sub(out=lof, in0=src_f32, in1=hf)
    nc.vector.tensor_copy(out=dst_lo, in_=lof)


def build_kernel(ncores=NCORES, n_edges=E, nloc=N // NCORES,
                 stop_after=None):
    EE = n_edges
    EB = EE // 8                 # packed bytes per row
    NVT = nloc // 128            # v-tiles per core
    ESH = EE // ncores           # e-shard per core
    NET = EE // 128              # 128-wide e-tiles
    PASUP = min(2048, EE)        # PA/PC streaming super width
    NSUP = EE // PASUP
    TSUP = min(1024, EE)         # PB/PD transposed-read super width
    NTSUP = EE // TSUP
    ETL = TSUP // 128            # e-tiles per transposed read
    CW = min(512, nloc)          # column chunk for nloc-wide ops
    NCH = nloc // CW
    FW = ESH // 128              # fold width for shard-scalar math
    GROUPS = [list(range(ncores))]

    nc = bacc.Bacc("TRN2", target_bir_lowering=False, debug=False,
                   num_devices=ncores)

    x0 = nc.declare_dram_parameter("x0", [nloc, D], F32, isOutput=False)
    bits = nc.declare_dram_parameter("bits", [nloc, EB], U8, isOutput=False)
    wpk = nc.declare_dram_parameter("wpack", [128, WPACK_COLS], F32,
                                    isOutput=False)
    out = nc.declare_dram_parameter("out", [1, 1], F32, isOutput=True)

    B16 = nc.dram_tensor("b16", [nloc, EE], BF16)

    with tile.TileContext(nc, num_cores=ncores) as tc:
        with tc.tile_pool(name="persist", bufs=1) as pp, \
             tc.tile_pool(name="dram", bufs=1, space="DRAM") as dp:
            # ---- constants / weights ----
            id_f32 = pp.tile([128, 128], F32, tag="id_f32")
            make_identity(nc, id_f32[:])
            id_bf16 = pp.tile([128, 128], BF16, tag="id_bf16")
            make_identity(nc, id_bf16[:])
            wall = pp.tile([128, WPACK_COLS], F32, tag="wall")
            nc.sync.dma_start(out=wall[:], in_=wpk[:])
            w01_0 = wall[:, WCOL_W01_0:WCOL_W01_0 + H]            # [128,64]
            w10_0 = wall[0:H, WCOL_W10_0:WCOL_W10_0 + H]          # [64,64]
            w01_1 = wall[0:H, WCOL_W01_1:WCOL_W01_1 + H]
            w10_1 = wall[0:H, WCOL_W10_1:WCOL_W10_1 + H]
            bb01_0 = pp.tile([H, 1], F32, tag="bb01_0")
            nc.vector.tensor_copy(out=bb01_0[:],
                                  in_=wall[0:H, WCOL_B01_0:WCOL_B01_0 + 1])
            bb10_0 = pp.tile([H, 1], F32, tag="bb10_0")
            nc.vector.tensor_copy(out=bb10_0[:],
                                  in_=wall[0:H, WCOL_B10_0:WCOL_B10_0 + 1])
            bb01_1 = pp.tile([H, 1], F32, tag="bb01_1")
            nc.vector.tensor_copy(out=bb01_1[:],
                                  in_=wall[0:H, WCOL_B01_1:WCOL_B01_1 + 1])
            bb10_1 = pp.tile([H, 1], F32, tag="bb10_1")
            nc.vector.tensor_copy(out=bb10_1[:],
                                  in_=wall[0:H, WCOL_B10_1:WCOL_B10_1 + 1])
            wout = pp.tile([H, 1], F32, tag="wout")
            nc.vector.tensor_copy(out=wout[:],
                                  in_=wall[0:H, WCOL_WOUT:WCOL_WOUT + 1])
            bbout = pp.tile([1, 1], F32, tag="bbout")
            nc.vector.tensor_copy(out=bbout[:],
                                  in_=wall[0:1, WCOL_BOUT:WCOL_BOUT + 1])

            # ---- persistent small state ----
            deg_all = pp.tile([128, NVT], F32, tag="deg_all")
            vb_all = pp.tile([128, NVT], F32, tag="vb_all")
            s0h = pp.tile([128, NVT, H + 2], BF16, tag="s0h")
            s0l = pp.tile([128, NVT, H + 2], BF16, tag="s0l")
            s1h = pp.tile([128, NVT, H], BF16, tag="s1h")
            s1l = pp.tile([128, NVT, H], BF16, tag="s1l")
            rbB = pp.tile([H, ESH], F32, tag="rbB")      # 1/beta_denom bcast
            eaB = pp.tile([H, ESH], F32, tag="eaB")      # e_alpha bcast
            raB = pp.tile([H, nloc], F32, tag="raB")     # 1/alpha_denom bcast
            vbB = pp.tile([H, nloc], F32, tag="vbB")     # v_beta bcast (free)

            # ====== P0: unpack bits -> bf16 B16 + row sums (node_deg) ======
            with tc.tile_pool(name="p0", bufs=2) as p0:
                for vt in range(NVT):
                    bt = p0.tile([128, EB], U8, tag="p0bits")
                    nc.sync.dma_start(
                        out=bt[:], in_=bits[vt * 128:(vt + 1) * 128, :])
                    ub = p0.tile([128, EE], BF16, tag="p0ub")
                    ubv = ub[:].rearrange("p (j t) -> p t j", t=8)
                    for t in range(8):
                        m = p0.tile([128, EB], U8, tag="p0m")
                        nc.vector.tensor_scalar(
                            out=m[:], in0=bt[:], scalar1=1 << t,
                            scalar2=None, op0=OP.bitwise_and)
                        nc.vector.tensor_scalar(
                            out=ubv[:, t, :], in0=m[:], scalar1=0,
                            scalar2=None, op0=OP.is_gt)
                    nc.vector.tensor_reduce(
                        out=deg_all[:, vt:vt + 1], in_=ub[:],
                        axis=mybir.AxisListType.X, op=OP.add)
                    nc.sync.dma_start(
                        out=B16[vt * 128:(vt + 1) * 128, :], in_=ub[:])

            if stop_after == "p0":
                with tc.tile_pool(name="ee", bufs=1) as ee:
                    e1 = ee.tile([1, 1], F32, tag="e1")
                    nc.vector.tensor_copy(out=e1[:], in_=deg_all[0:1, 0:1])
                    nc.sync.dma_start(out=out[:], in_=e1[:])
                nc.compile()
                return nc

            # node_deg -> v_beta
            with tc.tile_pool(name="vbp", bufs=1) as vbp:
                degc = vbp.tile([128, NVT], F32, tag="degc")
                nc.vector.tensor_scalar_max(out=degc[:], in0=deg_all[:],
                                            scalar1=1.0)
                sqd = vbp.tile([128, NVT], F32, tag="sqd")
                nc.scalar.sqrt(out=sqd[:], in_=degc[:])
                nc.vector.reciprocal(out=vb_all[:], in_=sqd[:])
                # v_beta to free-layout DRAM row then broadcast into vbB
                with tc.tile_pool(name="vbps", bufs=1, space="PSUM") as vps:
                    pt = vps.tile([NVT, 128], F32, tag="vb_t")
                    nc.tensor.transpose(pt[:], vb_all[:], id_f32[:])
                    vb8 = vbp.tile([NVT, 128], F32, tag="vb8")
                    nc.vector.tensor_copy(out=vb8[:], in_=pt[:])
                vrow = dp.tile([1, nloc], F32, tag="vrow")
                nc.gpsimd.dma_start(
                    out=vrow[:].rearrange("a (b c) -> (a b) c", b=NVT),
                    in_=vb8[:])
                nc.gpsimd.dma_start(out=vbB[:],
                                    in_=vrow[:].to_broadcast([H, nloc]))

            # ================= S0 prep ==================================
            with tc.tile_pool(name="s0p", bufs=2) as sp, \
                 tc.tile_pool(name="s0ps", bufs=2, space="PSUM") as sps:
                for vt in range(NVT):
                    xt = sp.tile([128, D], F32, tag="xt")
                    nc.sync.dma_start(out=xt[:],
                                      in_=x0[vt * 128:(vt + 1) * 128, :])
                    pxt = sps.tile([D, 128], F32, tag="pxt")
                    nc.tensor.transpose(pxt[:], xt[:], id_f32[:])
                    x0T = sp.tile([D, 128], F32, tag="x0T")
                    nc.vector.tensor_copy(out=x0T[:], in_=pxt[:])
                    py = sps.tile([128, H], F32, tag="py")
                    nc.tensor.matmul(py[:], lhsT=x0T[:], rhs=w01_0,
                                     start=True, stop=True)
                    s0f = sp.tile([128, H + 2], F32, tag="s0f")
                    nc.vector.tensor_scalar_mul(out=s0f[:, 0:H], in0=py[:],
                                                scalar1=vb_all[:, vt:vt + 1])
                    nc.vector.tensor_copy(out=s0f[:, H:H + 1],
                                          in_=vb_all[:, vt:vt + 1])
                    nc.vector.memset(s0f[:, H + 1:H + 2], 1.0)
                    _hi_lo(nc, sp, s0f[:], s0h[:, vt, :], s0l[:, vt, :],
                           [128, H + 2])

            # ================= PA: U0' = B^T S0 (+ RS) ==================
            u0s_d = dp.tile([H + 2, ESH], F32, tag="u0s_d")
            with tc.tile_pool(name="pa", bufs=2) as pa, \
                 tc.tile_pool(name="pa_acc", bufs=1) as paa, \
                 tc.tile_pool(name="paps", bufs=2, space="PSUM") as paps:
                u0acc = paa.tile([H + 2, EE], F32, tag="u0acc")
                for sup in range(NSUP):
                    bt = pa.tile([128, NVT, PASUP], BF16, tag="pa_bt")
                    nc.sync.dma_start(
                        out=bt[:],
                        in_=B16[:, sup * PASUP:(sup + 1) * PASUP].rearrange(
                            "(vt p) e -> p vt e", p=128))
                    pu = paps.tile([H + 2, PASUP], F32, tag="pa_pu")
                    for c in range(PASUP // 512):
                        for vt in range(NVT):
                            for hl, st in ((0, s0h), (1, s0l)):
                                nc.tensor.matmul(
                                    pu[:, c * 512:(c + 1) * 512],
                                    lhsT=st[:, vt, :],
                                    rhs=bt[:, vt, c * 512:(c + 1) * 512],
                                    start=(vt == 0 and hl == 0),
                                    stop=(vt == NVT - 1 and hl == 1))
                    nc.vector.tensor_copy(
                        out=u0acc[:, sup * PASUP:(sup + 1) * PASUP],
                        in_=pu[:])
                bu0 = dp.tile([ncores, H + 2, ESH], F32, tag="bu0")
                nc.sync.dma_start(
                    out=bu0[:].rearrange("s h e -> h s e"),
                    in_=u0acc[:].rearrange("h (s e) -> h s e", s=ncores))
                nc.gpsimd.collective_compute(
                    "ReduceScatter", OP.add, replica_groups=GROUPS,
                    ins=[bu0.opt()], outs=[u0s_d.opt()])

            if stop_after == "pa":
                with tc.tile_pool(name="ee", bufs=1) as ee:
                    e1 = ee.tile([1, 1], F32, tag="e1")
                    nc.gpsimd.dma_start(out=e1[:], in_=u0s_d[0:1, 0:1])
                    nc.sync.dma_start(out=out[:], in_=e1[:])
                nc.compile()
                return nc

            # ================= mid0: shard scalars + Z0' ================
            zg = dp.tile([ncores, 2, H + 1, ESH], BF16, tag="zg")
            with tc.tile_pool(name="m0", bufs=1) as m0, \
                 tc.tile_pool(name="m0ps", bufs=2, space="PSUM") as m0ps:
                u0s = m0.tile([H + 2, ESH], F32, tag="u0s")
                nc.scalar.dma_start(out=u0s[:], in_=u0s_d[:])
                # 1/beta_denom (guard 0 -> 1), via folded layout
                bd128 = m0.tile([128, FW], F32, tag="bd128")
                nc.gpsimd.dma_start(
                    out=bd128[:],
                    in_=u0s_d[H:H + 1, :].rearrange("a (p c) -> (a p) c",
                                                    p=128))
                msk = m0.tile([128, FW], F32, tag="msk")
                nc.vector.tensor_scalar(out=msk[:], in0=bd128[:], scalar1=0.0,
                                        scalar2=None, op0=OP.is_equal)
                nc.vector.tensor_add(out=bd128[:], in0=bd128[:], in1=msk[:])
                rb128 = m0.tile([128, FW], F32, tag="rb128")
                nc.vector.reciprocal(out=rb128[:], in_=bd128[:])
                rbrow = dp.tile([1, ESH], F32, tag="rbrow")
                nc.gpsimd.dma_start(
                    out=rbrow[:].rearrange("a (p c) -> (a p) c", p=128),
                    in_=rb128[:])
                nc.gpsimd.dma_start(out=rbB[:],
                                    in_=rbrow[:].to_broadcast([H, ESH]))
                # e_alpha = ecard'^-1.5 (guard 0 -> 1)
                ec128 = m0.tile([128, FW], F32, tag="ec128")
                nc.gpsimd.dma_start(
                    out=ec128[:],
                    in_=u0s_d[H + 1:H + 2, :].rearrange("a (p c) -> (a p) c",
                                                        p=128))
                nc.vector.tensor_scalar_max(out=ec128[:], in0=ec128[:],
                                            scalar1=1.0)
                sq = m0.tile([128, FW], F32, tag="sq")
                nc.scalar.sqrt(out=sq[:], in_=ec128[:])
                nc.vector.tensor_mul(out=sq[:], in0=sq[:], in1=ec128[:])
                ea128 = m0.tile([128, FW], F32, tag="ea128")
                nc.vector.reciprocal(out=ea128[:], in_=sq[:])
                earow = dp.tile([1, ESH], F32, tag="earow")
                nc.gpsimd.dma_start(
                    out=earow[:].rearrange("a (p c) -> (a p) c", p=128),
                    in_=ea128[:])
                nc.gpsimd.dma_start(out=eaB[:],
                                    in_=earow[:].to_broadcast([H, ESH]))
                # x1 shard (transposed layout [H, ESH])
                xs = m0.tile([H, ESH], F32, tag="xs")
                nc.vector.tensor_mul(out=xs[:], in0=u0s[0:H, :], in1=rbB[:])
                nc.scalar.activation(out=xs[:], in_=xs[:], func=AF.Relu,
                                     bias=bb01_0[:])
                # Z0'^T = (W10_0^T x1^T) * e_alpha ; extra row = e_alpha
                m0f = m0.tile([H + 1, ESH], F32, tag="m0f")
                for c in range(max(1, ESH // 512)):
                    zp = m0ps.tile([H, min(512, ESH)], F32, tag="zp")
                    nc.tensor.matmul(zp[:], lhsT=w10_0,
                                     rhs=xs[:, c * 512:(c + 1) * 512],
                                     start=True, stop=True)
                    nc.vector.tensor_mul(out=m0f[0:H, c * 512:(c + 1) * 512],
                                         in0=zp[:],
                                         in1=eaB[:, c * 512:(c + 1) * 512])
                nc.gpsimd.dma_start(out=m0f[H:H + 1, :], in_=earow[:])
                m0h = m0.tile([H + 1, ESH], BF16, tag="m0h")
                m0l = m0.tile([H + 1, ESH], BF16, tag="m0l")
                _hi_lo(nc, m0, m0f[:], m0h[:], m0l[:], [H + 1, ESH])
                bz = dp.tile([2, H + 1, ESH], BF16, tag="bz")
                nc.gpsimd.dma_start(out=bz[0], in_=m0h[:])
                nc.gpsimd.dma_start(out=bz[1], in_=m0l[:])
                nc.gpsimd.collective_compute(
                    "AllGather", OP.bypass, replica_groups=GROUPS,
                    ins=[bz.opt()], outs=[zg.opt()])

            # ================= PB: V0'^T = Z0'^T B^T ====================
            with tc.tile_pool(name="pbz", bufs=1) as pbz, \
                 tc.tile_pool(name="pb", bufs=2) as pb, \
                 tc.tile_pool(name="pbps", bufs=1, space="PSUM") as pbps, \
                 tc.tile_pool(name="pbps2", bufs=2, space="PSUM") as pbps2:
                zall = pbz.tile([H + 1, ncores, 2, ESH], BF16, tag="zall")
                nc.scalar.dma_start(
                    out=zall[:], in_=zg[:].rearrange("g h p e -> p g h e"))
                zsth = pbz.tile([128, NET, H + 1], BF16, tag="zsth")
                zstl = pbz.tile([128, NET, H + 1], BF16, tag="zstl")
                for et in range(NET):
                    g, w = divmod(et, ESH // 128)
                    for h, dst in ((0, zsth), (1, zstl)):
                        ptz = pbps2.tile([128, H + 1], BF16, tag="ptz")
                        nc.tensor.transpose(
                            ptz[:], zall[:, g, h, w * 128:(w + 1) * 128],
                            id_bf16[:H + 1, :H + 1])
                        nc.vector.tensor_copy(out=dst[:, et, :], in_=ptz[:])
                vp = pbps.tile([H + 1, nloc], F32, tag="vp")
                for sup in range(NTSUP):
                    btile = pb.tile([128, ETL, nloc], BF16, tag="pb_bt")
                    nc.sync.dma_start_transpose(
                        btile[:], B16[:, sup * TSUP:(sup + 1) * TSUP])
                    for etl in range(ETL):
                        et = sup * ETL + etl
                        for h, st in ((0, zsth), (1, zstl)):
                            for c in range(NCH):
                                nc.tensor.matmul(
                                    vp[:, c * CW:(c + 1) * CW],
                                    lhsT=st[:, et, :],
                                    rhs=btile[:, etl, c * CW:(c + 1) * CW],
                                    start=(et == 0 and h == 0),
                                    stop=(et == NET - 1 and h == 1))
                # alpha_denom -> 1/ad broadcast ; x = relu(V0/ad + b10_0)
                with tc.tile_pool(name="pbs", bufs=1) as pbs:
                    adm = pbs.tile([1, nloc], F32, tag="adm")
                    nc.vector.tensor_scalar(out=adm[:], in0=vp[H:H + 1, :],
                                            scalar1=0.0, scalar2=None,
                                            op0=OP.is_equal)
                    nc.vector.tensor_add(out=adm[:], in0=adm[:],
                                         in1=vp[H:H + 1, :])
                    ra = pbs.tile([1, nloc], F32, tag="ra")
                    nc.vector.reciprocal(out=ra[:], in_=adm[:])
                    rarow = dp.tile([1, nloc], F32, tag="rarow")
                    nc.gpsimd.dma_start(out=rarow[:], in_=ra[:])
                    nc.gpsimd.dma_start(out=raB[:],
                                        in_=rarow[:].to_broadcast([H, nloc]))
                    xl1 = pbs.tile([H, nloc], F32, tag="xl1")
                    nc.vector.tensor_mul(out=xl1[:], in0=vp[0:H, :],
                                         in1=raB[:])
                    nc.scalar.activation(out=xl1[:], in_=xl1[:], func=AF.Relu,
                                         bias=bb10_0[:])
                    # S1^T = (W01_1^T x^T) * v_beta
                    s1tf = pbs.tile([H, nloc], F32, tag="s1tf")
                    for c in range(NCH):
                        yp = pbps2.tile([H, CW], F32, tag="yp")
                        nc.tensor.matmul(yp[:], lhsT=w01_1,
                                         rhs=xl1[:, c * CW:(c + 1) * CW],
                                         start=True, stop=True)
                        nc.vector.tensor_mul(
                            out=s1tf[:, c * CW:(c + 1) * CW], in0=yp[:],
                            in1=vbB[:, c * CW:(c + 1) * CW])
                    s1th = pbs.tile([H, nloc], BF16, tag="s1th")
                    s1tl = pbs.tile([H, nloc], BF16, tag="s1tl")
                    _hi_lo(nc, pbs, s1tf[:], s1th[:], s1tl[:], [H, nloc])
                    for vt in range(NVT):
                        for src, dst in ((s1th, s1h), (s1tl, s1l)):
                            pts = pbps2.tile([128, H], BF16, tag="pts")
                            nc.tensor.transpose(
                                pts[:], src[:, vt * 128:(vt + 1) * 128],
                                id_bf16[:H, :H])
                            nc.vector.tensor_copy(out=dst[:, vt, :],
                                                  in_=pts[:])

            if stop_after == "pb":
                with tc.tile_pool(name="ee", bufs=1) as ee:
                    e1 = ee.tile([1, 1], F32, tag="e1")
                    nc.vector.tensor_copy(out=e1[:], in_=s1h[0:1, 0, 0:1])
                    nc.sync.dma_start(out=out[:], in_=e1[:])
                nc.compile()
                return nc

            # ================= PC: U1' = B^T S1 (+ RS) ==================
            u1s_d = dp.tile([H, ESH], F32, tag="u1s_d")
            with tc.tile_pool(name="pc", bufs=2) as pc, \
                 tc.tile_pool(name="pc_acc", bufs=1) as pca, \
                 tc.tile_pool(name="pcps", bufs=2, space="PSUM") as pcps:
                u1acc = pca.tile([H, EE], F32, tag="u1acc")
                for sup in range(NSUP):
                    bt = pc.tile([128, NVT, PASUP], BF16, tag="pc_bt")
                    nc.sync.dma_start(
                        out=bt[:],
                        in_=B16[:, sup * PASUP:(sup + 1) * PASUP].rearrange(
                            "(vt p) e -> p vt e", p=128))
                    pu = pcps.tile([H, PASUP], F32, tag="pc_pu")
                    for c in range(PASUP // 512):
                        for vt in range(NVT):
                            for hl, st in ((0, s1h), (1, s1l)):
                                nc.tensor.matmul(
                                    pu[:, c * 512:(c + 1) * 512],
                                    lhsT=st[:, vt, :],
                                    rhs=bt[:, vt, c * 512:(c + 1) * 512],
                                    start=(vt == 0 and hl == 0),
                                    stop=(vt == NVT - 1 and hl == 1))
                    nc.vector.tensor_copy(
                        out=u1acc[:, sup * PASUP:(sup + 1) * PASUP],
                        in_=pu[:])
                bu1 = dp.tile([ncores, H, ESH], F32, tag="bu1")
                nc.sync.dma_start(
                    out=bu1[:].rearrange("s h e -> h s e"),
                    in_=u1acc[:].rearrange("h (s e) -> h s e", s=ncores))
                nc.gpsimd.collective_compute(
                    "ReduceScatter", OP.add, replica_groups=GROUPS,
                    ins=[bu1.opt()], outs=[u1s_d.opt()])

            # ================= mid1 =====================================
            z1g = dp.tile([ncores, 2, H, ESH], BF16, tag="z1g")
            with tc.tile_pool(name="m1", bufs=1) as m1, \
                 tc.tile_pool(name="m1ps", bufs=2, space="PSUM") as m1ps:
                u1s = m1.tile([H, ESH], F32, tag="u1s")
                nc.scalar.dma_start(out=u1s[:], in_=u1s_d[:])
                xs2 = m1.tile([H, ESH], F32, tag="xs2")
                nc.vector.tensor_mul(out=xs2[:], in0=u1s[:], in1=rbB[:])
                nc.scalar.activation(out=xs2[:], in_=xs2[:], func=AF.Relu,
                                     bias=bb01_1[:])
                m1f = m1.tile([H, ESH], F32, tag="m1f")
                for c in range(max(1, ESH // 512)):
                    zp1 = m1ps.tile([H, min(512, ESH)], F32, tag="zp1")
                    nc.tensor.matmul(zp1[:], lhsT=w10_1,
                                     rhs=xs2[:, c * 512:(c + 1) * 512],
                                     start=True, stop=True)
                    nc.vector.tensor_mul(out=m1f[:, c * 512:(c + 1) * 512],
                                         in0=zp1[:],
                                         in1=eaB[:, c * 512:(c + 1) * 512])
                m1h = m1.tile([H, ESH], BF16, tag="m1h")
                m1l = m1.tile([H, ESH], BF16, tag="m1l")
                _hi_lo(nc, m1, m1f[:], m1h[:], m1l[:], [H, ESH])
                bz1 = dp.tile([2, H, ESH], BF16, tag="bz1")
                nc.gpsimd.dma_start(out=bz1[0], in_=m1h[:])
                nc.gpsimd.dma_start(out=bz1[1], in_=m1l[:])
                nc.gpsimd.collective_compute(
                    "AllGather", OP.bypass, replica_groups=GROUPS,
                    ins=[bz1.opt()], outs=[z1g.opt()])

            # ================= PD: V1^T + finale ========================
            with tc.tile_pool(name="pdz", bufs=1) as pdz, \
                 tc.tile_pool(name="pd", bufs=2) as pd, \
                 tc.tile_pool(name="pdps", bufs=1, space="PSUM") as pdps, \
                 tc.tile_pool(name="pdps2", bufs=2, space="PSUM") as pdps2:
                z1all = pdz.tile([H, ncores, 2, ESH], BF16, tag="z1all")
                nc.scalar.dma_start(
                    out=z1all[:], in_=z1g[:].rearrange("g h p e -> p g h e"))
                z1sth = pdz.tile([128, NET, H], BF16, tag="z1sth")
                z1stl = pdz.tile([128, NET, H], BF16, tag="z1stl")
                for et in range(NET):
                    g, w = divmod(et, ESH // 128)
                    for h, dst in ((0, z1sth), (1, z1stl)):
                        ptz = pdps2.tile([128, H], BF16, tag="ptz1")
                        nc.tensor.transpose(
                            ptz[:], z1all[:, g, h, w * 128:(w + 1) * 128],
                            id_bf16[:H, :H])
                        nc.vector.tensor_copy(out=dst[:, et, :], in_=ptz[:])
                vp1 = pdps.tile([H, nloc], F32, tag="vp1")
                for sup in range(NTSUP):
                    btile = pd.tile([128, ETL, nloc], BF16, tag="pd_bt")
                    nc.sync.dma_start_transpose(
                        btile[:], B16[:, sup * TSUP:(sup + 1) * TSUP])
                    for etl in range(ETL):
                        et = sup * ETL + etl
                        for h, st in ((0, z1sth), (1, z1stl)):
                            for c in range(NCH):
                                nc.tensor.matmul(
                                    vp1[:, c * CW:(c + 1) * CW],
                                    lhsT=st[:, et, :],
                                    rhs=btile[:, etl, c * CW:(c + 1) * CW],
                                    start=(et == 0 and h == 0),
                                    stop=(et == NET - 1 and h == 1))
                with tc.tile_pool(name="fin", bufs=1) as fin:
                    x2 = fin.tile([H, nloc], F32, tag="x2")
                    nc.vector.tensor_mul(out=x2[:], in0=vp1[:], in1=raB[:])
                    nc.scalar.activation(out=x2[:], in_=x2[:], func=AF.Relu,
                                         bias=bb10_1[:])
                    pool_p = fin.tile([H, 1], F32, tag="pool_p")
                    nc.vector.tensor_reduce(out=pool_p[:], in_=x2[:],
                                            axis=mybir.AxisListType.X,
                                            op=OP.max)
                    bp = dp.tile([H, 1], F32, tag="bp")
                    nc.gpsimd.dma_start(out=bp[:], in_=pool_p[:])
                    bpo = dp.tile([H, 1], F32, tag="bpo")
                    nc.gpsimd.collective_compute(
                        "AllReduce", OP.max, replica_groups=GROUPS,
                        ins=[bp.opt()], outs=[bpo.opt()])
                    pooled = fin.tile([H, 1], F32, tag="pooled")
                    nc.gpsimd.dma_start(out=pooled[:], in_=bpo[:])
                    po = pdps2.tile([1, 1], F32, tag="po")
                    nc.tensor.matmul(po[:], lhsT=pooled[:], rhs=wout[:],
                                     start=True, stop=True)
                    ob = fin.tile([1, 1], F32, tag="ob")
                    nc.vector.tensor_add(out=ob[:], in0=po[:], in1=bbout[:])
                    nc.sync.dma_start(out=out[:], in_=ob[:])

    nc.compile()
    return nc


_NC_CACHE = {}


def _get_nc():
    if "nc" not in _NC_CACHE:
        _NC_CACHE["nc"] = build_kernel()
    return _NC_CACHE["nc"]


def _make_in_maps(inputs, ncores=NCORES, nloc=N // NCORES):
    x0 = np.asarray(inputs["x0"], np.float32)
    inc = np.asarray(inputs["incidence"])
    bits = np.packbits(inc != 0, axis=1, bitorder="little")  # [N, E//8] u8
    wpack = np.zeros((128, WPACK_COLS), np.float32)
    wpack[:, WCOL_W01_0:WCOL_W01_0 + H] = np.asarray(inputs["W01_0"],
                                                     np.float32)
    wpack[0:H, WCOL_W10_0:WCOL_W10_0 + H] = np.asarray(inputs["W10_0"],
                                                       np.float32)
    wpack[0:H, WCOL_W01_1:WCOL_W01_1 + H] = np.asarray(inputs["W01_1"],
                                                       np.float32)
    wpack[0:H, WCOL_W10_1:WCOL_W10_1 + H] = np.asarray(inputs["W10_1"],
                                                       np.float32)
    wpack[0:H, WCOL_B01_0] = np.asarray(inputs["b01_0"],
                                        np.float32).reshape(-1)
    wpack[0:H, WCOL_B10_0] = np.asarray(inputs["b10_0"],
                                        np.float32).reshape(-1)
    wpack[0:H, WCOL_B01_1] = np.asarray(inputs["b01_1"],
                                        np.float32).reshape(-1)
    wpack[0:H, WCOL_B10_1] = np.asarray(inputs["b10_1"],
                                        np.float32).reshape(-1)
    wpack[0:H, WCOL_WOUT] = np.asarray(inputs["Wout"], np.float32).reshape(-1)
    wpack[0:1, WCOL_BOUT] = np.asarray(inputs["bout"], np.float32).reshape(-1)
    in_maps = []
    for c in range(ncores):
        m = {"x0": np.ascontiguousarray(x0[c * nloc:(c + 1) * nloc]),
             "bits": np.ascontiguousarray(bits[c * nloc:(c + 1) * nloc]),
             "wpack": wpack}
        in_maps.append(m)
    return in_maps


def kernel(**inputs) -> np.ndarray:
    nc = _get_nc()
    in_maps = _make_in_maps(inputs)
    res = run_bass_kernel_spmd(nc, in_maps, list(range(NCORES)))
    return res.results[0]["out"].reshape(1).astype(np.float32)


if __name__ == "__main__":
    pass


# revision 9
# speedup vs baseline: 11.8868x; 1.0216x over previous
"""HNHN hypergraph model on 8 Trainium2 NeuronCores (Bass/Tile), v2.

Wall-time-optimized: warm-run wall is dominated by host->device upload
through the axon tunnel (~105-150 MB/s) plus a ~190ms dispatch floor, so
the binary incidence matrix is bit-packed on host (512MB fp32 -> 16MB u8,
exact) and unpacked to bf16 on device; weights/biases ride in one small
array.  Device work is single-bf16 (tolerance 2e-2; measured pipeline
error ~1.3e-3) and each layer needs just ONE f32 AllReduce of the edge
features: every core then computes the (tiny) full-width edge-stage math
redundantly, which removes the ReduceScatter/AllGather pair and the
e-shard scatter/gather layout gymnastics.

Sharding: rows (nodes) of the incidence matrix and x0 over 8 cores.
Pipeline per core (v = own 1024 nodes, e = all 16384 hyperedges):
  P0   : unpack packed bits -> bf16 B16 tile (strided bit-planes)
         -> row sums (node_deg) -> DRAM scratch B16
  S0   : Y0 = x0 @ W01_0, S0 = [Y0*v_beta | v_beta | 1]  bf16
  PA   : U0' = B^T S0 partials (PSUM accum over own v) -> AllReduce(add)
         U0'[64] = beta_denom partials, U0'[65] = edge_card partials
  mid0 : full width on every core: x1 = relu(U0/beta_denom + b01_0)
         Z0' = [x1@W10_0 * e_alpha ; e_alpha]  bf16  (no collective)
  PB   : V0'^T = Z0'^T B^T via transposed-DMA reads of B16 (accum, all e)
         row 64 = alpha_denom;  x = relu(V0/alpha_denom + b10_0)
  PC   : U1' = B^T S1 partials -> AllReduce(add)
  mid1 : x1_l2 = relu(U1/beta_denom + b01_1); Z1' = x1_l2@W10_1 * e_alpha
  PD   : V1^T accum;  x2 = relu(V1/alpha_denom + b10_1)
  fin  : per-core max-pool -> AllReduce(max) -> pooled @ Wout + bout
"""

import numpy as np

import concourse.bass as bass
import concourse.bacc as bacc
import concourse.mybir as mybir
import concourse.tile as tile
from concourse.bass_utils import run_bass_kernel_spmd
from concourse.masks import make_identity

F32 = mybir.dt.float32
BF16 = mybir.dt.bfloat16
U8 = mybir.dt.uint8
AF = mybir.ActivationFunctionType
OP = mybir.AluOpType

N, E, D, H = 8192, 16384, 128, 64
NCORES = 8
# packed weight array column map (see _make_in_maps); all blocks start at
# partition row 0 so on-device slices never cross partition offsets
WCOL_W01_0 = 0            # [128, 64]   rows 0:128
WCOL_W10_0 = 64           # [64, 64]    rows 0:64
WCOL_W01_1 = 128          # [64, 64]    rows 0:64
WCOL_W10_1 = 192          # [64, 64]    rows 0:64
WCOL_B01_0 = 256          # [64, 1]
WCOL_B10_0 = 257          # [64, 1]
WCOL_B01_1 = 258          # [64, 1]
WCOL_B10_1 = 259          # [64, 1]
WCOL_WOUT = 260           # [64, 1]
WCOL_BOUT = 261           # [1, 1]
WPACK_COLS = 262


def build_kernel(ncores=NCORES, n_edges=E, nloc=N // NCORES):
    EE = n_edges
    EB = EE // 8                 # packed bytes per row
    NVT = nloc // 128            # v-tiles per core
    NET = EE // 128              # 128-wide e-tiles
    PASUP = min(2048, EE)        # PA/PC streaming super width
    NSUP = EE // PASUP
    TSUP = min(1024, EE)         # PB/PD transposed-read super width
    NTSUP = EE // TSUP
    ETL = TSUP // 128            # e-tiles per transposed read
    CW = min(512, nloc)          # column chunk for nloc-wide ops
    NCH = nloc // CW
    MCH = 2048                   # mid-phase e-chunk
    NMCH = EE // MCH
    FW = EE // 128               # fold width for full-width scalar math
    GROUPS = [list(range(ncores))]

    nc = bacc.Bacc("TRN2", target_bir_lowering=False, debug=False,
                   num_devices=ncores)

    x0 = nc.declare_dram_parameter("x0", [nloc, D], F32, isOutput=False)
    bits = nc.declare_dram_parameter("bits", [nloc, EB], U8, isOutput=False)
    wpk = nc.declare_dram_parameter("wpack", [128, WPACK_COLS], F32,
                                    isOutput=False)
    out = nc.declare_dram_parameter("out", [1, 1], F32, isOutput=True)

    B16 = nc.dram_tensor("b16", [nloc, EE], BF16)

    with tile.TileContext(nc, num_cores=ncores) as tc:
        with tc.tile_pool(name="persist", bufs=1) as pp, \
             tc.tile_pool(name="dram", bufs=1, space="DRAM") as dp:
            # ---- constants / weights ----
            id_f32 = pp.tile([128, 128], F32, tag="id_f32")
            make_identity(nc, id_f32[:])
            id_bf16 = pp.tile([128, 128], BF16, tag="id_bf16")
            make_identity(nc, id_bf16[:])
            wall = pp.tile([128, WPACK_COLS], F32, tag="wall")
            nc.sync.dma_start(out=wall[:], in_=wpk[:])
            w01_0 = wall[:, WCOL_W01_0:WCOL_W01_0 + H]            # [128,64]
            w10_0 = wall[0:H, WCOL_W10_0:WCOL_W10_0 + H]          # [64,64]
            w01_1 = wall[0:H, WCOL_W01_1:WCOL_W01_1 + H]
            w10_1 = wall[0:H, WCOL_W10_1:WCOL_W10_1 + H]
            bb01_0 = pp.tile([H, 1], F32, tag="bb01_0")
            nc.vector.tensor_copy(out=bb01_0[:],
                                  in_=wall[0:H, WCOL_B01_0:WCOL_B01_0 + 1])
            bb10_0 = pp.tile([H, 1], F32, tag="bb10_0")
            nc.vector.tensor_copy(out=bb10_0[:],
                                  in_=wall[0:H, WCOL_B10_0:WCOL_B10_0 + 1])
            bb01_1 = pp.tile([H, 1], F32, tag="bb01_1")
            nc.vector.tensor_copy(out=bb01_1[:],
                                  in_=wall[0:H, WCOL_B01_1:WCOL_B01_1 + 1])
            bb10_1 = pp.tile([H, 1], F32, tag="bb10_1")
            nc.vector.tensor_copy(out=bb10_1[:],
                                  in_=wall[0:H, WCOL_B10_1:WCOL_B10_1 + 1])
            wout = pp.tile([H, 1], F32, tag="wout")
            nc.vector.tensor_copy(out=wout[:],
                                  in_=wall[0:H, WCOL_WOUT:WCOL_WOUT + 1])
            bbout = pp.tile([1, 1], F32, tag="bbout")
            nc.vector.tensor_copy(out=bbout[:],
                                  in_=wall[0:1, WCOL_BOUT:WCOL_BOUT + 1])

            # ---- persistent small state ----
            deg_all = pp.tile([128, NVT], F32, tag="deg_all")
            vb_all = pp.tile([128, NVT], F32, tag="vb_all")
            s0b = pp.tile([128, NVT, H + 2], BF16, tag="s0b")
            s1b = pp.tile([128, NVT, H], BF16, tag="s1b")
            raB = pp.tile([H, nloc], F32, tag="raB")     # 1/alpha_denom bcast
            vbB = pp.tile([H, nloc], F32, tag="vbB")     # v_beta bcast (free)
            rbrow = dp.tile([1, EE], F32, tag="rbrow")   # 1/beta_denom (DRAM)
            earow = dp.tile([1, EE], F32, tag="earow")   # e_alpha (DRAM)
            earow_b = dp.tile([1, EE], BF16, tag="earow_b")

            # ====== P0: unpack bits -> bf16 B16 + row sums (node_deg) ======
            with tc.tile_pool(name="p0", bufs=2) as p0:
                for vt in range(NVT):
                    bt = p0.tile([128, EB], U8, tag="p0bits")
                    nc.sync.dma_start(
                        out=bt[:], in_=bits[vt * 128:(vt + 1) * 128, :])
                    ub = p0.tile([128, EE], BF16, tag="p0ub")
                    ubv = ub[:].rearrange("p (j t) -> p t j", t=8)
                    for t in range(8):
                        m = p0.tile([128, EB], U8, tag="p0m")
                        nc.vector.tensor_scalar(
                            out=m[:], in0=bt[:], scalar1=1 << t,
                            scalar2=None, op0=OP.bitwise_and)
                        nc.vector.tensor_scalar(
                            out=ubv[:, t, :], in0=m[:], scalar1=0,
                            scalar2=None, op0=OP.is_gt)
                    nc.vector.tensor_reduce(
                        out=deg_all[:, vt:vt + 1], in_=ub[:],
                        axis=mybir.AxisListType.X, op=OP.add)
                    nc.sync.dma_start(
                        out=B16[vt * 128:(vt + 1) * 128, :], in_=ub[:])

            # node_deg -> v_beta
            with tc.tile_pool(name="vbp", bufs=1) as vbp:
                degc = vbp.tile([128, NVT], F32, tag="degc")
                nc.vector.tensor_scalar_max(out=degc[:], in0=deg_all[:],
                                            scalar1=1.0)
                sqd = vbp.tile([128, NVT], F32, tag="sqd")
                nc.scalar.sqrt(out=sqd[:], in_=degc[:])
                nc.vector.reciprocal(out=vb_all[:], in_=sqd[:])
                # v_beta to free-layout DRAM row then broadcast into vbB
                with tc.tile_pool(name="vbps", bufs=1, space="PSUM") as vps:
                    pt = vps.tile([NVT, 128], F32, tag="vb_t")
                    nc.tensor.transpose(pt[:], vb_all[:], id_f32[:])
                    vb8 = vbp.tile([NVT, 128], F32, tag="vb8")
                    nc.vector.tensor_copy(out=vb8[:], in_=pt[:])
                vrow = dp.tile([1, nloc], F32, tag="vrow")
                nc.gpsimd.dma_start(
                    out=vrow[:].rearrange("a (b c) -> (a b) c", b=NVT),
                    in_=vb8[:])
                nc.gpsimd.dma_start(out=vbB[:],
                                    in_=vrow[:].to_broadcast([H, nloc]))

            # ================= S0 prep (single bf16) ====================
            with tc.tile_pool(name="s0p", bufs=2) as sp, \
                 tc.tile_pool(name="s0ps", bufs=2, space="PSUM") as sps:
                for vt in range(NVT):
                    xt = sp.tile([128, D], F32, tag="xt")
                    nc.sync.dma_start(out=xt[:],
                                      in_=x0[vt * 128:(vt + 1) * 128, :])
                    pxt = sps.tile([D, 128], F32, tag="pxt")
                    nc.tensor.transpose(pxt[:], xt[:], id_f32[:])
                    x0T = sp.tile([D, 128], F32, tag="x0T")
                    nc.vector.tensor_copy(out=x0T[:], in_=pxt[:])
                    py = sps.tile([128, H], F32, tag="py")
                    nc.tensor.matmul(py[:], lhsT=x0T[:], rhs=w01_0,
                                     start=True, stop=True)
                    s0f = sp.tile([128, H + 2], F32, tag="s0f")
                    nc.vector.tensor_scalar_mul(out=s0f[:, 0:H], in0=py[:],
                                                scalar1=vb_all[:, vt:vt + 1])
                    nc.vector.tensor_copy(out=s0f[:, H:H + 1],
                                          in_=vb_all[:, vt:vt + 1])
                    nc.vector.memset(s0f[:, H + 1:H + 2], 1.0)
                    nc.vector.tensor_copy(out=s0b[:, vt, :], in_=s0f[:])

            # ================= PA: U0' = B^T S0 -> AllReduce ============
            bo0 = dp.tile([H + 2, EE], F32, tag="bo0")
            with tc.tile_pool(name="pa", bufs=2) as pa, \
                 tc.tile_pool(name="pa_acc", bufs=1) as paa, \
                 tc.tile_pool(name="paps", bufs=2, space="PSUM") as paps:
                u0acc = paa.tile([H + 2, EE], F32, tag="u0acc")
                for sup in range(NSUP):
                    bt = pa.tile([128, NVT, PASUP], BF16, tag="pa_bt")
                    nc.sync.dma_start(
                        out=bt[:],
                        in_=B16[:, sup * PASUP:(sup + 1) * PASUP].rearrange(
                            "(vt p) e -> p vt e", p=128))
                    pu = paps.tile([H + 2, PASUP], F32, tag="pa_pu")
                    for c in range(PASUP // 512):
                        for vt in range(NVT):
                            nc.tensor.matmul(
                                pu[:, c * 512:(c + 1) * 512],
                                lhsT=s0b[:, vt, :],
                                rhs=bt[:, vt, c * 512:(c + 1) * 512],
                                start=(vt == 0), stop=(vt == NVT - 1))
                    nc.vector.tensor_copy(
                        out=u0acc[:, sup * PASUP:(sup + 1) * PASUP],
                        in_=pu[:])
                bi0 = dp.tile([H + 2, EE], F32, tag="bi0")
                nc.sync.dma_start(out=bi0[:], in_=u0acc[:])
                nc.gpsimd.collective_compute(
                    "AllReduce", OP.add, replica_groups=GROUPS,
                    ins=[bi0.opt()], outs=[bo0.opt()])

            # ====== mid0: full-width edge stage on every core ===========
            z0t = pp.tile([H + 1, EE], BF16, tag="z0t")
            with tc.tile_pool(name="m0", bufs=1) as m0, \
                 tc.tile_pool(name="m0c", bufs=2) as m0c, \
                 tc.tile_pool(name="m0ps", bufs=2, space="PSUM") as m0ps:
                # 1/beta_denom (guard 0 -> 1), via folded layout
                bd128 = m0.tile([128, FW], F32, tag="bd128")
                nc.gpsimd.dma_start(
                    out=bd128[:],
                    in_=bo0[H:H + 1, :].rearrange("a (p c) -> (a p) c",
                                                  p=128))
                msk = m0.tile([128, FW], F32, tag="msk")
                nc.vector.tensor_scalar(out=msk[:], in0=bd128[:], scalar1=0.0,
                                        scalar2=None, op0=OP.is_equal)
                nc.vector.tensor_add(out=bd128[:], in0=bd128[:], in1=msk[:])
                rb128 = m0.tile([128, FW], F32, tag="rb128")
                nc.vector.reciprocal(out=rb128[:], in_=bd128[:])
                nc.gpsimd.dma_start(
                    out=rbrow[:].rearrange("a (p c) -> (a p) c", p=128),
                    in_=rb128[:])
                # e_alpha = ecard'^-1.5 (guard 0 -> 1)
                ec128 = m0.tile([128, FW], F32, tag="ec128")
                nc.gpsimd.dma_start(
                    out=ec128[:],
                    in_=bo0[H + 1:H + 2, :].rearrange("a (p c) -> (a p) c",
                                                      p=128))
                nc.vector.tensor_scalar_max(out=ec128[:], in0=ec128[:],
                                            scalar1=1.0)
                sq = m0.tile([128, FW], F32, tag="sq")
                nc.scalar.sqrt(out=sq[:], in_=ec128[:])
                nc.vector.tensor_mul(out=sq[:], in0=sq[:], in1=ec128[:])
                ea128 = m0.tile([128, FW], F32, tag="ea128")
                nc.vector.reciprocal(out=ea128[:], in_=sq[:])
                nc.gpsimd.dma_start(
                    out=earow[:].rearrange("a (p c) -> (a p) c", p=128),
                    in_=ea128[:])
                eab128 = m0.tile([128, FW], BF16, tag="eab128")
                nc.vector.tensor_copy(out=eab128[:], in_=ea128[:])
                nc.gpsimd.dma_start(
                    out=earow_b[:].rearrange("a (p c) -> (a p) c", p=128),
                    in_=eab128[:])
                nc.sync.dma_start(out=z0t[H:H + 1, :], in_=earow_b[:])
                # x1 / Z0 chunks
                for ch in range(NMCH):
                    sl = slice(ch * MCH, (ch + 1) * MCH)
                    u0c = m0c.tile([H, MCH], F32, tag="u0c")
                    nc.sync.dma_start(out=u0c[:], in_=bo0[0:H, sl])
                    rbc = m0c.tile([H, MCH], F32, tag="rbc")
                    nc.gpsimd.dma_start(
                        out=rbc[:], in_=rbrow[:, sl].to_broadcast([H, MCH]))
                    eac = m0c.tile([H, MCH], F32, tag="eac")
                    nc.gpsimd.dma_start(
                        out=eac[:], in_=earow[:, sl].to_broadcast([H, MCH]))
                    xs = m0c.tile([H, MCH], F32, tag="xs")
                    nc.vector.tensor_mul(out=xs[:], in0=u0c[:], in1=rbc[:])
                    nc.scalar.activation(out=xs[:], in_=xs[:], func=AF.Relu,
                                         bias=bb01_0[:])
                    for c in range(MCH // 512):
                        zp = m0ps.tile([H, 512], F32, tag="zp")
                        nc.tensor.matmul(zp[:], lhsT=w10_0,
                                         rhs=xs[:, c * 512:(c + 1) * 512],
                                         start=True, stop=True)
                        nc.vector.tensor_mul(
                            out=z0t[0:H, ch * MCH + c * 512:
                                    ch * MCH + (c + 1) * 512],
                            in0=zp[:], in1=eac[:, c * 512:(c + 1) * 512])

            # ================= PB: V0'^T = Z0'^T B^T ====================
            with tc.tile_pool(name="pbz", bufs=1) as pbz, \
                 tc.tile_pool(name="pb", bufs=2) as pb, \
                 tc.tile_pool(name="pbps", bufs=1, space="PSUM") as pbps, \
                 tc.tile_pool(name="pbps2", bufs=2, space="PSUM") as pbps2:
                zst = pbz.tile([128, NET, H + 1], BF16, tag="zst")
                for et in range(NET):
                    ptz = pbps2.tile([128, H + 1], BF16, tag="ptz")
                    nc.tensor.transpose(
                        ptz[:], z0t[:, et * 128:(et + 1) * 128],
                        id_bf16[:H + 1, :H + 1])
                    nc.vector.tensor_copy(out=zst[:, et, :], in_=ptz[:])
                vp = pbps.tile([H + 1, nloc], F32, tag="vp")
                for sup in range(NTSUP):
                    btile = pb.tile([128, ETL, nloc], BF16, tag="pb_bt")
                    nc.sync.dma_start_transpose(
                        btile[:], B16[:, sup * TSUP:(sup + 1) * TSUP])
                    for etl in range(ETL):
                        et = sup * ETL + etl
                        for c in range(NCH):
                            nc.tensor.matmul(
                                vp[:, c * CW:(c + 1) * CW],
                                lhsT=zst[:, et, :],
                                rhs=btile[:, etl, c * CW:(c + 1) * CW],
                                start=(et == 0), stop=(et == NET - 1))
                # alpha_denom -> 1/ad broadcast ; x = relu(V0/ad + b10_0)
                with tc.tile_pool(name="pbs", bufs=1) as pbs:
                    adm = pbs.tile([1, nloc], F32, tag="adm")
                    nc.vector.tensor_scalar(out=adm[:], in0=vp[H:H + 1, :],
                                            scalar1=0.0, scalar2=None,
                                            op0=OP.is_equal)
                    nc.vector.tensor_add(out=adm[:], in0=adm[:],
                                         in1=vp[H:H + 1, :])
                    ra = pbs.tile([1, nloc], F32, tag="ra")
                    nc.vector.reciprocal(out=ra[:], in_=adm[:])
                    rarow = dp.tile([1, nloc], F32, tag="rarow")
                    nc.gpsimd.dma_start(out=rarow[:], in_=ra[:])
                    nc.gpsimd.dma_start(out=raB[:],
                                        in_=rarow[:].to_broadcast([H, nloc]))
                    xl1 = pbs.tile([H, nloc], F32, tag="xl1")
                    nc.vector.tensor_mul(out=xl1[:], in0=vp[0:H, :],
                                         in1=raB[:])
                    nc.scalar.activation(out=xl1[:], in_=xl1[:], func=AF.Relu,
                                         bias=bb10_0[:])
                    # S1^T = (W01_1^T x^T) * v_beta
                    s1tb = pbs.tile([H, nloc], BF16, tag="s1tb")
                    for c in range(NCH):
                        yp = pbps2.tile([H, CW], F32, tag="yp")
                        nc.tensor.matmul(yp[:], lhsT=w01_1,
                                         rhs=xl1[:, c * CW:(c + 1) * CW],
                                         start=True, stop=True)
                        nc.vector.tensor_mul(
                            out=s1tb[:, c * CW:(c + 1) * CW], in0=yp[:],
                            in1=vbB[:, c * CW:(c + 1) * CW])
                    for vt in range(NVT):
                        pts = pbps2.tile([128, H], BF16, tag="pts")
                        nc.tensor.transpose(
                            pts[:], s1tb[:, vt * 128:(vt + 1) * 128],
                            id_bf16[:H, :H])
                        nc.vector.tensor_copy(out=s1b[:, vt, :], in_=pts[:])

            # ================= PC: U1' = B^T S1 -> AllReduce ============
            bo1 = dp.tile([H, EE], F32, tag="bo1")
            with tc.tile_pool(name="pc", bufs=2) as pc, \
                 tc.tile_pool(name="pc_acc", bufs=1) as pca, \
                 tc.tile_pool(name="pcps", bufs=2, space="PSUM") as pcps:
                u1acc = pca.tile([H, EE], F32, tag="u1acc")
                for sup in range(NSUP):
                    bt = pc.tile([128, NVT, PASUP], BF16, tag="pc_bt")
                    nc.sync.dma_start(
                        out=bt[:],
                        in_=B16[:, sup * PASUP:(sup + 1) * PASUP].rearrange(
                            "(vt p) e -> p vt e", p=128))
                    pu = pcps.tile([H, PASUP], F32, tag="pc_pu")
                    for c in range(PASUP // 512):
                        for vt in range(NVT):
                            nc.tensor.matmul(
                                pu[:, c * 512:(c + 1) * 512],
                                lhsT=s1b[:, vt, :],
                                rhs=bt[:, vt, c * 512:(c + 1) * 512],
                                start=(vt == 0), stop=(vt == NVT - 1))
                    nc.vector.tensor_copy(
                        out=u1acc[:, sup * PASUP:(sup + 1) * PASUP],
                        in_=pu[:])
                bi1 = dp.tile([H, EE], F32, tag="bi1")
                nc.sync.dma_start(out=bi1[:], in_=u1acc[:])
                nc.gpsimd.collective_compute(
                    "AllReduce", OP.add, replica_groups=GROUPS,
                    ins=[bi1.opt()], outs=[bo1.opt()])

            # ====== mid1: full-width edge stage on every core ===========
            z1t = pp.tile([H, EE], BF16, tag="z1t")
            with tc.tile_pool(name="m1c", bufs=2) as m1c, \
                 tc.tile_pool(name="m1ps", bufs=2, space="PSUM") as m1ps:
                for ch in range(NMCH):
                    sl = slice(ch * MCH, (ch + 1) * MCH)
                    u1c = m1c.tile([H, MCH], F32, tag="u1c")
                    nc.sync.dma_start(out=u1c[:], in_=bo1[0:H, sl])
                    rbc = m1c.tile([H, MCH], F32, tag="rbc1")
                    nc.gpsimd.dma_start(
                        out=rbc[:], in_=rbrow[:, sl].to_broadcast([H, MCH]))
                    eac = m1c.tile([H, MCH], F32, tag="eac1")
                    nc.gpsimd.dma_start(
                        out=eac[:], in_=earow[:, sl].to_broadcast([H, MCH]))
                    xs2 = m1c.tile([H, MCH], F32, tag="xs2")
                    nc.vector.tensor_mul(out=xs2[:], in0=u1c[:], in1=rbc[:])
                    nc.scalar.activation(out=xs2[:], in_=xs2[:], func=AF.Relu,
                                         bias=bb01_1[:])
                    for c in range(MCH // 512):
                        zp1 = m1ps.tile([H, 512], F32, tag="zp1")
                        nc.tensor.matmul(zp1[:], lhsT=w10_1,
                                         rhs=xs2[:, c * 512:(c + 1) * 512],
                                         start=True, stop=True)
                        nc.vector.tensor_mul(
                            out=z1t[:, ch * MCH + c * 512:
                                    ch * MCH + (c + 1) * 512],
                            in0=zp1[:], in1=eac[:, c * 512:(c + 1) * 512])

            # ================= PD: V1^T + finale ========================
            with tc.tile_pool(name="pdz", bufs=1) as pdz, \
                 tc.tile_pool(name="pd", bufs=2) as pd, \
                 tc.tile_pool(name="pdps", bufs=1, space="PSUM") as pdps, \
                 tc.tile_pool(name="pdps2", bufs=2, space="PSUM") as pdps2:
                z1st = pdz.tile([128, NET, H], BF16, tag="z1st")
                for et in range(NET):
                    ptz = pdps2.tile([128, H], BF16, tag="ptz1")
                    nc.tensor.transpose(
                        ptz[:], z1t[:, et * 128:(et + 1) * 128],
                        id_bf16[:H, :H])
                    nc.vector.tensor_copy(out=z1st[:, et, :], in_=ptz[:])
                vp1 = pdps.tile([H, nloc], F32, tag="vp1")
                for sup in range(NTSUP):
                    btile = pd.tile([128, ETL, nloc], BF16, tag="pd_bt")
                    nc.sync.dma_start_transpose(
                        btile[:], B16[:, sup * TSUP:(sup + 1) * TSUP])
                    for etl in range(ETL):
                        et = sup * ETL + etl
                        for c in range(NCH):
                            nc.tensor.matmul(
                                vp1[:, c * CW:(c + 1) * CW],
                                lhsT=z1st[:, et, :],
                                rhs=btile[:, etl, c * CW:(c + 1) * CW],
                                start=(et == 0), stop=(et == NET - 1))
                with tc.tile_pool(name="fin", bufs=1) as fin:
                    x2 = fin.tile([H, nloc], F32, tag="x2")
                    nc.vector.tensor_mul(out=x2[:], in0=vp1[:], in1=raB[:])
                    nc.scalar.activation(out=x2[:], in_=x2[:], func=AF.Relu,
                                         bias=bb10_1[:])
                    pool_p = fin.tile([H, 1], F32, tag="pool_p")
                    nc.vector.tensor_reduce(out=pool_p[:], in_=x2[:],
                                            axis=mybir.AxisListType.X,
                                            op=OP.max)
                    bp = dp.tile([H, 1], F32, tag="bp")
                    nc.gpsimd.dma_start(out=bp[:], in_=pool_p[:])
                    bpo = dp.tile([H, 1], F32, tag="bpo")
                    nc.gpsimd.collective_compute(
                        "AllReduce", OP.max, replica_groups=GROUPS,
                        ins=[bp.opt()], outs=[bpo.opt()])
                    pooled = fin.tile([H, 1], F32, tag="pooled")
                    nc.gpsimd.dma_start(out=pooled[:], in_=bpo[:])
                    po = pdps2.tile([1, 1], F32, tag="po")
                    nc.tensor.matmul(po[:], lhsT=pooled[:], rhs=wout[:],
                                     start=True, stop=True)
                    ob = fin.tile([1, 1], F32, tag="ob")
                    nc.vector.tensor_add(out=ob[:], in0=po[:], in1=bbout[:])
                    nc.sync.dma_start(out=out[:], in_=ob[:])

    nc.compile()
    return nc


_NC_CACHE = {}


def _get_nc():
    if "nc" not in _NC_CACHE:
        _NC_CACHE["nc"] = build_kernel()
    return _NC_CACHE["nc"]


def _make_in_maps(inputs, ncores=NCORES, nloc=N // NCORES):
    x0 = np.asarray(inputs["x0"], np.float32)
    inc = np.asarray(inputs["incidence"])
    bits = np.packbits(inc != 0, axis=1, bitorder="little")  # [N, E//8] u8
    wpack = np.zeros((128, WPACK_COLS), np.float32)
    wpack[:, WCOL_W01_0:WCOL_W01_0 + H] = np.asarray(inputs["W01_0"],
                                                     np.float32)
    wpack[0:H, WCOL_W10_0:WCOL_W10_0 + H] = np.asarray(inputs["W10_0"],
                                                       np.float32)
    wpack[0:H, WCOL_W01_1:WCOL_W01_1 + H] = np.asarray(inputs["W01_1"],
                                                       np.float32)
    wpack[0:H, WCOL_W10_1:WCOL_W10_1 + H] = np.asarray(inputs["W10_1"],
                                                       np.float32)
    wpack[0:H, WCOL_B01_0] = np.asarray(inputs["b01_0"],
                                        np.float32).reshape(-1)
    wpack[0:H, WCOL_B10_0] = np.asarray(inputs["b10_0"],
                                        np.float32).reshape(-1)
    wpack[0:H, WCOL_B01_1] = np.asarray(inputs["b01_1"],
                                        np.float32).reshape(-1)
    wpack[0:H, WCOL_B10_1] = np.asarray(inputs["b10_1"],
                                        np.float32).reshape(-1)
    wpack[0:H, WCOL_WOUT] = np.asarray(inputs["Wout"], np.float32).reshape(-1)
    wpack[0:1, WCOL_BOUT] = np.asarray(inputs["bout"], np.float32).reshape(-1)
    in_maps = []
    for c in range(ncores):
        m = {"x0": np.ascontiguousarray(x0[c * nloc:(c + 1) * nloc]),
             "bits": np.ascontiguousarray(bits[c * nloc:(c + 1) * nloc]),
             "wpack": wpack}
        in_maps.append(m)
    return in_maps


def kernel(**inputs) -> np.ndarray:
    nc = _get_nc()
    in_maps = _make_in_maps(inputs)
    res = run_bass_kernel_spmd(nc, in_maps, list(range(NCORES)))
    return res.results[0]["out"].reshape(1).astype(np.float32)


if __name__ == "__main__":
    pass


# revision 10
# speedup vs baseline: 12.4160x; 1.0445x over previous
"""HNHN hypergraph model on 8 Trainium2 NeuronCores (Bass/Tile), v5.

Wall-time-optimized: warm-run wall is dominated by host->device upload
through the axon tunnel (~105-150 MB/s) plus a ~190ms dispatch floor, so
the binary incidence matrix is bit-packed on host (512MB fp32 -> 16MB u8,
exact) and unpacked to bf16 on device; weights/biases ride in one small
array.  Device work is single-bf16 (tolerance 2e-2; measured pipeline
error ~1.3e-3) and each layer needs just ONE f32 AllReduce of the edge
features: every core then computes the (tiny) full-width edge-stage math
redundantly, which removes the ReduceScatter/AllGather pair and the
e-shard scatter/gather layout gymnastics.

Sharding: rows (nodes) of the incidence matrix and x0 over 8 cores.
Pipeline per core (v = own 1024 nodes, e = all 16384 hyperedges):
  P0   : unpack packed bits -> bf16 B16 tile (strided bit-planes)
         -> row sums (node_deg) -> DRAM scratch B16
  S0   : Y0 = x0 @ W01_0, S0 = [Y0*v_beta | v_beta | 1]  bf16
  PA   : U0' = B^T S0 partials (PSUM accum over own v) -> AllReduce(add)
         U0'[64] = beta_denom partials, U0'[65] = edge_card partials
  mid0 : full width on every core: x1 = relu(U0/beta_denom + b01_0)
         Z0' = [x1@W10_0 * e_alpha ; e_alpha]  bf16  (no collective)
  PB   : V0'^T = Z0'^T B^T via transposed-DMA reads of B16 (accum, all e)
         row 64 = alpha_denom;  x = relu(V0/alpha_denom + b10_0)
  PC   : U1' = B^T S1 partials -> AllReduce(add)
  mid1 : x1_l2 = relu(U1/beta_denom + b01_1); Z1' = x1_l2@W10_1 * e_alpha
  PD   : V1^T accum;  x2 = relu(V1/alpha_denom + b10_1)
  fin  : per-core max-pool -> AllReduce(max) -> pooled @ Wout + bout
"""

import numpy as np

import concourse.bass as bass
import concourse.bacc as bacc
import concourse.mybir as mybir
import concourse.tile as tile
from concourse.bass_utils import run_bass_kernel_spmd
from concourse.masks import make_identity

F32 = mybir.dt.float32
BF16 = mybir.dt.bfloat16
U8 = mybir.dt.uint8
AF = mybir.ActivationFunctionType
OP = mybir.AluOpType

N, E, D, H = 8192, 16384, 128, 64
NCORES = 8
# packed weight array column map (see _make_in_maps); all blocks start at
# partition row 0 so on-device slices never cross partition offsets
WCOL_W01_0 = 0            # [128, 64]   rows 0:128
WCOL_W10_0 = 64           # [64, 64]    rows 0:64
WCOL_W01_1 = 128          # [64, 64]    rows 0:64
WCOL_W10_1 = 192          # [64, 64]    rows 0:64
WCOL_B01_0 = 256          # [64, 1]
WCOL_B10_0 = 257          # [64, 1]
WCOL_B01_1 = 258          # [64, 1]
WCOL_B10_1 = 259          # [64, 1]
WCOL_WOUT = 260           # [64, 1]
WCOL_BOUT = 261           # [1, 1]
WPACK_COLS = 262


def build_kernel(ncores=NCORES, n_edges=E, nloc=N // NCORES):
    EE = n_edges
    EB = EE // 8                 # packed bytes per row
    NVT = nloc // 128            # v-tiles per core
    NET = EE // 128              # 128-wide e-tiles
    PASUP = min(2048, EE)        # PA/PC streaming super width
    NSUP = EE // PASUP
    TSUP = min(1024, EE)         # PB/PD transposed-read super width
    NTSUP = EE // TSUP
    ETL = TSUP // 128            # e-tiles per transposed read
    CW = min(512, nloc)          # column chunk for nloc-wide ops
    NCH = nloc // CW
    MCH = 2048                   # mid-phase e-chunk
    NMCH = EE // MCH
    FW = EE // 128               # fold width for full-width scalar math
    GROUPS = [list(range(ncores))]

    nc = bacc.Bacc("TRN2", target_bir_lowering=False, debug=False,
                   num_devices=ncores)

    x0 = nc.declare_dram_parameter("x0", [nloc, D], BF16, isOutput=False)
    bits = nc.declare_dram_parameter("bits", [nloc, EB], U8, isOutput=False)
    wpk = nc.declare_dram_parameter("wpack", [128, WPACK_COLS], F32,
                                    isOutput=False)
    out = nc.declare_dram_parameter("out", [1, 1], F32, isOutput=True)

    B16 = nc.dram_tensor("b16", [nloc, EE], BF16)

    with tile.TileContext(nc, num_cores=ncores) as tc:
        with tc.tile_pool(name="persist", bufs=1) as pp, \
             tc.tile_pool(name="dram", bufs=1, space="DRAM") as dp:
            # ---- constants / weights ----
            id_f32 = pp.tile([128, 128], F32, tag="id_f32")
            make_identity(nc, id_f32[:])
            id_bf16 = pp.tile([128, 128], BF16, tag="id_bf16")
            make_identity(nc, id_bf16[:])
            wall = pp.tile([128, WPACK_COLS], F32, tag="wall")
            nc.sync.dma_start(out=wall[:], in_=wpk[:])
            w01_0 = wall[:, WCOL_W01_0:WCOL_W01_0 + H]            # [128,64]
            w10_0 = wall[0:H, WCOL_W10_0:WCOL_W10_0 + H]          # [64,64]
            w01_1 = wall[0:H, WCOL_W01_1:WCOL_W01_1 + H]
            w10_1 = wall[0:H, WCOL_W10_1:WCOL_W10_1 + H]
            bb01_0 = pp.tile([H, 1], F32, tag="bb01_0")
            nc.vector.tensor_copy(out=bb01_0[:],
                                  in_=wall[0:H, WCOL_B01_0:WCOL_B01_0 + 1])
            bb10_0 = pp.tile([H, 1], F32, tag="bb10_0")
            nc.vector.tensor_copy(out=bb10_0[:],
                                  in_=wall[0:H, WCOL_B10_0:WCOL_B10_0 + 1])
            bb01_1 = pp.tile([H, 1], F32, tag="bb01_1")
            nc.vector.tensor_copy(out=bb01_1[:],
                                  in_=wall[0:H, WCOL_B01_1:WCOL_B01_1 + 1])
            bb10_1 = pp.tile([H, 1], F32, tag="bb10_1")
            nc.vector.tensor_copy(out=bb10_1[:],
                                  in_=wall[0:H, WCOL_B10_1:WCOL_B10_1 + 1])
            wout = pp.tile([H, 1], F32, tag="wout")
            nc.vector.tensor_copy(out=wout[:],
                                  in_=wall[0:H, WCOL_WOUT:WCOL_WOUT + 1])
            bbout = pp.tile([1, 1], F32, tag="bbout")
            nc.vector.tensor_copy(out=bbout[:],
                                  in_=wall[0:1, WCOL_BOUT:WCOL_BOUT + 1])

            # ---- persistent small state ----
            deg_all = pp.tile([128, NVT], F32, tag="deg_all")
            vb_all = pp.tile([128, NVT], F32, tag="vb_all")
            s0b = pp.tile([128, NVT, H + 2], BF16, tag="s0b")
            s1b = pp.tile([128, NVT, H], BF16, tag="s1b")
            raB = pp.tile([H, nloc], F32, tag="raB")     # 1/alpha_denom bcast
            vbB = pp.tile([H, nloc], F32, tag="vbB")     # v_beta bcast (free)
            rbrow = dp.tile([1, EE], F32, tag="rbrow")   # 1/beta_denom (DRAM)
            earow = dp.tile([1, EE], F32, tag="earow")   # e_alpha (DRAM)
            earow_b = dp.tile([1, EE], BF16, tag="earow_b")

            # ====== P0: unpack bits -> bf16 B16 + row sums (node_deg) ======
            with tc.tile_pool(name="p0", bufs=2) as p0:
                for vt in range(NVT):
                    bt = p0.tile([128, EB], U8, tag="p0bits")
                    nc.sync.dma_start(
                        out=bt[:], in_=bits[vt * 128:(vt + 1) * 128, :])
                    ub = p0.tile([128, EE], BF16, tag="p0ub")
                    ubv = ub[:].rearrange("p (j t) -> p t j", t=8)
                    for t in range(8):
                        m = p0.tile([128, EB], U8, tag="p0m")
                        nc.vector.tensor_scalar(
                            out=m[:], in0=bt[:], scalar1=1 << t,
                            scalar2=None, op0=OP.bitwise_and)
                        nc.vector.tensor_scalar(
                            out=ubv[:, t, :], in0=m[:], scalar1=0,
                            scalar2=None, op0=OP.is_gt)
                    nc.vector.tensor_reduce(
                        out=deg_all[:, vt:vt + 1], in_=ub[:],
                        axis=mybir.AxisListType.X, op=OP.add)
                    nc.sync.dma_start(
                        out=B16[vt * 128:(vt + 1) * 128, :], in_=ub[:])

            # node_deg -> v_beta
            with tc.tile_pool(name="vbp", bufs=1) as vbp:
                degc = vbp.tile([128, NVT], F32, tag="degc")
                nc.vector.tensor_scalar_max(out=degc[:], in0=deg_all[:],
                                            scalar1=1.0)
                sqd = vbp.tile([128, NVT], F32, tag="sqd")
                nc.scalar.sqrt(out=sqd[:], in_=degc[:])
                nc.vector.reciprocal(out=vb_all[:], in_=sqd[:])
                # v_beta to free-layout DRAM row then broadcast into vbB
                with tc.tile_pool(name="vbps", bufs=1, space="PSUM") as vps:
                    pt = vps.tile([NVT, 128], F32, tag="vb_t")
                    nc.tensor.transpose(pt[:], vb_all[:], id_f32[:])
                    vb8 = vbp.tile([NVT, 128], F32, tag="vb8")
                    nc.vector.tensor_copy(out=vb8[:], in_=pt[:])
                vrow = dp.tile([1, nloc], F32, tag="vrow")
                nc.gpsimd.dma_start(
                    out=vrow[:].rearrange("a (b c) -> (a b) c", b=NVT),
                    in_=vb8[:])
                nc.gpsimd.dma_start(out=vbB[:],
                                    in_=vrow[:].to_broadcast([H, nloc]))

            # ================= S0 prep (single bf16) ====================
            with tc.tile_pool(name="s0p", bufs=2) as sp, \
                 tc.tile_pool(name="s0ps", bufs=2, space="PSUM") as sps:
                for vt in range(NVT):
                    xt = sp.tile([128, D], BF16, tag="xt")
                    nc.sync.dma_start(out=xt[:],
                                      in_=x0[vt * 128:(vt + 1) * 128, :])
                    pxt = sps.tile([D, 128], BF16, tag="pxt")
                    nc.tensor.transpose(pxt[:], xt[:], id_bf16[:])
                    x0T = sp.tile([D, 128], F32, tag="x0T")
                    nc.vector.tensor_copy(out=x0T[:], in_=pxt[:])
                    py = sps.tile([128, H], F32, tag="py")
                    nc.tensor.matmul(py[:], lhsT=x0T[:], rhs=w01_0,
                                     start=True, stop=True)
                    s0f = sp.tile([128, H + 2], F32, tag="s0f")
                    nc.vector.tensor_scalar_mul(out=s0f[:, 0:H], in0=py[:],
                                                scalar1=vb_all[:, vt:vt + 1])
                    nc.vector.tensor_copy(out=s0f[:, H:H + 1],
                                          in_=vb_all[:, vt:vt + 1])
                    nc.vector.memset(s0f[:, H + 1:H + 2], 1.0)
                    nc.vector.tensor_copy(out=s0b[:, vt, :], in_=s0f[:])

            # ================= PA: U0' = B^T S0 -> AllReduce ============
            bo0 = dp.tile([H + 2, EE], F32, tag="bo0")
            with tc.tile_pool(name="pa", bufs=2) as pa, \
                 tc.tile_pool(name="pa_acc", bufs=1) as paa, \
                 tc.tile_pool(name="paps", bufs=2, space="PSUM") as paps:
                u0acc = paa.tile([H + 2, EE], F32, tag="u0acc")
                for sup in range(NSUP):
                    bt = pa.tile([128, NVT, PASUP], BF16, tag="pa_bt")
                    nc.sync.dma_start(
                        out=bt[:],
                        in_=B16[:, sup * PASUP:(sup + 1) * PASUP].rearrange(
                            "(vt p) e -> p vt e", p=128))
                    pu = paps.tile([H + 2, PASUP], F32, tag="pa_pu")
                    for c in range(PASUP // 512):
                        for vt in range(NVT):
                            nc.tensor.matmul(
                                pu[:, c * 512:(c + 1) * 512],
                                lhsT=s0b[:, vt, :],
                                rhs=bt[:, vt, c * 512:(c + 1) * 512],
                                start=(vt == 0), stop=(vt == NVT - 1))
                    nc.vector.tensor_copy(
                        out=u0acc[:, sup * PASUP:(sup + 1) * PASUP],
                        in_=pu[:])
                bi0 = dp.tile([H + 2, EE], F32, tag="bi0")
                nc.sync.dma_start(out=bi0[:], in_=u0acc[:])
                nc.gpsimd.collective_compute(
                    "AllReduce", OP.add, replica_groups=GROUPS,
                    ins=[bi0.opt()], outs=[bo0.opt()])

            # ====== mid0: full-width edge stage on every core ===========
            z0t = pp.tile([H + 1, EE], BF16, tag="z0t")
            with tc.tile_pool(name="m0", bufs=1) as m0, \
                 tc.tile_pool(name="m0c", bufs=2) as m0c, \
                 tc.tile_pool(name="m0ps", bufs=2, space="PSUM") as m0ps:
                # 1/beta_denom (guard 0 -> 1), via folded layout
                bd128 = m0.tile([128, FW], F32, tag="bd128")
                nc.gpsimd.dma_start(
                    out=bd128[:],
                    in_=bo0[H:H + 1, :].rearrange("a (p c) -> (a p) c",
                                                  p=128))
                msk = m0.tile([128, FW], F32, tag="msk")
                nc.vector.tensor_scalar(out=msk[:], in0=bd128[:], scalar1=0.0,
                                        scalar2=None, op0=OP.is_equal)
                nc.vector.tensor_add(out=bd128[:], in0=bd128[:], in1=msk[:])
                rb128 = m0.tile([128, FW], F32, tag="rb128")
                nc.vector.reciprocal(out=rb128[:], in_=bd128[:])
                nc.gpsimd.dma_start(
                    out=rbrow[:].rearrange("a (p c) -> (a p) c", p=128),
                    in_=rb128[:])
                # e_alpha = ecard'^-1.5 (guard 0 -> 1)
                ec128 = m0.tile([128, FW], F32, tag="ec128")
                nc.gpsimd.dma_start(
                    out=ec128[:],
                    in_=bo0[H + 1:H + 2, :].rearrange("a (p c) -> (a p) c",
                                                      p=128))
                nc.vector.tensor_scalar_max(out=ec128[:], in0=ec128[:],
                                            scalar1=1.0)
                sq = m0.tile([128, FW], F32, tag="sq")
                nc.scalar.sqrt(out=sq[:], in_=ec128[:])
                nc.vector.tensor_mul(out=sq[:], in0=sq[:], in1=ec128[:])
                ea128 = m0.tile([128, FW], F32, tag="ea128")
                nc.vector.reciprocal(out=ea128[:], in_=sq[:])
                nc.gpsimd.dma_start(
                    out=earow[:].rearrange("a (p c) -> (a p) c", p=128),
                    in_=ea128[:])
                eab128 = m0.tile([128, FW], BF16, tag="eab128")
                nc.vector.tensor_copy(out=eab128[:], in_=ea128[:])
                nc.gpsimd.dma_start(
                    out=earow_b[:].rearrange("a (p c) -> (a p) c", p=128),
                    in_=eab128[:])
                nc.sync.dma_start(out=z0t[H:H + 1, :], in_=earow_b[:])
                # x1 / Z0 chunks
                for ch in range(NMCH):
                    sl = slice(ch * MCH, (ch + 1) * MCH)
                    u0c = m0c.tile([H, MCH], F32, tag="u0c")
                    nc.sync.dma_start(out=u0c[:], in_=bo0[0:H, sl])
                    rbc = m0c.tile([H, MCH], F32, tag="rbc")
                    nc.gpsimd.dma_start(
                        out=rbc[:], in_=rbrow[:, sl].to_broadcast([H, MCH]))
                    eac = m0c.tile([H, MCH], F32, tag="eac")
                    nc.gpsimd.dma_start(
                        out=eac[:], in_=earow[:, sl].to_broadcast([H, MCH]))
                    xs = m0c.tile([H, MCH], F32, tag="xs")
                    nc.vector.tensor_mul(out=xs[:], in0=u0c[:], in1=rbc[:])
                    nc.scalar.activation(out=xs[:], in_=xs[:], func=AF.Relu,
                                         bias=bb01_0[:])
                    for c in range(MCH // 512):
                        zp = m0ps.tile([H, 512], F32, tag="zp")
                        nc.tensor.matmul(zp[:], lhsT=w10_0,
                                         rhs=xs[:, c * 512:(c + 1) * 512],
                                         start=True, stop=True)
                        nc.vector.tensor_mul(
                            out=z0t[0:H, ch * MCH + c * 512:
                                    ch * MCH + (c + 1) * 512],
                            in0=zp[:], in1=eac[:, c * 512:(c + 1) * 512])

            # ================= PB: V0'^T = Z0'^T B^T ====================
            with tc.tile_pool(name="pbz", bufs=1) as pbz, \
                 tc.tile_pool(name="pb", bufs=2) as pb, \
                 tc.tile_pool(name="pbps", bufs=1, space="PSUM") as pbps, \
                 tc.tile_pool(name="pbps2", bufs=2, space="PSUM") as pbps2:
                zst = pbz.tile([128, NET, H + 1], BF16, tag="zst")
                for et in range(NET):
                    ptz = pbps2.tile([128, H + 1], BF16, tag="ptz")
                    nc.tensor.transpose(
                        ptz[:], z0t[:, et * 128:(et + 1) * 128],
                        id_bf16[:H + 1, :H + 1])
                    nc.vector.tensor_copy(out=zst[:, et, :], in_=ptz[:])
                vp = pbps.tile([H + 1, nloc], F32, tag="vp")
                for sup in range(NTSUP):
                    btile = pb.tile([128, ETL, nloc], BF16, tag="pb_bt")
                    nc.sync.dma_start_transpose(
                        btile[:], B16[:, sup * TSUP:(sup + 1) * TSUP])
                    for etl in range(ETL):
                        et = sup * ETL + etl
                        for c in range(NCH):
                            nc.tensor.matmul(
                                vp[:, c * CW:(c + 1) * CW],
                                lhsT=zst[:, et, :],
                                rhs=btile[:, etl, c * CW:(c + 1) * CW],
                                start=(et == 0), stop=(et == NET - 1))
                # alpha_denom -> 1/ad broadcast ; x = relu(V0/ad + b10_0)
                with tc.tile_pool(name="pbs", bufs=1) as pbs:
                    adm = pbs.tile([1, nloc], F32, tag="adm")
                    nc.vector.tensor_scalar(out=adm[:], in0=vp[H:H + 1, :],
                                            scalar1=0.0, scalar2=None,
                                            op0=OP.is_equal)
                    nc.vector.tensor_add(out=adm[:], in0=adm[:],
                                         in1=vp[H:H + 1, :])
                    ra = pbs.tile([1, nloc], F32, tag="ra")
                    nc.vector.reciprocal(out=ra[:], in_=adm[:])
                    rarow = dp.tile([1, nloc], F32, tag="rarow")
                    nc.gpsimd.dma_start(out=rarow[:], in_=ra[:])
                    nc.gpsimd.dma_start(out=raB[:],
                                        in_=rarow[:].to_broadcast([H, nloc]))
                    xl1 = pbs.tile([H, nloc], F32, tag="xl1")
                    nc.vector.tensor_mul(out=xl1[:], in0=vp[0:H, :],
                                         in1=raB[:])
                    nc.scalar.activation(out=xl1[:], in_=xl1[:], func=AF.Relu,
                                         bias=bb10_0[:])
                    # S1^T = (W01_1^T x^T) * v_beta
                    s1tb = pbs.tile([H, nloc], BF16, tag="s1tb")
                    for c in range(NCH):
                        yp = pbps2.tile([H, CW], F32, tag="yp")
                        nc.tensor.matmul(yp[:], lhsT=w01_1,
                                         rhs=xl1[:, c * CW:(c + 1) * CW],
                                         start=True, stop=True)
                        nc.vector.tensor_mul(
                            out=s1tb[:, c * CW:(c + 1) * CW], in0=yp[:],
                            in1=vbB[:, c * CW:(c + 1) * CW])
                    for vt in range(NVT):
                        pts = pbps2.tile([128, H], BF16, tag="pts")
                        nc.tensor.transpose(
                            pts[:], s1tb[:, vt * 128:(vt + 1) * 128],
                            id_bf16[:H, :H])
                        nc.vector.tensor_copy(out=s1b[:, vt, :], in_=pts[:])

            # ================= PC: U1' = B^T S1 -> AllReduce ============
            bo1 = dp.tile([H, EE], F32, tag="bo1")
            with tc.tile_pool(name="pc", bufs=2) as pc, \
                 tc.tile_pool(name="pc_acc", bufs=1) as pca, \
                 tc.tile_pool(name="pcps", bufs=2, space="PSUM") as pcps:
                u1acc = pca.tile([H, EE], F32, tag="u1acc")
                for sup in range(NSUP):
                    bt = pc.tile([128, NVT, PASUP], BF16, tag="pc_bt")
                    nc.sync.dma_start(
                        out=bt[:],
                        in_=B16[:, sup * PASUP:(sup + 1) * PASUP].rearrange(
                            "(vt p) e -> p vt e", p=128))
                    pu = pcps.tile([H, PASUP], F32, tag="pc_pu")
                    for c in range(PASUP // 512):
                        for vt in range(NVT):
                            nc.tensor.matmul(
                                pu[:, c * 512:(c + 1) * 512],
                                lhsT=s1b[:, vt, :],
                                rhs=bt[:, vt, c * 512:(c + 1) * 512],
                                start=(vt == 0), stop=(vt == NVT - 1))
                    nc.vector.tensor_copy(
                        out=u1acc[:, sup * PASUP:(sup + 1) * PASUP],
                        in_=pu[:])
                bi1 = dp.tile([H, EE], F32, tag="bi1")
                nc.sync.dma_start(out=bi1[:], in_=u1acc[:])
                nc.gpsimd.collective_compute(
                    "AllReduce", OP.add, replica_groups=GROUPS,
                    ins=[bi1.opt()], outs=[bo1.opt()])

            # ====== mid1: full-width edge stage on every core ===========
            z1t = pp.tile([H, EE], BF16, tag="z1t")
            with tc.tile_pool(name="m1c", bufs=2) as m1c, \
                 tc.tile_pool(name="m1ps", bufs=2, space="PSUM") as m1ps:
                for ch in range(NMCH):
                    sl = slice(ch * MCH, (ch + 1) * MCH)
                    u1c = m1c.tile([H, MCH], F32, tag="u1c")
                    nc.sync.dma_start(out=u1c[:], in_=bo1[0:H, sl])
                    rbc = m1c.tile([H, MCH], F32, tag="rbc1")
                    nc.gpsimd.dma_start(
                        out=rbc[:], in_=rbrow[:, sl].to_broadcast([H, MCH]))
                    eac = m1c.tile([H, MCH], F32, tag="eac1")
                    nc.gpsimd.dma_start(
                        out=eac[:], in_=earow[:, sl].to_broadcast([H, MCH]))
                    xs2 = m1c.tile([H, MCH], F32, tag="xs2")
                    nc.vector.tensor_mul(out=xs2[:], in0=u1c[:], in1=rbc[:])
                    nc.scalar.activation(out=xs2[:], in_=xs2[:], func=AF.Relu,
                                         bias=bb01_1[:])
                    for c in range(MCH // 512):
                        zp1 = m1ps.tile([H, 512], F32, tag="zp1")
                        nc.tensor.matmul(zp1[:], lhsT=w10_1,
                                         rhs=xs2[:, c * 512:(c + 1) * 512],
                                         start=True, stop=True)
                        nc.vector.tensor_mul(
                            out=z1t[:, ch * MCH + c * 512:
                                    ch * MCH + (c + 1) * 512],
                            in0=zp1[:], in1=eac[:, c * 512:(c + 1) * 512])

            # ================= PD: V1^T + finale ========================
            with tc.tile_pool(name="pdz", bufs=1) as pdz, \
                 tc.tile_pool(name="pd", bufs=2) as pd, \
                 tc.tile_pool(name="pdps", bufs=1, space="PSUM") as pdps, \
                 tc.tile_pool(name="pdps2", bufs=2, space="PSUM") as pdps2:
                z1st = pdz.tile([128, NET, H], BF16, tag="z1st")
                for et in range(NET):
                    ptz = pdps2.tile([128, H], BF16, tag="ptz1")
                    nc.tensor.transpose(
                        ptz[:], z1t[:, et * 128:(et + 1) * 128],
                        id_bf16[:H, :H])
                    nc.vector.tensor_copy(out=z1st[:, et, :], in_=ptz[:])
                vp1 = pdps.tile([H, nloc], F32, tag="vp1")
                for sup in range(NTSUP):
                    btile = pd.tile([128, ETL, nloc], BF16, tag="pd_bt")
                    nc.sync.dma_start_transpose(
                        btile[:], B16[:, sup * TSUP:(sup + 1) * TSUP])
                    for etl in range(ETL):
                        et = sup * ETL + etl
                        for c in range(NCH):
                            nc.tensor.matmul(
                                vp1[:, c * CW:(c + 1) * CW],
                                lhsT=z1st[:, et, :],
                                rhs=btile[:, etl, c * CW:(c + 1) * CW],
                                start=(et == 0), stop=(et == NET - 1))
                with tc.tile_pool(name="fin", bufs=1) as fin:
                    x2 = fin.tile([H, nloc], F32, tag="x2")
                    nc.vector.tensor_mul(out=x2[:], in0=vp1[:], in1=raB[:])
                    nc.scalar.activation(out=x2[:], in_=x2[:], func=AF.Relu,
                                         bias=bb10_1[:])
                    pool_p = fin.tile([H, 1], F32, tag="pool_p")
                    nc.vector.tensor_reduce(out=pool_p[:], in_=x2[:],
                                            axis=mybir.AxisListType.X,
                                            op=OP.max)
                    bp = dp.tile([H, 1], F32, tag="bp")
                    nc.gpsimd.dma_start(out=bp[:], in_=pool_p[:])
                    bpo = dp.tile([H, 1], F32, tag="bpo")
                    nc.gpsimd.collective_compute(
                        "AllReduce", OP.max, replica_groups=GROUPS,
                        ins=[bp.opt()], outs=[bpo.opt()])
                    pooled = fin.tile([H, 1], F32, tag="pooled")
                    nc.gpsimd.dma_start(out=pooled[:], in_=bpo[:])
                    po = pdps2.tile([1, 1], F32, tag="po")
                    nc.tensor.matmul(po[:], lhsT=pooled[:], rhs=wout[:],
                                     start=True, stop=True)
                    ob = fin.tile([1, 1], F32, tag="ob")
                    nc.vector.tensor_add(out=ob[:], in0=po[:], in1=bbout[:])
                    nc.sync.dma_start(out=out[:], in_=ob[:])

    nc.compile()
    return nc


_NC_CACHE = {}


def _get_nc():
    if "nc" not in _NC_CACHE:
        _NC_CACHE["nc"] = build_kernel()
    return _NC_CACHE["nc"]


def _make_in_maps(inputs, ncores=NCORES, nloc=N // NCORES):
    from ml_dtypes import bfloat16
    x0 = np.asarray(inputs["x0"], np.float32).astype(bfloat16)
    inc = np.asarray(inputs["incidence"])
    bits = np.packbits(inc != 0, axis=1, bitorder="little")  # [N, E//8] u8
    wpack = np.zeros((128, WPACK_COLS), np.float32)
    wpack[:, WCOL_W01_0:WCOL_W01_0 + H] = np.asarray(inputs["W01_0"],
                                                     np.float32)
    wpack[0:H, WCOL_W10_0:WCOL_W10_0 + H] = np.asarray(inputs["W10_0"],
                                                       np.float32)
    wpack[0:H, WCOL_W01_1:WCOL_W01_1 + H] = np.asarray(inputs["W01_1"],
                                                       np.float32)
    wpack[0:H, WCOL_W10_1:WCOL_W10_1 + H] = np.asarray(inputs["W10_1"],
                                                       np.float32)
    wpack[0:H, WCOL_B01_0] = np.asarray(inputs["b01_0"],
                                        np.float32).reshape(-1)
    wpack[0:H, WCOL_B10_0] = np.asarray(inputs["b10_0"],
                                        np.float32).reshape(-1)
    wpack[0:H, WCOL_B01_1] = np.asarray(inputs["b01_1"],
                                        np.float32).reshape(-1)
    wpack[0:H, WCOL_B10_1] = np.asarray(inputs["b10_1"],
                                        np.float32).reshape(-1)
    wpack[0:H, WCOL_WOUT] = np.asarray(inputs["Wout"], np.float32).reshape(-1)
    wpack[0:1, WCOL_BOUT] = np.asarray(inputs["bout"], np.float32).reshape(-1)
    in_maps = []
    for c in range(ncores):
        m = {"x0": np.ascontiguousarray(x0[c * nloc:(c + 1) * nloc]),
             "bits": np.ascontiguousarray(bits[c * nloc:(c + 1) * nloc]),
             "wpack": wpack}
        in_maps.append(m)
    return in_maps


def kernel(**inputs) -> np.ndarray:
    nc = _get_nc()
    in_maps = _make_in_maps(inputs)
    res = run_bass_kernel_spmd(nc, in_maps, list(range(NCORES)))
    return res.results[0]["out"].reshape(1).astype(np.float32)


if __name__ == "__main__":
    pass


# revision 11
# speedup vs baseline: 13.1011x; 1.0552x over previous
"""HNHN hypergraph model on 8 Trainium2 NeuronCores (Bass/Tile), v7.

Wall-time-optimized: warm-run wall is dominated by host->device upload
through the axon tunnel (~105-150 MB/s) plus a ~190ms dispatch floor, so
the binary incidence matrix is bit-packed on host (512MB fp32 -> 16MB u8,
exact) and unpacked to bf16 on device; weights/biases ride in one small
array.  Device work is single-bf16 (tolerance 2e-2; measured pipeline
error ~1.3e-3) and each layer needs just ONE f32 AllReduce of the edge
features: every core then computes the (tiny) full-width edge-stage math
redundantly, which removes the ReduceScatter/AllGather pair and the
e-shard scatter/gather layout gymnastics.

Sharding: rows (nodes) of the incidence matrix and x0 over 8 cores.
Pipeline per core (v = own 1024 nodes, e = all 16384 hyperedges):
  P0   : unpack packed bits -> bf16 B16 tile (strided bit-planes)
         -> row sums (node_deg) -> DRAM scratch B16
  S0   : Y0 = x0 @ W01_0, S0 = [Y0*v_beta | v_beta | 1]  bf16
  PA   : U0' = B^T S0 partials (PSUM accum over own v) -> AllReduce(add)
         U0'[64] = beta_denom partials, U0'[65] = edge_card partials
  mid0 : full width on every core: x1 = relu(U0/beta_denom + b01_0)
         Z0' = [x1@W10_0 * e_alpha ; e_alpha]  bf16  (no collective)
  PB   : V0'^T = Z0'^T B^T via transposed-DMA reads of B16 (accum, all e)
         row 64 = alpha_denom;  x = relu(V0/alpha_denom + b10_0)
  PC   : U1' = B^T S1 partials -> AllReduce(add)
  mid1 : x1_l2 = relu(U1/beta_denom + b01_1); Z1' = x1_l2@W10_1 * e_alpha
  PD   : V1^T accum;  x2 = relu(V1/alpha_denom + b10_1)
  fin  : per-core max-pool -> AllReduce(max) -> pooled @ Wout + bout
"""

import numpy as np

import concourse.bass as bass
import concourse.bacc as bacc
import concourse.mybir as mybir
import concourse.tile as tile
from concourse.bass_utils import run_bass_kernel_spmd
from concourse.masks import make_identity

F32 = mybir.dt.float32
BF16 = mybir.dt.bfloat16
U8 = mybir.dt.uint8
AF = mybir.ActivationFunctionType
OP = mybir.AluOpType

N, E, D, H = 8192, 16384, 128, 64
NCORES = 8
# packed weight array column map (see _make_in_maps); all blocks start at
# partition row 0 so on-device slices never cross partition offsets
WCOL_W01_0 = 0            # [128, 64]   rows 0:128
WCOL_W10_0 = 64           # [64, 64]    rows 0:64
WCOL_W01_1 = 128          # [64, 64]    rows 0:64
WCOL_W10_1 = 192          # [64, 64]    rows 0:64
WCOL_B01_0 = 256          # [64, 1]
WCOL_B10_0 = 257          # [64, 1]
WCOL_B01_1 = 258          # [64, 1]
WCOL_B10_1 = 259          # [64, 1]
WCOL_WOUT = 260           # [64, 1]
WCOL_BOUT = 261           # [1, 1]
WPACK_COLS = 262


def build_kernel(ncores=NCORES, n_edges=E, nloc=N // NCORES):
    EE = n_edges
    EB = EE // 8                 # packed bytes per row
    NVT = nloc // 128            # v-tiles per core
    NET = EE // 128              # 128-wide e-tiles
    PASUP = min(2048, EE)        # PA/PC streaming super width
    NSUP = EE // PASUP
    TSUP = min(1024, EE)         # PB/PD transposed-read super width
    NTSUP = EE // TSUP
    ETL = TSUP // 128            # e-tiles per transposed read
    CW = min(512, nloc)          # column chunk for nloc-wide ops
    NCH = nloc // CW
    MCH = 2048                   # mid-phase e-chunk
    NMCH = EE // MCH
    FW = EE // 128               # fold width for full-width scalar math
    GROUPS = [list(range(ncores))]

    nc = bacc.Bacc("TRN2", target_bir_lowering=False, debug=False,
                   num_devices=ncores)

    x0 = nc.declare_dram_parameter("x0", [nloc, D], BF16, isOutput=False)
    bits = nc.declare_dram_parameter("bits", [nloc, EB], U8, isOutput=False)
    wpk = nc.declare_dram_parameter("wpack", [128, WPACK_COLS], F32,
                                    isOutput=False)
    out = nc.declare_dram_parameter("out", [1, 1], F32, isOutput=True)

    B16 = nc.dram_tensor("b16", [nloc, EE], BF16)

    with tile.TileContext(nc, num_cores=ncores) as tc:
        with tc.tile_pool(name="persist", bufs=1) as pp, \
             tc.tile_pool(name="dram", bufs=1, space="DRAM") as dp:
            # ---- constants / weights ----
            id_f32 = pp.tile([128, 128], F32, tag="id_f32")
            make_identity(nc, id_f32[:])
            id_bf16 = pp.tile([128, 128], BF16, tag="id_bf16")
            make_identity(nc, id_bf16[:])
            wall = pp.tile([128, WPACK_COLS], F32, tag="wall")
            nc.sync.dma_start(out=wall[:], in_=wpk[:])
            w01_0 = wall[:, WCOL_W01_0:WCOL_W01_0 + H]            # [128,64]
            w10_0 = wall[0:H, WCOL_W10_0:WCOL_W10_0 + H]          # [64,64]
            w01_1 = wall[0:H, WCOL_W01_1:WCOL_W01_1 + H]
            w10_1 = wall[0:H, WCOL_W10_1:WCOL_W10_1 + H]
            bb01_0 = pp.tile([H, 1], F32, tag="bb01_0")
            nc.vector.tensor_copy(out=bb01_0[:],
                                  in_=wall[0:H, WCOL_B01_0:WCOL_B01_0 + 1])
            bb10_0 = pp.tile([H, 1], F32, tag="bb10_0")
            nc.vector.tensor_copy(out=bb10_0[:],
                                  in_=wall[0:H, WCOL_B10_0:WCOL_B10_0 + 1])
            bb01_1 = pp.tile([H, 1], F32, tag="bb01_1")
            nc.vector.tensor_copy(out=bb01_1[:],
                                  in_=wall[0:H, WCOL_B01_1:WCOL_B01_1 + 1])
            bb10_1 = pp.tile([H, 1], F32, tag="bb10_1")
            nc.vector.tensor_copy(out=bb10_1[:],
                                  in_=wall[0:H, WCOL_B10_1:WCOL_B10_1 + 1])
            wout = pp.tile([H, 1], F32, tag="wout")
            nc.vector.tensor_copy(out=wout[:],
                                  in_=wall[0:H, WCOL_WOUT:WCOL_WOUT + 1])
            bbout = pp.tile([1, 1], F32, tag="bbout")
            nc.vector.tensor_copy(out=bbout[:],
                                  in_=wall[0:1, WCOL_BOUT:WCOL_BOUT + 1])

            # ---- persistent small state ----
            deg_all = pp.tile([128, NVT], F32, tag="deg_all")
            vb_all = pp.tile([128, NVT], F32, tag="vb_all")
            s0b = pp.tile([128, NVT, H + 2], BF16, tag="s0b")
            s1b = pp.tile([128, NVT, H], BF16, tag="s1b")
            raB = pp.tile([H, nloc], F32, tag="raB")     # 1/alpha_denom bcast
            vbB = pp.tile([H, nloc], F32, tag="vbB")     # v_beta bcast (free)
            rbrow = dp.tile([1, EE], F32, tag="rbrow")   # 1/beta_denom (DRAM)
            earow = dp.tile([1, EE], F32, tag="earow")   # e_alpha (DRAM)
            earow_b = dp.tile([1, EE], BF16, tag="earow_b")

            # ====== P0: unpack bits -> bf16 B16 + row sums (node_deg) ======
            with tc.tile_pool(name="p0", bufs=2) as p0:
                for vt in range(NVT):
                    bt = p0.tile([128, EB], U8, tag="p0bits")
                    nc.sync.dma_start(
                        out=bt[:], in_=bits[vt * 128:(vt + 1) * 128, :])
                    ub = p0.tile([128, EE], BF16, tag="p0ub")
                    ubv = ub[:].rearrange("p (j t) -> p t j", t=8)
                    for t in range(8):
                        m = p0.tile([128, EB], U8, tag="p0m")
                        nc.vector.tensor_scalar(
                            out=m[:], in0=bt[:], scalar1=1 << t,
                            scalar2=None, op0=OP.bitwise_and)
                        nc.vector.tensor_scalar(
                            out=ubv[:, t, :], in0=m[:], scalar1=0,
                            scalar2=None, op0=OP.is_gt)
                    nc.vector.tensor_reduce(
                        out=deg_all[:, vt:vt + 1], in_=ub[:],
                        axis=mybir.AxisListType.X, op=OP.add)
                    nc.sync.dma_start(
                        out=B16[vt * 128:(vt + 1) * 128, :], in_=ub[:])

            # node_deg -> v_beta
            with tc.tile_pool(name="vbp", bufs=1) as vbp:
                degc = vbp.tile([128, NVT], F32, tag="degc")
                nc.vector.tensor_scalar_max(out=degc[:], in0=deg_all[:],
                                            scalar1=1.0)
                sqd = vbp.tile([128, NVT], F32, tag="sqd")
                nc.scalar.sqrt(out=sqd[:], in_=degc[:])
                nc.vector.reciprocal(out=vb_all[:], in_=sqd[:])
                # v_beta to free-layout DRAM row then broadcast into vbB
                with tc.tile_pool(name="vbps", bufs=1, space="PSUM") as vps:
                    pt = vps.tile([NVT, 128], F32, tag="vb_t")
                    nc.tensor.transpose(pt[:], vb_all[:], id_f32[:])
                    vb8 = vbp.tile([NVT, 128], F32, tag="vb8")
                    nc.vector.tensor_copy(out=vb8[:], in_=pt[:])
                vrow = dp.tile([1, nloc], F32, tag="vrow")
                nc.gpsimd.dma_start(
                    out=vrow[:].rearrange("a (b c) -> (a b) c", b=NVT),
                    in_=vb8[:])
                nc.gpsimd.dma_start(out=vbB[:],
                                    in_=vrow[:].to_broadcast([H, nloc]))

            # ================= S0 prep (single bf16) ====================
            with tc.tile_pool(name="s0p", bufs=2) as sp, \
                 tc.tile_pool(name="s0ps", bufs=2, space="PSUM") as sps:
                for vt in range(NVT):
                    xt = sp.tile([128, D], BF16, tag="xt")
                    nc.sync.dma_start(out=xt[:],
                                      in_=x0[vt * 128:(vt + 1) * 128, :])
                    pxt = sps.tile([D, 128], BF16, tag="pxt")
                    nc.tensor.transpose(pxt[:], xt[:], id_bf16[:])
                    x0T = sp.tile([D, 128], F32, tag="x0T")
                    nc.vector.tensor_copy(out=x0T[:], in_=pxt[:])
                    py = sps.tile([128, H], F32, tag="py")
                    nc.tensor.matmul(py[:], lhsT=x0T[:], rhs=w01_0,
                                     start=True, stop=True)
                    s0f = sp.tile([128, H + 2], F32, tag="s0f")
                    nc.vector.tensor_scalar_mul(out=s0f[:, 0:H], in0=py[:],
                                                scalar1=vb_all[:, vt:vt + 1])
                    nc.vector.tensor_copy(out=s0f[:, H:H + 1],
                                          in_=vb_all[:, vt:vt + 1])
                    nc.vector.memset(s0f[:, H + 1:H + 2], 1.0)
                    nc.vector.tensor_copy(out=s0b[:, vt, :], in_=s0f[:])

            # ================= PA: U0' = B^T S0 -> AllReduce ============
            bo0 = dp.tile([H + 2, EE], F32, tag="bo0")
            with tc.tile_pool(name="pa", bufs=2) as pa, \
                 tc.tile_pool(name="pa_acc", bufs=1) as paa, \
                 tc.tile_pool(name="paps", bufs=2, space="PSUM") as paps:
                u0acc = paa.tile([H + 2, EE], F32, tag="u0acc")
                for sup in range(NSUP):
                    bt = pa.tile([128, NVT, PASUP], BF16, tag="pa_bt")
                    nc.sync.dma_start(
                        out=bt[:],
                        in_=B16[:, sup * PASUP:(sup + 1) * PASUP].rearrange(
                            "(vt p) e -> p vt e", p=128))
                    pu = paps.tile([H + 2, PASUP], F32, tag="pa_pu")
                    for c in range(PASUP // 512):
                        for vt in range(NVT):
                            nc.tensor.matmul(
                                pu[:, c * 512:(c + 1) * 512],
                                lhsT=s0b[:, vt, :],
                                rhs=bt[:, vt, c * 512:(c + 1) * 512],
                                start=(vt == 0), stop=(vt == NVT - 1))
                    nc.vector.tensor_copy(
                        out=u0acc[:, sup * PASUP:(sup + 1) * PASUP],
                        in_=pu[:])
                bi0 = dp.tile([H + 2, EE], F32, tag="bi0")
                nc.sync.dma_start(out=bi0[:], in_=u0acc[:])
                nc.gpsimd.collective_compute(
                    "AllReduce", OP.add, replica_groups=GROUPS,
                    ins=[bi0.opt()], outs=[bo0.opt()])

            # ====== mid0: full-width edge stage on every core ===========
            z0t = pp.tile([H + 1, EE], BF16, tag="z0t")
            with tc.tile_pool(name="m0", bufs=1) as m0, \
                 tc.tile_pool(name="m0c", bufs=2) as m0c, \
                 tc.tile_pool(name="m0ps", bufs=2, space="PSUM") as m0ps:
                # 1/beta_denom (guard 0 -> 1), via folded layout
                bd128 = m0.tile([128, FW], F32, tag="bd128")
                nc.gpsimd.dma_start(
                    out=bd128[:],
                    in_=bo0[H:H + 1, :].rearrange("a (p c) -> (a p) c",
                                                  p=128))
                msk = m0.tile([128, FW], F32, tag="msk")
                nc.vector.tensor_scalar(out=msk[:], in0=bd128[:], scalar1=0.0,
                                        scalar2=None, op0=OP.is_equal)
                nc.vector.tensor_add(out=bd128[:], in0=bd128[:], in1=msk[:])
                rb128 = m0.tile([128, FW], F32, tag="rb128")
                nc.vector.reciprocal(out=rb128[:], in_=bd128[:])
                nc.gpsimd.dma_start(
                    out=rbrow[:].rearrange("a (p c) -> (a p) c", p=128),
                    in_=rb128[:])
                # e_alpha = ecard'^-1.5 (guard 0 -> 1)
                ec128 = m0.tile([128, FW], F32, tag="ec128")
                nc.gpsimd.dma_start(
                    out=ec128[:],
                    in_=bo0[H + 1:H + 2, :].rearrange("a (p c) -> (a p) c",
                                                      p=128))
                nc.vector.tensor_scalar_max(out=ec128[:], in0=ec128[:],
                                            scalar1=1.0)
                sq = m0.tile([128, FW], F32, tag="sq")
                nc.scalar.sqrt(out=sq[:], in_=ec128[:])
                nc.vector.tensor_mul(out=sq[:], in0=sq[:], in1=ec128[:])
                ea128 = m0.tile([128, FW], F32, tag="ea128")
                nc.vector.reciprocal(out=ea128[:], in_=sq[:])
                nc.gpsimd.dma_start(
                    out=earow[:].rearrange("a (p c) -> (a p) c", p=128),
                    in_=ea128[:])
                eab128 = m0.tile([128, FW], BF16, tag="eab128")
                nc.vector.tensor_copy(out=eab128[:], in_=ea128[:])
                nc.gpsimd.dma_start(
                    out=earow_b[:].rearrange("a (p c) -> (a p) c", p=128),
                    in_=eab128[:])
                nc.sync.dma_start(out=z0t[H:H + 1, :], in_=earow_b[:])
                # x1 / Z0 chunks
                for ch in range(NMCH):
                    sl = slice(ch * MCH, (ch + 1) * MCH)
                    u0c = m0c.tile([H, MCH], F32, tag="u0c")
                    nc.sync.dma_start(out=u0c[:], in_=bo0[0:H, sl])
                    rbc = m0c.tile([H, MCH], F32, tag="rbc")
                    nc.gpsimd.dma_start(
                        out=rbc[:], in_=rbrow[:, sl].to_broadcast([H, MCH]))
                    eac = m0c.tile([H, MCH], F32, tag="eac")
                    nc.gpsimd.dma_start(
                        out=eac[:], in_=earow[:, sl].to_broadcast([H, MCH]))
                    xs = m0c.tile([H, MCH], F32, tag="xs")
                    nc.vector.tensor_mul(out=xs[:], in0=u0c[:], in1=rbc[:])
                    nc.scalar.activation(out=xs[:], in_=xs[:], func=AF.Relu,
                                         bias=bb01_0[:])
                    for c in range(MCH // 512):
                        zp = m0ps.tile([H, 512], F32, tag="zp")
                        nc.tensor.matmul(zp[:], lhsT=w10_0,
                                         rhs=xs[:, c * 512:(c + 1) * 512],
                                         start=True, stop=True)
                        nc.vector.tensor_mul(
                            out=z0t[0:H, ch * MCH + c * 512:
                                    ch * MCH + (c + 1) * 512],
                            in0=zp[:], in1=eac[:, c * 512:(c + 1) * 512])

            # ================= PB: V0'^T = Z0'^T B^T ====================
            with tc.tile_pool(name="pbz", bufs=1) as pbz, \
                 tc.tile_pool(name="pb", bufs=2) as pb, \
                 tc.tile_pool(name="pbps", bufs=1, space="PSUM") as pbps, \
                 tc.tile_pool(name="pbps2", bufs=2, space="PSUM") as pbps2:
                zst = pbz.tile([128, NET, H + 1], BF16, tag="zst")
                zstf = zst[:].rearrange("p n h -> p (n h)")
                with tc.For_i(0, NET) as ei:
                    stg = pbz.tile([H + 1, 128], BF16, tag="zstg")
                    nc.scalar.activation(out=stg[:],
                                         in_=z0t[:, bass.ts(ei, 128)],
                                         func=AF.Copy)
                    ptz = pbps2.tile([128, H + 1], BF16, tag="ptz")
                    nc.tensor.transpose(
                        ptz[:], stg[:], id_bf16[:H + 1, :H + 1])
                    nc.vector.tensor_copy(out=zstf[:, bass.ts(ei, H + 1)],
                                          in_=ptz[:])
                vp = pbps.tile([H + 1, nloc], F32, tag="vp")
                for sup in range(NTSUP):
                    btile = pb.tile([128, ETL, nloc], BF16, tag="pb_bt")
                    nc.sync.dma_start_transpose(
                        btile[:], B16[:, sup * TSUP:(sup + 1) * TSUP])
                    for etl in range(ETL):
                        et = sup * ETL + etl
                        for c in range(NCH):
                            nc.tensor.matmul(
                                vp[:, c * CW:(c + 1) * CW],
                                lhsT=zst[:, et, :],
                                rhs=btile[:, etl, c * CW:(c + 1) * CW],
                                start=(et == 0), stop=(et == NET - 1))
                # alpha_denom -> 1/ad broadcast ; x = relu(V0/ad + b10_0)
                with tc.tile_pool(name="pbs", bufs=1) as pbs:
                    adm = pbs.tile([1, nloc], F32, tag="adm")
                    nc.vector.tensor_scalar(out=adm[:], in0=vp[H:H + 1, :],
                                            scalar1=0.0, scalar2=None,
                                            op0=OP.is_equal)
                    nc.vector.tensor_add(out=adm[:], in0=adm[:],
                                         in1=vp[H:H + 1, :])
                    ra = pbs.tile([1, nloc], F32, tag="ra")
                    nc.vector.reciprocal(out=ra[:], in_=adm[:])
                    rarow = dp.tile([1, nloc], F32, tag="rarow")
                    nc.gpsimd.dma_start(out=rarow[:], in_=ra[:])
                    nc.gpsimd.dma_start(out=raB[:],
                                        in_=rarow[:].to_broadcast([H, nloc]))
                    xl1 = pbs.tile([H, nloc], F32, tag="xl1")
                    nc.vector.tensor_mul(out=xl1[:], in0=vp[0:H, :],
                                         in1=raB[:])
                    nc.scalar.activation(out=xl1[:], in_=xl1[:], func=AF.Relu,
                                         bias=bb10_0[:])
                    # S1^T = (W01_1^T x^T) * v_beta
                    s1tb = pbs.tile([H, nloc], BF16, tag="s1tb")
                    for c in range(NCH):
                        yp = pbps2.tile([H, CW], F32, tag="yp")
                        nc.tensor.matmul(yp[:], lhsT=w01_1,
                                         rhs=xl1[:, c * CW:(c + 1) * CW],
                                         start=True, stop=True)
                        nc.vector.tensor_mul(
                            out=s1tb[:, c * CW:(c + 1) * CW], in0=yp[:],
                            in1=vbB[:, c * CW:(c + 1) * CW])
                    for vt in range(NVT):
                        pts = pbps2.tile([128, H], BF16, tag="pts")
                        nc.tensor.transpose(
                            pts[:], s1tb[:, vt * 128:(vt + 1) * 128],
                            id_bf16[:H, :H])
                        nc.vector.tensor_copy(out=s1b[:, vt, :], in_=pts[:])

            # ================= PC: U1' = B^T S1 -> AllReduce ============
            bo1 = dp.tile([H, EE], F32, tag="bo1")
            with tc.tile_pool(name="pc", bufs=2) as pc, \
                 tc.tile_pool(name="pc_acc", bufs=1) as pca, \
                 tc.tile_pool(name="pcps", bufs=2, space="PSUM") as pcps:
                u1acc = pca.tile([H, EE], F32, tag="u1acc")
                for sup in range(NSUP):
                    bt = pc.tile([128, NVT, PASUP], BF16, tag="pc_bt")
                    nc.sync.dma_start(
                        out=bt[:],
                        in_=B16[:, sup * PASUP:(sup + 1) * PASUP].rearrange(
                            "(vt p) e -> p vt e", p=128))
                    pu = pcps.tile([H, PASUP], F32, tag="pc_pu")
                    for c in range(PASUP // 512):
                        for vt in range(NVT):
                            nc.tensor.matmul(
                                pu[:, c * 512:(c + 1) * 512],
                                lhsT=s1b[:, vt, :],
                                rhs=bt[:, vt, c * 512:(c + 1) * 512],
                                start=(vt == 0), stop=(vt == NVT - 1))
                    nc.vector.tensor_copy(
                        out=u1acc[:, sup * PASUP:(sup + 1) * PASUP],
                        in_=pu[:])
                bi1 = dp.tile([H, EE], F32, tag="bi1")
                nc.sync.dma_start(out=bi1[:], in_=u1acc[:])
                nc.gpsimd.collective_compute(
                    "AllReduce", OP.add, replica_groups=GROUPS,
                    ins=[bi1.opt()], outs=[bo1.opt()])

            # ====== mid1: full-width edge stage on every core ===========
            z1t = pp.tile([H, EE], BF16, tag="z1t")
            with tc.tile_pool(name="m1c", bufs=2) as m1c, \
                 tc.tile_pool(name="m1ps", bufs=2, space="PSUM") as m1ps:
                for ch in range(NMCH):
                    sl = slice(ch * MCH, (ch + 1) * MCH)
                    u1c = m1c.tile([H, MCH], F32, tag="u1c")
                    nc.sync.dma_start(out=u1c[:], in_=bo1[0:H, sl])
                    rbc = m1c.tile([H, MCH], F32, tag="rbc1")
                    nc.gpsimd.dma_start(
                        out=rbc[:], in_=rbrow[:, sl].to_broadcast([H, MCH]))
                    eac = m1c.tile([H, MCH], F32, tag="eac1")
                    nc.gpsimd.dma_start(
                        out=eac[:], in_=earow[:, sl].to_broadcast([H, MCH]))
                    xs2 = m1c.tile([H, MCH], F32, tag="xs2")
                    nc.vector.tensor_mul(out=xs2[:], in0=u1c[:], in1=rbc[:])
                    nc.scalar.activation(out=xs2[:], in_=xs2[:], func=AF.Relu,
                                         bias=bb01_1[:])
                    for c in range(MCH // 512):
                        zp1 = m1ps.tile([H, 512], F32, tag="zp1")
                        nc.tensor.matmul(zp1[:], lhsT=w10_1,
                                         rhs=xs2[:, c * 512:(c + 1) * 512],
                                         start=True, stop=True)
                        nc.vector.tensor_mul(
                            out=z1t[:, ch * MCH + c * 512:
                                    ch * MCH + (c + 1) * 512],
                            in0=zp1[:], in1=eac[:, c * 512:(c + 1) * 512])

            # ================= PD: V1^T + finale ========================
            with tc.tile_pool(name="pdz", bufs=1) as pdz, \
                 tc.tile_pool(name="pd", bufs=2) as pd, \
                 tc.tile_pool(name="pdps", bufs=1, space="PSUM") as pdps, \
                 tc.tile_pool(name="pdps2", bufs=2, space="PSUM") as pdps2:
                z1st = pdz.tile([128, NET, H], BF16, tag="z1st")
                z1stf = z1st[:].rearrange("p n h -> p (n h)")
                with tc.For_i(0, NET) as ei:
                    stg1 = pdz.tile([H, 128], BF16, tag="z1stg")
                    nc.scalar.activation(out=stg1[:],
                                         in_=z1t[:, bass.ts(ei, 128)],
                                         func=AF.Copy)
                    ptz = pdps2.tile([128, H], BF16, tag="ptz1")
                    nc.tensor.transpose(
                        ptz[:], stg1[:], id_bf16[:H, :H])
                    nc.vector.tensor_copy(out=z1stf[:, bass.ts(ei, H)],
                                          in_=ptz[:])
                vp1 = pdps.tile([H, nloc], F32, tag="vp1")
                for sup in range(NTSUP):
                    btile = pd.tile([128, ETL, nloc], BF16, tag="pd_bt")
                    nc.sync.dma_start_transpose(
                        btile[:], B16[:, sup * TSUP:(sup + 1) * TSUP])
                    for etl in range(ETL):
                        et = sup * ETL + etl
                        for c in range(NCH):
                            nc.tensor.matmul(
                                vp1[:, c * CW:(c + 1) * CW],
                                lhsT=z1st[:, et, :],
                                rhs=btile[:, etl, c * CW:(c + 1) * CW],
                                start=(et == 0), stop=(et == NET - 1))
                with tc.tile_pool(name="fin", bufs=1) as fin:
                    x2 = fin.tile([H, nloc], F32, tag="x2")
                    nc.vector.tensor_mul(out=x2[:], in0=vp1[:], in1=raB[:])
                    nc.scalar.activation(out=x2[:], in_=x2[:], func=AF.Relu,
                                         bias=bb10_1[:])
                    pool_p = fin.tile([H, 1], F32, tag="pool_p")
                    nc.vector.tensor_reduce(out=pool_p[:], in_=x2[:],
                                            axis=mybir.AxisListType.X,
                                            op=OP.max)
                    bp = dp.tile([H, 1], F32, tag="bp")
                    nc.gpsimd.dma_start(out=bp[:], in_=pool_p[:])
                    bpo = dp.tile([H, 1], F32, tag="bpo")
                    nc.gpsimd.collective_compute(
                        "AllReduce", OP.max, replica_groups=GROUPS,
                        ins=[bp.opt()], outs=[bpo.opt()])
                    pooled = fin.tile([H, 1], F32, tag="pooled")
                    nc.gpsimd.dma_start(out=pooled[:], in_=bpo[:])
                    po = pdps2.tile([1, 1], F32, tag="po")
                    nc.tensor.matmul(po[:], lhsT=pooled[:], rhs=wout[:],
                                     start=True, stop=True)
                    ob = fin.tile([1, 1], F32, tag="ob")
                    nc.vector.tensor_add(out=ob[:], in0=po[:], in1=bbout[:])
                    nc.sync.dma_start(out=out[:], in_=ob[:])

    nc.compile()
    return nc


_NC_CACHE = {}


def _get_nc():
    if "nc" not in _NC_CACHE:
        _NC_CACHE["nc"] = build_kernel()
    return _NC_CACHE["nc"]


def _make_in_maps(inputs, ncores=NCORES, nloc=N // NCORES):
    from ml_dtypes import bfloat16
    x0 = np.asarray(inputs["x0"], np.float32).astype(bfloat16)
    inc = np.asarray(inputs["incidence"])
    bits = np.packbits(inc != 0, axis=1, bitorder="little")  # [N, E//8] u8
    wpack = np.zeros((128, WPACK_COLS), np.float32)
    wpack[:, WCOL_W01_0:WCOL_W01_0 + H] = np.asarray(inputs["W01_0"],
                                                     np.float32)
    wpack[0:H, WCOL_W10_0:WCOL_W10_0 + H] = np.asarray(inputs["W10_0"],
                                                       np.float32)
    wpack[0:H, WCOL_W01_1:WCOL_W01_1 + H] = np.asarray(inputs["W01_1"],
                                                       np.float32)
    wpack[0:H, WCOL_W10_1:WCOL_W10_1 + H] = np.asarray(inputs["W10_1"],
                                                       np.float32)
    wpack[0:H, WCOL_B01_0] = np.asarray(inputs["b01_0"],
                                        np.float32).reshape(-1)
    wpack[0:H, WCOL_B10_0] = np.asarray(inputs["b10_0"],
                                        np.float32).reshape(-1)
    wpack[0:H, WCOL_B01_1] = np.asarray(inputs["b01_1"],
                                        np.float32).reshape(-1)
    wpack[0:H, WCOL_B10_1] = np.asarray(inputs["b10_1"],
                                        np.float32).reshape(-1)
    wpack[0:H, WCOL_WOUT] = np.asarray(inputs["Wout"], np.float32).reshape(-1)
    wpack[0:1, WCOL_BOUT] = np.asarray(inputs["bout"], np.float32).reshape(-1)
    in_maps = []
    for c in range(ncores):
        m = {"x0": np.ascontiguousarray(x0[c * nloc:(c + 1) * nloc]),
             "bits": np.ascontiguousarray(bits[c * nloc:(c + 1) * nloc]),
             "wpack": wpack}
        in_maps.append(m)
    return in_maps


def kernel(**inputs) -> np.ndarray:
    nc = _get_nc()
    in_maps = _make_in_maps(inputs)
    res = run_bass_kernel_spmd(nc, in_maps, list(range(NCORES)))
    return res.results[0]["out"].reshape(1).astype(np.float32)


if __name__ == "__main__":
    pass


# revision 12
# speedup vs baseline: 13.4611x; 1.0275x over previous
"""HNHN hypergraph model on 8 Trainium2 NeuronCores (Bass/Tile), v8.

Wall-time-optimized: warm-run wall is dominated by host->device upload
through the axon tunnel (~105-150 MB/s) plus a ~190ms dispatch floor, so
the binary incidence matrix is bit-packed on host (512MB fp32 -> 16MB u8,
exact) and unpacked to bf16 on device; weights/biases ride in one small
array.  Device work is single-bf16 (tolerance 2e-2; measured pipeline
error ~1.3e-3) and each layer needs just ONE f32 AllReduce of the edge
features: every core then computes the (tiny) full-width edge-stage math
redundantly, which removes the ReduceScatter/AllGather pair and the
e-shard scatter/gather layout gymnastics.

Sharding: rows (nodes) of the incidence matrix and x0 over 8 cores.
Pipeline per core (v = own 1024 nodes, e = all 16384 hyperedges):
  P0   : unpack packed bits -> bf16 B16 tile (strided bit-planes)
         -> row sums (node_deg) -> DRAM scratch B16
  S0   : Y0 = x0 @ W01_0, S0 = [Y0*v_beta | v_beta | 1]  bf16
  PA   : U0' = B^T S0 partials (PSUM accum over own v) -> AllReduce(add)
         U0'[64] = beta_denom partials, U0'[65] = edge_card partials
  mid0 : full width on every core: x1 = relu(U0/beta_denom + b01_0)
         Z0' = [x1@W10_0 * e_alpha ; e_alpha]  bf16  (no collective)
  PB   : V0'^T = Z0'^T B^T via transposed-DMA reads of B16 (accum, all e)
         row 64 = alpha_denom;  x = relu(V0/alpha_denom + b10_0)
  PC   : U1' = B^T S1 partials -> AllReduce(add)
  mid1 : x1_l2 = relu(U1/beta_denom + b01_1); Z1' = x1_l2@W10_1 * e_alpha
  PD   : V1^T accum;  x2 = relu(V1/alpha_denom + b10_1)
  fin  : per-core max-pool -> AllReduce(max) -> pooled @ Wout + bout
"""

import numpy as np

import concourse.bass as bass
import concourse.bacc as bacc
import concourse.mybir as mybir
import concourse.tile as tile
from concourse.bass_utils import run_bass_kernel_spmd
from concourse.masks import make_identity

F32 = mybir.dt.float32
BF16 = mybir.dt.bfloat16
U8 = mybir.dt.uint8
AF = mybir.ActivationFunctionType
OP = mybir.AluOpType

N, E, D, H = 8192, 16384, 128, 64
NCORES = 8
# packed weight array column map (see _make_in_maps); all blocks start at
# partition row 0 so on-device slices never cross partition offsets
WCOL_W01_0 = 0            # [128, 64]   rows 0:128
WCOL_W10_0 = 64           # [64, 64]    rows 0:64
WCOL_W01_1 = 128          # [64, 64]    rows 0:64
WCOL_W10_1 = 192          # [64, 64]    rows 0:64
WCOL_B01_0 = 256          # [64, 1]
WCOL_B10_0 = 257          # [64, 1]
WCOL_B01_1 = 258          # [64, 1]
WCOL_B10_1 = 259          # [64, 1]
WCOL_WOUT = 260           # [64, 1]
WCOL_BOUT = 261           # [1, 1]
WPACK_COLS = 262


def build_kernel(ncores=NCORES, n_edges=E, nloc=N // NCORES):
    EE = n_edges
    EB = EE // 8                 # packed bytes per row
    NVT = nloc // 128            # v-tiles per core
    NET = EE // 128              # 128-wide e-tiles
    PASUP = min(2048, EE)        # PA/PC streaming super width
    NSUP = EE // PASUP
    TSUP = min(1024, EE)         # PB/PD transposed-read super width
    NTSUP = EE // TSUP
    ETL = TSUP // 128            # e-tiles per transposed read
    CW = min(512, nloc)          # column chunk for nloc-wide ops
    NCH = nloc // CW
    MCH = 2048                   # mid-phase e-chunk
    NMCH = EE // MCH
    FW = EE // 128               # fold width for full-width scalar math
    GROUPS = [list(range(ncores))]

    nc = bacc.Bacc("TRN2", target_bir_lowering=False, debug=False,
                   num_devices=ncores)

    x0 = nc.declare_dram_parameter("x0", [nloc, D], BF16, isOutput=False)
    bits = nc.declare_dram_parameter("bits", [nloc, EB], U8, isOutput=False)
    wpk = nc.declare_dram_parameter("wpack", [128, WPACK_COLS], F32,
                                    isOutput=False)
    out = nc.declare_dram_parameter("out", [1, 1], F32, isOutput=True)

    B16 = nc.dram_tensor("b16", [nloc, EE], BF16)

    with tile.TileContext(nc, num_cores=ncores) as tc:
        with tc.tile_pool(name="persist", bufs=1) as pp, \
             tc.tile_pool(name="dram", bufs=1, space="DRAM") as dp:
            # ---- constants / weights ----
            id_f32 = pp.tile([128, 128], F32, tag="id_f32")
            make_identity(nc, id_f32[:])
            id_bf16 = pp.tile([128, 128], BF16, tag="id_bf16")
            make_identity(nc, id_bf16[:])
            wall = pp.tile([128, WPACK_COLS], F32, tag="wall")
            nc.sync.dma_start(out=wall[:], in_=wpk[:])
            w01_0 = wall[:, WCOL_W01_0:WCOL_W01_0 + H]            # [128,64]
            w10_0 = wall[0:H, WCOL_W10_0:WCOL_W10_0 + H]          # [64,64]
            w01_1 = wall[0:H, WCOL_W01_1:WCOL_W01_1 + H]
            w10_1 = wall[0:H, WCOL_W10_1:WCOL_W10_1 + H]
            bb01_0 = pp.tile([H, 1], F32, tag="bb01_0")
            nc.vector.tensor_copy(out=bb01_0[:],
                                  in_=wall[0:H, WCOL_B01_0:WCOL_B01_0 + 1])
            bb10_0 = pp.tile([H, 1], F32, tag="bb10_0")
            nc.vector.tensor_copy(out=bb10_0[:],
                                  in_=wall[0:H, WCOL_B10_0:WCOL_B10_0 + 1])
            bb01_1 = pp.tile([H, 1], F32, tag="bb01_1")
            nc.vector.tensor_copy(out=bb01_1[:],
                                  in_=wall[0:H, WCOL_B01_1:WCOL_B01_1 + 1])
            bb10_1 = pp.tile([H, 1], F32, tag="bb10_1")
            nc.vector.tensor_copy(out=bb10_1[:],
                                  in_=wall[0:H, WCOL_B10_1:WCOL_B10_1 + 1])
            wout = pp.tile([H, 1], F32, tag="wout")
            nc.vector.tensor_copy(out=wout[:],
                                  in_=wall[0:H, WCOL_WOUT:WCOL_WOUT + 1])
            bbout = pp.tile([1, 1], F32, tag="bbout")
            nc.vector.tensor_copy(out=bbout[:],
                                  in_=wall[0:1, WCOL_BOUT:WCOL_BOUT + 1])

            # ---- persistent small state ----
            deg_all = pp.tile([128, NVT], F32, tag="deg_all")
            vb_all = pp.tile([128, NVT], F32, tag="vb_all")
            s0b = pp.tile([128, NVT, H + 2], BF16, tag="s0b")
            s1b = pp.tile([128, NVT, H], BF16, tag="s1b")
            raB = pp.tile([H, nloc], F32, tag="raB")     # 1/alpha_denom bcast
            vbB = pp.tile([H, nloc], F32, tag="vbB")     # v_beta bcast (free)
            rbrow = dp.tile([1, EE], F32, tag="rbrow")   # 1/beta_denom (DRAM)
            earow = dp.tile([1, EE], F32, tag="earow")   # e_alpha (DRAM)
            earow_b = dp.tile([1, EE], BF16, tag="earow_b")

            # ====== P0: unpack bits -> bf16 B16 + row sums (node_deg) ======
            with tc.tile_pool(name="p0", bufs=2) as p0:
                for vt in range(NVT):
                    bt = p0.tile([128, EB], U8, tag="p0bits")
                    nc.sync.dma_start(
                        out=bt[:], in_=bits[vt * 128:(vt + 1) * 128, :])
                    ub = p0.tile([128, EE], BF16, tag="p0ub")
                    ubv = ub[:].rearrange("p (j t) -> p t j", t=8)
                    for t in range(8):
                        m = p0.tile([128, EB], U8, tag="p0m")
                        nc.vector.tensor_scalar(
                            out=m[:], in0=bt[:], scalar1=1 << t,
                            scalar2=None, op0=OP.bitwise_and)
                        nc.vector.tensor_scalar(
                            out=ubv[:, t, :], in0=m[:], scalar1=0,
                            scalar2=None, op0=OP.is_gt)
                    nc.vector.tensor_reduce(
                        out=deg_all[:, vt:vt + 1], in_=ub[:],
                        axis=mybir.AxisListType.X, op=OP.add)
                    nc.sync.dma_start(
                        out=B16[vt * 128:(vt + 1) * 128, :], in_=ub[:])

            # node_deg -> v_beta
            with tc.tile_pool(name="vbp", bufs=1) as vbp:
                degc = vbp.tile([128, NVT], F32, tag="degc")
                nc.vector.tensor_scalar_max(out=degc[:], in0=deg_all[:],
                                            scalar1=1.0)
                sqd = vbp.tile([128, NVT], F32, tag="sqd")
                nc.scalar.sqrt(out=sqd[:], in_=degc[:])
                nc.vector.reciprocal(out=vb_all[:], in_=sqd[:])
                # v_beta to free-layout DRAM row then broadcast into vbB
                with tc.tile_pool(name="vbps", bufs=1, space="PSUM") as vps:
                    pt = vps.tile([NVT, 128], F32, tag="vb_t")
                    nc.tensor.transpose(pt[:], vb_all[:], id_f32[:])
                    vb8 = vbp.tile([NVT, 128], F32, tag="vb8")
                    nc.vector.tensor_copy(out=vb8[:], in_=pt[:])
                vrow = dp.tile([1, nloc], F32, tag="vrow")
                nc.gpsimd.dma_start(
                    out=vrow[:].rearrange("a (b c) -> (a b) c", b=NVT),
                    in_=vb8[:])
                nc.gpsimd.dma_start(out=vbB[:],
                                    in_=vrow[:].to_broadcast([H, nloc]))

            # ================= S0 prep (single bf16) ====================
            with tc.tile_pool(name="s0p", bufs=2) as sp, \
                 tc.tile_pool(name="s0ps", bufs=2, space="PSUM") as sps:
                for vt in range(NVT):
                    xt = sp.tile([128, D], BF16, tag="xt")
                    nc.sync.dma_start(out=xt[:],
                                      in_=x0[vt * 128:(vt + 1) * 128, :])
                    pxt = sps.tile([D, 128], BF16, tag="pxt")
                    nc.tensor.transpose(pxt[:], xt[:], id_bf16[:])
                    x0T = sp.tile([D, 128], F32, tag="x0T")
                    nc.vector.tensor_copy(out=x0T[:], in_=pxt[:])
                    py = sps.tile([128, H], F32, tag="py")
                    nc.tensor.matmul(py[:], lhsT=x0T[:], rhs=w01_0,
                                     start=True, stop=True)
                    s0f = sp.tile([128, H + 2], F32, tag="s0f")
                    nc.vector.tensor_scalar_mul(out=s0f[:, 0:H], in0=py[:],
                                                scalar1=vb_all[:, vt:vt + 1])
                    nc.vector.tensor_copy(out=s0f[:, H:H + 1],
                                          in_=vb_all[:, vt:vt + 1])
                    nc.vector.memset(s0f[:, H + 1:H + 2], 1.0)
                    nc.vector.tensor_copy(out=s0b[:, vt, :], in_=s0f[:])

            # ================= PA: U0' = B^T S0 -> AllReduce ============
            bo0 = dp.tile([H + 2, EE], F32, tag="bo0")
            with tc.tile_pool(name="pa", bufs=2) as pa, \
                 tc.tile_pool(name="pa_acc", bufs=1) as paa, \
                 tc.tile_pool(name="paps", bufs=2, space="PSUM") as paps:
                u0acc = paa.tile([H + 2, EE], F32, tag="u0acc")
                for sup in range(NSUP):
                    bt = pa.tile([128, NVT, PASUP], BF16, tag="pa_bt")
                    nc.sync.dma_start(
                        out=bt[:],
                        in_=B16[:, sup * PASUP:(sup + 1) * PASUP].rearrange(
                            "(vt p) e -> p vt e", p=128))
                    pu = paps.tile([H + 2, PASUP], F32, tag="pa_pu")
                    for c in range(PASUP // 512):
                        for vt in range(NVT):
                            nc.tensor.matmul(
                                pu[:, c * 512:(c + 1) * 512],
                                lhsT=s0b[:, vt, :],
                                rhs=bt[:, vt, c * 512:(c + 1) * 512],
                                start=(vt == 0), stop=(vt == NVT - 1))
                    nc.vector.tensor_copy(
                        out=u0acc[:, sup * PASUP:(sup + 1) * PASUP],
                        in_=pu[:])
                bi0 = dp.tile([H + 2, EE], F32, tag="bi0")
                nc.sync.dma_start(out=bi0[:], in_=u0acc[:])
                nc.gpsimd.collective_compute(
                    "AllReduce", OP.add, replica_groups=GROUPS,
                    ins=[bi0.opt()], outs=[bo0.opt()])

            # ====== mid0: full-width edge stage on every core ===========
            z0t = pp.tile([H + 1, EE], BF16, tag="z0t")
            with tc.tile_pool(name="m0", bufs=1) as m0, \
                 tc.tile_pool(name="m0c", bufs=2) as m0c, \
                 tc.tile_pool(name="m0ps", bufs=2, space="PSUM") as m0ps:
                # 1/beta_denom (guard 0 -> 1), via folded layout
                bd128 = m0.tile([128, FW], F32, tag="bd128")
                nc.gpsimd.dma_start(
                    out=bd128[:],
                    in_=bo0[H:H + 1, :].rearrange("a (p c) -> (a p) c",
                                                  p=128))
                msk = m0.tile([128, FW], F32, tag="msk")
                nc.vector.tensor_scalar(out=msk[:], in0=bd128[:], scalar1=0.0,
                                        scalar2=None, op0=OP.is_equal)
                nc.vector.tensor_add(out=bd128[:], in0=bd128[:], in1=msk[:])
                rb128 = m0.tile([128, FW], F32, tag="rb128")
                nc.vector.reciprocal(out=rb128[:], in_=bd128[:])
                nc.gpsimd.dma_start(
                    out=rbrow[:].rearrange("a (p c) -> (a p) c", p=128),
                    in_=rb128[:])
                # e_alpha = ecard'^-1.5 (guard 0 -> 1)
                ec128 = m0.tile([128, FW], F32, tag="ec128")
                nc.gpsimd.dma_start(
                    out=ec128[:],
                    in_=bo0[H + 1:H + 2, :].rearrange("a (p c) -> (a p) c",
                                                      p=128))
                nc.vector.tensor_scalar_max(out=ec128[:], in0=ec128[:],
                                            scalar1=1.0)
                sq = m0.tile([128, FW], F32, tag="sq")
                nc.scalar.sqrt(out=sq[:], in_=ec128[:])
                nc.vector.tensor_mul(out=sq[:], in0=sq[:], in1=ec128[:])
                ea128 = m0.tile([128, FW], F32, tag="ea128")
                nc.vector.reciprocal(out=ea128[:], in_=sq[:])
                nc.gpsimd.dma_start(
                    out=earow[:].rearrange("a (p c) -> (a p) c", p=128),
                    in_=ea128[:])
                eab128 = m0.tile([128, FW], BF16, tag="eab128")
                nc.vector.tensor_copy(out=eab128[:], in_=ea128[:])
                nc.gpsimd.dma_start(
                    out=earow_b[:].rearrange("a (p c) -> (a p) c", p=128),
                    in_=eab128[:])
                nc.sync.dma_start(out=z0t[H:H + 1, :], in_=earow_b[:])
                # x1 / Z0 chunks
                for ch in range(NMCH):
                    sl = slice(ch * MCH, (ch + 1) * MCH)
                    u0c = m0c.tile([H, MCH], F32, tag="u0c")
                    nc.sync.dma_start(out=u0c[:], in_=bo0[0:H, sl])
                    rbc = m0c.tile([H, MCH], F32, tag="rbc")
                    nc.gpsimd.dma_start(
                        out=rbc[:], in_=rbrow[:, sl].to_broadcast([H, MCH]))
                    eac = m0c.tile([H, MCH], F32, tag="eac")
                    nc.gpsimd.dma_start(
                        out=eac[:], in_=earow[:, sl].to_broadcast([H, MCH]))
                    xs = m0c.tile([H, MCH], F32, tag="xs")
                    nc.vector.tensor_mul(out=xs[:], in0=u0c[:], in1=rbc[:])
                    nc.scalar.activation(out=xs[:], in_=xs[:], func=AF.Relu,
                                         bias=bb01_0[:])
                    for c in range(MCH // 512):
                        zp = m0ps.tile([H, 512], F32, tag="zp")
                        nc.tensor.matmul(zp[:], lhsT=w10_0,
                                         rhs=xs[:, c * 512:(c + 1) * 512],
                                         start=True, stop=True)
                        nc.vector.tensor_mul(
                            out=z0t[0:H, ch * MCH + c * 512:
                                    ch * MCH + (c + 1) * 512],
                            in0=zp[:], in1=eac[:, c * 512:(c + 1) * 512])

            # ================= PB: V0'^T = Z0'^T B^T ====================
            with tc.tile_pool(name="pbz", bufs=1) as pbz, \
                 tc.tile_pool(name="pb", bufs=2) as pb, \
                 tc.tile_pool(name="pbps", bufs=1, space="PSUM") as pbps, \
                 tc.tile_pool(name="pbps2", bufs=2, space="PSUM") as pbps2:
                zst = pbz.tile([128, NET, H + 1], BF16, tag="zst")
                zstf = zst[:].rearrange("p n h -> p (n h)")
                with tc.For_i(0, NET) as ei:
                    stg = pbz.tile([H + 1, 128], BF16, tag="zstg")
                    nc.scalar.activation(out=stg[:],
                                         in_=z0t[:, bass.ts(ei, 128)],
                                         func=AF.Copy)
                    ptz = pbps2.tile([128, H + 1], BF16, tag="ptz")
                    nc.tensor.transpose(
                        ptz[:], stg[:], id_bf16[:H + 1, :H + 1])
                    nc.vector.tensor_copy(out=zstf[:, bass.ts(ei, H + 1)],
                                          in_=ptz[:])
                vp = pbps.tile([H + 1, nloc], F32, tag="vp")

                def _pb_sup(sup_first, sup_last, btile, zsrc, base):
                    for etl in range(ETL):
                        for c in range(NCH):
                            nc.tensor.matmul(
                                vp[:, c * CW:(c + 1) * CW],
                                lhsT=zsrc[:, base + etl, :],
                                rhs=btile[:, etl, c * CW:(c + 1) * CW],
                                start=(sup_first and etl == 0),
                                stop=(sup_last and etl == ETL - 1))

                for sup in (0, NTSUP - 1):
                    btile = pb.tile([128, ETL, nloc], BF16, tag="pb_bt")
                    nc.sync.dma_start_transpose(
                        btile[:], B16[:, sup * TSUP:(sup + 1) * TSUP])
                    _pb_sup(sup == 0, sup == NTSUP - 1, btile, zst,
                            sup * ETL)
                with tc.For_i(1, NTSUP - 1) as si:
                    btile = pb.tile([128, ETL, nloc], BF16, tag="pb_btl")
                    nc.sync.dma_start_transpose(
                        btile[:],
                        B16[:].rearrange("v e -> v e")[:, bass.ts(si, TSUP)])
                    zsg = pbz.tile([128, ETL, H + 1], BF16, tag="zsg")
                    nc.scalar.activation(
                        out=zsg[:].rearrange("p n h -> p (n h)"),
                        in_=zst[:].rearrange("p n h -> p (n h)")[
                            :, bass.ts(si, ETL * (H + 1))],
                        func=AF.Copy)
                    _pb_sup(False, False, btile, zsg, 0)
                # alpha_denom -> 1/ad broadcast ; x = relu(V0/ad + b10_0)
                with tc.tile_pool(name="pbs", bufs=1) as pbs:
                    adm = pbs.tile([1, nloc], F32, tag="adm")
                    nc.vector.tensor_scalar(out=adm[:], in0=vp[H:H + 1, :],
                                            scalar1=0.0, scalar2=None,
                                            op0=OP.is_equal)
                    nc.vector.tensor_add(out=adm[:], in0=adm[:],
                                         in1=vp[H:H + 1, :])
                    ra = pbs.tile([1, nloc], F32, tag="ra")
                    nc.vector.reciprocal(out=ra[:], in_=adm[:])
                    rarow = dp.tile([1, nloc], F32, tag="rarow")
                    nc.gpsimd.dma_start(out=rarow[:], in_=ra[:])
                    nc.gpsimd.dma_start(out=raB[:],
                                        in_=rarow[:].to_broadcast([H, nloc]))
                    xl1 = pbs.tile([H, nloc], F32, tag="xl1")
                    nc.vector.tensor_mul(out=xl1[:], in0=vp[0:H, :],
                                         in1=raB[:])
                    nc.scalar.activation(out=xl1[:], in_=xl1[:], func=AF.Relu,
                                         bias=bb10_0[:])
                    # S1^T = (W01_1^T x^T) * v_beta
                    s1tb = pbs.tile([H, nloc], BF16, tag="s1tb")
                    for c in range(NCH):
                        yp = pbps2.tile([H, CW], F32, tag="yp")
                        nc.tensor.matmul(yp[:], lhsT=w01_1,
                                         rhs=xl1[:, c * CW:(c + 1) * CW],
                                         start=True, stop=True)
                        nc.vector.tensor_mul(
                            out=s1tb[:, c * CW:(c + 1) * CW], in0=yp[:],
                            in1=vbB[:, c * CW:(c + 1) * CW])
                    for vt in range(NVT):
                        pts = pbps2.tile([128, H], BF16, tag="pts")
                        nc.tensor.transpose(
                            pts[:], s1tb[:, vt * 128:(vt + 1) * 128],
                            id_bf16[:H, :H])
                        nc.vector.tensor_copy(out=s1b[:, vt, :], in_=pts[:])

            # ================= PC: U1' = B^T S1 -> AllReduce ============
            bo1 = dp.tile([H, EE], F32, tag="bo1")
            with tc.tile_pool(name="pc", bufs=2) as pc, \
                 tc.tile_pool(name="pc_acc", bufs=1) as pca, \
                 tc.tile_pool(name="pcps", bufs=2, space="PSUM") as pcps:
                u1acc = pca.tile([H, EE], F32, tag="u1acc")
                for sup in range(NSUP):
                    bt = pc.tile([128, NVT, PASUP], BF16, tag="pc_bt")
                    nc.sync.dma_start(
                        out=bt[:],
                        in_=B16[:, sup * PASUP:(sup + 1) * PASUP].rearrange(
                            "(vt p) e -> p vt e", p=128))
                    pu = pcps.tile([H, PASUP], F32, tag="pc_pu")
                    for c in range(PASUP // 512):
                        for vt in range(NVT):
                            nc.tensor.matmul(
                                pu[:, c * 512:(c + 1) * 512],
                                lhsT=s1b[:, vt, :],
                                rhs=bt[:, vt, c * 512:(c + 1) * 512],
                                start=(vt == 0), stop=(vt == NVT - 1))
                    nc.vector.tensor_copy(
                        out=u1acc[:, sup * PASUP:(sup + 1) * PASUP],
                        in_=pu[:])
                bi1 = dp.tile([H, EE], F32, tag="bi1")
                nc.sync.dma_start(out=bi1[:], in_=u1acc[:])
                nc.gpsimd.collective_compute(
                    "AllReduce", OP.add, replica_groups=GROUPS,
                    ins=[bi1.opt()], outs=[bo1.opt()])

            # ====== mid1: full-width edge stage on every core ===========
            z1t = pp.tile([H, EE], BF16, tag="z1t")
            with tc.tile_pool(name="m1c", bufs=2) as m1c, \
                 tc.tile_pool(name="m1ps", bufs=2, space="PSUM") as m1ps:
                for ch in range(NMCH):
                    sl = slice(ch * MCH, (ch + 1) * MCH)
                    u1c = m1c.tile([H, MCH], F32, tag="u1c")
                    nc.sync.dma_start(out=u1c[:], in_=bo1[0:H, sl])
                    rbc = m1c.tile([H, MCH], F32, tag="rbc1")
                    nc.gpsimd.dma_start(
                        out=rbc[:], in_=rbrow[:, sl].to_broadcast([H, MCH]))
                    eac = m1c.tile([H, MCH], F32, tag="eac1")
                    nc.gpsimd.dma_start(
                        out=eac[:], in_=earow[:, sl].to_broadcast([H, MCH]))
                    xs2 = m1c.tile([H, MCH], F32, tag="xs2")
                    nc.vector.tensor_mul(out=xs2[:], in0=u1c[:], in1=rbc[:])
                    nc.scalar.activation(out=xs2[:], in_=xs2[:], func=AF.Relu,
                                         bias=bb01_1[:])
                    for c in range(MCH // 512):
                        zp1 = m1ps.tile([H, 512], F32, tag="zp1")
                        nc.tensor.matmul(zp1[:], lhsT=w10_1,
                                         rhs=xs2[:, c * 512:(c + 1) * 512],
                                         start=True, stop=True)
                        nc.vector.tensor_mul(
                            out=z1t[:, ch * MCH + c * 512:
                                    ch * MCH + (c + 1) * 512],
                            in0=zp1[:], in1=eac[:, c * 512:(c + 1) * 512])

            # ================= PD: V1^T + finale ========================
            with tc.tile_pool(name="pdz", bufs=1) as pdz, \
                 tc.tile_pool(name="pd", bufs=2) as pd, \
                 tc.tile_pool(name="pdps", bufs=1, space="PSUM") as pdps, \
                 tc.tile_pool(name="pdps2", bufs=2, space="PSUM") as pdps2:
                z1st = pdz.tile([128, NET, H], BF16, tag="z1st")
                z1stf = z1st[:].rearrange("p n h -> p (n h)")
                with tc.For_i(0, NET) as ei:
                    stg1 = pdz.tile([H, 128], BF16, tag="z1stg")
                    nc.scalar.activation(out=stg1[:],
                                         in_=z1t[:, bass.ts(ei, 128)],
                                         func=AF.Copy)
                    ptz = pdps2.tile([128, H], BF16, tag="ptz1")
                    nc.tensor.transpose(
                        ptz[:], stg1[:], id_bf16[:H, :H])
                    nc.vector.tensor_copy(out=z1stf[:, bass.ts(ei, H)],
                                          in_=ptz[:])
                vp1 = pdps.tile([H, nloc], F32, tag="vp1")

                def _pd_sup(sup_first, sup_last, btile, zsrc, base):
                    for etl in range(ETL):
                        for c in range(NCH):
                            nc.tensor.matmul(
                                vp1[:, c * CW:(c + 1) * CW],
                                lhsT=zsrc[:, base + etl, :],
                                rhs=btile[:, etl, c * CW:(c + 1) * CW],
                                start=(sup_first and etl == 0),
                                stop=(sup_last and etl == ETL - 1))

                for sup in (0, NTSUP - 1):
                    btile = pd.tile([128, ETL, nloc], BF16, tag="pd_bt")
                    nc.sync.dma_start_transpose(
                        btile[:], B16[:, sup * TSUP:(sup + 1) * TSUP])
                    _pd_sup(sup == 0, sup == NTSUP - 1, btile, z1st,
                            sup * ETL)
                with tc.For_i(1, NTSUP - 1) as si:
                    btile = pd.tile([128, ETL, nloc], BF16, tag="pd_btl")
                    nc.sync.dma_start_transpose(
                        btile[:],
                        B16[:].rearrange("v e -> v e")[:, bass.ts(si, TSUP)])
                    zsg1 = pdz.tile([128, ETL, H], BF16, tag="zsg1")
                    nc.scalar.activation(
                        out=zsg1[:].rearrange("p n h -> p (n h)"),
                        in_=z1st[:].rearrange("p n h -> p (n h)")[
                            :, bass.ts(si, ETL * H)],
                        func=AF.Copy)
                    _pd_sup(False, False, btile, zsg1, 0)
                with tc.tile_pool(name="fin", bufs=1) as fin:
                    x2 = fin.tile([H, nloc], F32, tag="x2")
                    nc.vector.tensor_mul(out=x2[:], in0=vp1[:], in1=raB[:])
                    nc.scalar.activation(out=x2[:], in_=x2[:], func=AF.Relu,
                                         bias=bb10_1[:])
                    pool_p = fin.tile([H, 1], F32, tag="pool_p")
                    nc.vector.tensor_reduce(out=pool_p[:], in_=x2[:],
                                            axis=mybir.AxisListType.X,
                                            op=OP.max)
                    bp = dp.tile([H, 1], F32, tag="bp")
                    nc.gpsimd.dma_start(out=bp[:], in_=pool_p[:])
                    bpo = dp.tile([H, 1], F32, tag="bpo")
                    nc.gpsimd.collective_compute(
                        "AllReduce", OP.max, replica_groups=GROUPS,
                        ins=[bp.opt()], outs=[bpo.opt()])
                    pooled = fin.tile([H, 1], F32, tag="pooled")
                    nc.gpsimd.dma_start(out=pooled[:], in_=bpo[:])
                    po = pdps2.tile([1, 1], F32, tag="po")
                    nc.tensor.matmul(po[:], lhsT=pooled[:], rhs=wout[:],
                                     start=True, stop=True)
                    ob = fin.tile([1, 1], F32, tag="ob")
                    nc.vector.tensor_add(out=ob[:], in0=po[:], in1=bbout[:])
                    nc.sync.dma_start(out=out[:], in_=ob[:])

    nc.compile()
    return nc


_NC_CACHE = {}


def _get_nc():
    if "nc" not in _NC_CACHE:
        _NC_CACHE["nc"] = build_kernel()
    return _NC_CACHE["nc"]


def _make_in_maps(inputs, ncores=NCORES, nloc=N // NCORES):
    from ml_dtypes import bfloat16
    x0 = np.asarray(inputs["x0"], np.float32).astype(bfloat16)
    inc = np.asarray(inputs["incidence"])
    bits = np.packbits(inc != 0, axis=1, bitorder="little")  # [N, E//8] u8
    wpack = np.zeros((128, WPACK_COLS), np.float32)
    wpack[:, WCOL_W01_0:WCOL_W01_0 + H] = np.asarray(inputs["W01_0"],
                                                     np.float32)
    wpack[0:H, WCOL_W10_0:WCOL_W10_0 + H] = np.asarray(inputs["W10_0"],
                                                       np.float32)
    wpack[0:H, WCOL_W01_1:WCOL_W01_1 + H] = np.asarray(inputs["W01_1"],
                                                       np.float32)
    wpack[0:H, WCOL_W10_1:WCOL_W10_1 + H] = np.asarray(inputs["W10_1"],
                                                       np.float32)
    wpack[0:H, WCOL_B01_0] = np.asarray(inputs["b01_0"],
                                        np.float32).reshape(-1)
    wpack[0:H, WCOL_B10_0] = np.asarray(inputs["b10_0"],
                                        np.float32).reshape(-1)
    wpack[0:H, WCOL_B01_1] = np.asarray(inputs["b01_1"],
                                        np.float32).reshape(-1)
    wpack[0:H, WCOL_B10_1] = np.asarray(inputs["b10_1"],
                                        np.float32).reshape(-1)
    wpack[0:H, WCOL_WOUT] = np.asarray(inputs["Wout"], np.float32).reshape(-1)
    wpack[0:1, WCOL_BOUT] = np.asarray(inputs["bout"], np.float32).reshape(-1)
    in_maps = []
    for c in range(ncores):
        m = {"x0": np.ascontiguousarray(x0[c * nloc:(c + 1) * nloc]),
             "bits": np.ascontiguousarray(bits[c * nloc:(c + 1) * nloc]),
             "wpack": wpack}
        in_maps.append(m)
    return in_maps


def kernel(**inputs) -> np.ndarray:
    nc = _get_nc()
    in_maps = _make_in_maps(inputs)
    res = run_bass_kernel_spmd(nc, in_maps, list(range(NCORES)))
    return res.results[0]["out"].reshape(1).astype(np.float32)


if __name__ == "__main__":
    pass
